# revision 9
# baseline (speedup 1.0000x reference)
"""Fused transformer-block kernel for TRN2, 8-way data parallel over batch.

Layout strategy per core (128 sequences of 96 tokens = 12288 tokens):
  - Residual stream kept in N-layout [token_part, feature_free]; LayerNorm
    stats are free-dim reductions.
  - LN outputs written as bf16 and transposed to feature-major T-layout
    [feature_part, token_free] via DMA-xbar transposes; these feed the QKV
    and MLP1 matmuls (bf16).
  - Attention computed per (seq, head) with T=96 <= 128: scores in [t, s]
    layout (softmax over free dim), exp without max-subtraction (scores are
    bounded for this problem scale), 0/1 causal mask multiply, probs
    transposed on the PE, then attn@V gives head outputs directly in
    T-layout.
  - proj and MLP2 run in float32r (full PE speed at N=512, ~1e-4 rel err).
  - gamma/beta of both LNs and all biases are folded into the weight
    matrices / bias vectors on the host (exact algebra, see fold()).
"""

import sys

sys.path.insert(0, "/opt/trn_rl_repo")

from contextlib import ExitStack

import ml_dtypes
import numpy as np

import concourse.bass as bass  # noqa: F401  (registers AP types)
import concourse.tile as tile
from concourse import bacc, bass_utils, mybir

# Cache walrus-compiled NEFFs on disk keyed by BIR hash: re-running an
# unchanged program skips the multi-minute backend compile.
try:
    import hashlib
    import os as _os
    import shutil as _shutil

    import concourse.bass2jax as _b2j

    _orig_cbk = _b2j.compile_bir_kernel

    def _cached_cbk(bir_json, tmpdir, neff_name="file.neff"):
        try:
            raw = bir_json if isinstance(bir_json, bytes) else bir_json.encode()
            h = hashlib.sha256(raw).hexdigest()[:24]
            cdir = "/tmp/neff_cache"
            _os.makedirs(cdir, exist_ok=True)
            cpath = _os.path.join(cdir, h + ".neff")
            if _os.path.exists(cpath):
                return cpath
        except Exception:
            return _orig_cbk(bir_json, tmpdir, neff_name)
        p = _orig_cbk(bir_json, tmpdir, neff_name)
        try:
            _shutil.copy(p, cpath)
        except Exception:
            pass
        return p

    if _orig_cbk.__name__ != "_cached_cbk":
        _b2j.compile_bir_kernel = _cached_cbk
except Exception:
    pass

B, T, C = 1024, 96, 512
H, D = 4, 128
F = 4 * C
EPS = 1e-5
SCALE = D**-0.5

NCORES = 8
SEQ_PER_CORE = B // NCORES  # 128
S = SEQ_PER_CORE * T  # 12288 tokens per core
NB = 4  # sequences per block
TOK = NB * T  # 384 tokens per block
NBLK = SEQ_PER_CORE // NB  # 32 blocks
TCH = TOK // 128  # 3 token chunks per block
KC = C // 128  # 4 feature chunks of C
FM = F // 128  # 16 feature chunks of F

F32 = mybir.dt.float32
F32R = mybir.dt.float32r
BF16 = mybir.dt.bfloat16
AF = mybir.ActivationFunctionType
OP = mybir.AluOpType


def build(nblk=NBLK, has_bq=False, has_bk=False, has_bv=False, has_bp=False,
          has_b2=False):
    nc = bacc.Bacc("TRN2", target_bir_lowering=False, debug=False)

    def din(name, shape, dt):
        return nc.dram_tensor(name, shape, dt, kind="ExternalInput").ap()

    x_d = din("x", [S, C], F32)
    wq_d = din("wq", [C, C], BF16)
    wk_d = din("wk", [C, C], BF16)
    wv_d = din("wv", [C, C], BF16)
    wp_d = din("wp", [C, C], F32R)
    w1_d = din("w1", [C, F], BF16)
    w2_d = din("w2", [F, C], F32R)
    b1_d = din("b1", [F], F32)
    mask_d = din("mask", [T, T], BF16)
    ident_d = din("ident", [128, 128], BF16)
    bq_d = din("bq", [C], F32) if has_bq else None
    bk_d = din("bk", [C], F32) if has_bk else None
    bv_d = din("bv_b", [T, C], F32) if has_bv else None
    bp_d = din("bp_b", [128, C], F32) if has_bp else None
    b2_d = din("b2_b", [128, C], F32) if has_b2 else None
    y_d = nc.dram_tensor("y", [S, C], F32, kind="ExternalOutput").ap()

    with tile.TileContext(nc) as tc, ExitStack() as ctx:
        wp = ctx.enter_context(tc.tile_pool(name="wpool", bufs=1))
        ap_ = ctx.enter_context(tc.tile_pool(name="act", bufs=2))
        st = ctx.enter_context(tc.tile_pool(name="stat", bufs=3))
        hp = ctx.enter_context(tc.tile_pool(name="ht", bufs=1))
        ps = ctx.enter_context(tc.tile_pool(name="psum", bufs=1, space="PSUM"))

        # ---- resident weights ----
        def wload(name, d_ap, kchunks, fdim, dt):
            t = wp.tile([128, kchunks, fdim], dt, tag=name)
            nc.sync.dma_start(t[:], d_ap.rearrange("(kc p) f -> p kc f", p=128))
            return t

        wq_sb = wload("wq", wq_d, KC, C, BF16)
        wk_sb = wload("wk", wk_d, KC, C, BF16)
        wv_sb = wload("wv", wv_d, KC, C, BF16)
        wp_sb = wload("wp", wp_d, KC, C, F32R)
        w1_sb = wload("w1", w1_d, KC, F, BF16)
        w2_sb = wload("w2", w2_d, FM, C, F32R)

        b1_sb = wp.tile([128, FM], F32, tag="b1")
        nc.sync.dma_start(b1_sb[:], b1_d.rearrange("(fm p) -> p fm", p=128))
        mask_sb = wp.tile([T, T], BF16, tag="mask")
        nc.sync.dma_start(mask_sb[:], mask_d)
        ident_sb = wp.tile([128, 128], BF16, tag="ident")
        nc.sync.dma_start(ident_sb[:], ident_d)
        eps_sb = wp.tile([128, 1], F32, tag="eps")
        nc.vector.memset(eps_sb[:], EPS)
        if has_bq:
            bq_sb = wp.tile([128, H], F32, tag="bq")
            nc.sync.dma_start(bq_sb[:], bq_d.rearrange("(h d) -> d h", d=128))
        if has_bk:
            bk_sb = wp.tile([128, H], F32, tag="bk")
            nc.sync.dma_start(bk_sb[:], bk_d.rearrange("(h d) -> d h", d=128))
        if has_bv:
            bv_sb = wp.tile([T, C], F32, tag="bv")
            nc.sync.dma_start(bv_sb[:], bv_d)
        if has_bp:
            bp_sb = wp.tile([128, C], F32, tag="bp")
            nc.sync.dma_start(bp_sb[:], bp_d)
        if has_b2:
            b2_sb = wp.tile([128, C], F32, tag="b2")
            nc.sync.dma_start(b2_sb[:], b2_d)

        # ---- per-block helpers ----
        def layer_norm(src, pref):
            """src: [128, TCH, C] f32 -> xn bf16 [128, TCH, C]."""
            sums = st.tile([128, TCH], F32, tag=pref + "sums")
            nc.vector.tensor_reduce(sums[:], src[:], axis=mybir.AxisListType.X,
                                    op=OP.add)
            sumsq = st.tile([128, TCH], F32, tag=pref + "sumsq")
            for i in range(TCH):
                scr = st.tile([128, C], BF16, tag="scr")
                nc.vector.scalar_tensor_tensor(
                    scr[:], src[:, i, :], 1.0, src[:, i, :], OP.mult, OP.mult,
                    accum_out=sumsq[:, i : i + 1])
            mu = st.tile([128, TCH], F32, tag=pref + "mu")
            nc.vector.tensor_scalar_mul(mu[:], sums[:], 1.0 / C)
            msq = st.tile([128, TCH], F32, tag=pref + "msq")
            nc.vector.tensor_mul(out=msq[:], in0=mu[:], in1=mu[:])
            var = st.tile([128, TCH], F32, tag=pref + "var")
            nc.vector.scalar_tensor_tensor(var[:], sumsq[:], 1.0 / C, msq[:],
                                           OP.mult, OP.subtract)
            std = st.tile([128, TCH], F32, tag=pref + "std")
            nc.scalar.activation(std[:], var[:], AF.Sqrt, bias=eps_sb[:, 0:1])
            rstd = st.tile([128, TCH], F32, tag=pref + "rstd")
            nc.vector.reciprocal(rstd[:], std[:])
            nmr = st.tile([128, TCH], F32, tag=pref + "nmr")
            nc.vector.scalar_tensor_tensor(nmr[:], mu[:], -1.0, rstd[:],
                                           OP.mult, OP.mult)
            xn = ap_.tile([128, TCH, C], BF16, tag=pref + "xn")
            for i in range(TCH):
                nc.vector.scalar_tensor_tensor(
                    xn[:, i, :], src[:, i, :], rstd[:, i : i + 1],
                    nmr[:, i : i + 1].to_broadcast([128, C]), OP.mult, OP.add)
            return xn

        def transpose_xn(xn, pref):
            xnT = ap_.tile([128, KC, TOK], BF16, tag=pref + "xnT")
            for kc in range(KC):
                for mc in range(TCH):
                    nc.sync.dma_start_transpose(
                        out=xnT[:, kc, mc * 128 : (mc + 1) * 128],
                        in_=xn[:, mc, kc * 128 : (kc + 1) * 128])
            return xnT

        # ---- block loop ----
        for blk in range(nblk):
            row0 = blk * TOK
            x_sb = ap_.tile([128, TCH, C], F32, tag="x")
            nc.sync.dma_start(
                x_sb[:],
                x_d[row0 : row0 + TOK, :].rearrange("(ch p) c -> p ch c", p=128))

            xn = layer_norm(x_sb, "a")
            xnT = transpose_xn(xn, "a")

            # QKV projections (bf16)
            qt = ap_.tile([128, H, TOK], BF16, tag="qt")
            kt = ap_.tile([128, H, TOK], BF16, tag="kt")
            for dst, w_sb, bias_sb in ((qt, wq_sb, bq_sb if has_bq else None),
                                       (kt, wk_sb, bk_sb if has_bk else None)):
                for h in range(H):
                    p = ps.tile([128, TOK], F32, tag="qkv", bufs=2)
                    for kc in range(KC):
                        nc.tensor.matmul(p[:], w_sb[:, kc, h * 128 : (h + 1) * 128],
                                         xnT[:, kc, :], start=(kc == 0),
                                         stop=(kc == KC - 1))
                    if bias_sb is not None:
                        nc.scalar.activation(dst[:, h, :], p[:], AF.Identity,
                                             bias=bias_sb[:, h : h + 1])
                    else:
                        nc.vector.tensor_copy(out=dst[:, h, :], in_=p[:])
            vt = ap_.tile([T, NB, C], BF16, tag="vt")
            for b in range(NB):
                p = ps.tile([T, C], F32, tag="qkv", bufs=2)
                for kc in range(KC):
                    nc.tensor.matmul(p[:], xnT[:, kc, b * T : (b + 1) * T],
                                     wv_sb[:, kc, :], start=(kc == 0),
                                     stop=(kc == KC - 1))
                if has_bv:
                    nc.vector.tensor_add(out=vt[:, b, :], in0=p[:], in1=bv_sb[:])
                else:
                    nc.vector.tensor_copy(out=vt[:, b, :], in_=p[:])

            # attention: scores [t, s] per (h, b), exp, mask, row-normalize
            ee = ap_.tile([T, H * NB, T], BF16, tag="ee")
            for h in range(H):
                p = ps.tile([T, NB, T], F32, tag="sc", bufs=1)
                for b in range(NB):
                    nc.tensor.matmul(p[:, b, :], qt[:, h, b * T : (b + 1) * T],
                                     kt[:, h, b * T : (b + 1) * T],
                                     start=True, stop=True)
                nc.scalar.activation(ee[:, h * NB : (h + 1) * NB, :], p[:],
                                     AF.Exp, scale=SCALE)
            nc.vector.tensor_mul(
                out=ee[:], in0=ee[:],
                in1=mask_sb[:].unsqueeze(1).to_broadcast([T, H * NB, T]))
            dsum = st.tile([T, H * NB], F32, tag="dsum")
            nc.vector.tensor_reduce(dsum[:], ee[:], axis=mybir.AxisListType.X,
                                    op=OP.add)
            rr = st.tile([T, H * NB], F32, tag="rr")
            nc.vector.reciprocal(rr[:], dsum[:])
            nc.vector.tensor_mul(
                out=ee[:], in0=ee[:],
                in1=rr[:].unsqueeze(2).to_broadcast([T, H * NB, T]))

            # transpose probs on PE, then attn @ V -> OT (T-layout, f32r)
            pt = ap_.tile([T, H * NB, T], BF16, tag="pt")
            for h in range(H):
                p = ps.tile([T, NB, T], BF16, tag="psb", bufs=1)
                for b in range(NB):
                    nc.tensor.transpose(p[:, b, :], ee[:, h * NB + b, :],
                                        ident_sb[:T, :T])
                nc.vector.tensor_copy(out=pt[:, h * NB : (h + 1) * NB, :], in_=p[:])
            ot = ap_.tile([128, H, TOK], F32R, tag="ot")
            for h in range(H):
                p = ps.tile([128, NB, T], F32, tag="ot", bufs=1)
                for b in range(NB):
                    nc.tensor.matmul(p[:, b, :], vt[:, b, h * 128 : (h + 1) * 128],
                                     pt[:, h * NB + b, :], start=True, stop=True)
                nc.scalar.activation(ot[:, h, :], p[:], AF.Identity)

            # proj (f32r) + residual
            x2 = ap_.tile([128, TCH, C], F32, tag="x2")
            for mc in range(TCH):
                p = ps.tile([128, C], F32, tag="pm", bufs=1)
                for kc in range(H):
                    nc.tensor.matmul(p[:], ot[:, kc, mc * 128 : (mc + 1) * 128],
                                     wp_sb[:, kc, :], start=(kc == 0),
                                     stop=(kc == H - 1))
                if has_bp:
                    nc.vector.tensor_add(out=p[:], in0=p[:], in1=bp_sb[:])
                nc.vector.tensor_add(out=x2[:, mc, :], in0=p[:],
                                     in1=x_sb[:, mc, :])

            # MLP
            xn2 = layer_norm(x2, "b")
            xn2T = transpose_xn(xn2, "b")
            ht = hp.tile([128, FM, TOK], F32R, tag="ht")
            for fm in range(FM):
                p = ps.tile([128, TOK], F32, tag="m1", bufs=2)
                for kc in range(KC):
                    nc.tensor.matmul(p[:], w1_sb[:, kc, fm * 128 : (fm + 1) * 128],
                                     xn2T[:, kc, :], start=(kc == 0),
                                     stop=(kc == KC - 1))
                nc.scalar.activation(ht[:, fm, :], p[:], AF.Gelu,
                                     bias=b1_sb[:, fm : fm + 1])
            xo = ap_.tile([128, TCH, C], F32, tag="xo")
            for mc in range(TCH):
                p = ps.tile([128, C], F32, tag="pm", bufs=1)
                for fk in range(FM):
                    nc.tensor.matmul(p[:], ht[:, fk, mc * 128 : (mc + 1) * 128],
                                     w2_sb[:, fk, :], start=(fk == 0),
                                     stop=(fk == FM - 1))
                if has_b2:
                    nc.vector.tensor_add(out=p[:], in0=p[:], in1=b2_sb[:])
                nc.vector.tensor_add(out=xo[:, mc, :], in0=p[:],
                                     in1=x2[:, mc, :])
            nc.sync.dma_start(
                y_d[row0 : row0 + TOK, :].rearrange("(ch p) c -> p ch c", p=128),
                xo[:])

    nc.compile()
    return nc


def fold(inputs):
    """Host-side exact folding of LN affines and biases into weights.

    Returns dict of staged arrays for the device program + bias flags.
    """
    f32 = np.float32
    g1 = inputs["g1"].astype(f32)
    be1 = inputs["be1"].astype(f32)
    g2 = inputs["g2"].astype(f32)
    be2 = inputs["be2"].astype(f32)

    def headcat(w):  # [H, C, D] -> [C, H*D]
        return np.concatenate([w[h] for h in range(H)], axis=1)

    wq = headcat(np.asarray(inputs["wq"], f32))
    wk = headcat(np.asarray(inputs["wk"], f32))
    wv = headcat(np.asarray(inputs["wv"], f32))
    wp_ = np.asarray(inputs["w_proj"], f32)
    w1 = np.asarray(inputs["w1"], f32)
    w2 = np.asarray(inputs["w2"], f32)

    wq_f = g1[:, None] * wq
    wk_f = g1[:, None] * wk
    wv_f = g1[:, None] * wv
    bq = be1 @ wq
    bk = be1 @ wk
    bv = be1 @ wv
    bp = np.asarray(inputs["b_proj"], f32)
    w1_f = g2[:, None] * w1
    b1 = np.asarray(inputs["b1"], f32) + be2 @ w1
    b2 = np.asarray(inputs["b2"], f32)

    mask = np.tril(np.ones((T, T), np.float32)).astype(ml_dtypes.bfloat16)
    ident = np.eye(128, dtype=ml_dtypes.bfloat16)

    staged = {
        "wq": wq_f.astype(ml_dtypes.bfloat16),
        "wk": wk_f.astype(ml_dtypes.bfloat16),
        "wv": wv_f.astype(ml_dtypes.bfloat16),
        "wp": wp_.astype(f32),
        "w1": w1_f.astype(ml_dtypes.bfloat16),
        "w2": w2.astype(f32),
        "b1": b1,
        "mask": mask,
        "ident": ident,
    }
    flags = {
        "has_bq": bool(np.any(bq)),
        "has_bk": bool(np.any(bk)),
        "has_bv": bool(np.any(bv)),
        "has_bp": bool(np.any(bp)),
        "has_b2": bool(np.any(b2)),
    }
    if flags["has_bq"]:
        staged["bq"] = bq
    if flags["has_bk"]:
        staged["bk"] = bk
    if flags["has_bv"]:
        staged["bv_b"] = np.broadcast_to(bv, (T, C)).copy()
    if flags["has_bp"]:
        staged["bp_b"] = np.broadcast_to(bp, (128, C)).copy()
    if flags["has_b2"]:
        staged["b2_b"] = np.broadcast_to(b2, (128, C)).copy()
    return staged, flags


_CACHE = {}


def kernel(**inputs):
    staged, flags = fold(inputs)
    key = tuple(sorted(flags.items()))
    if key not in _CACHE:
        _CACHE[key] = build(**flags)
    nc = _CACHE[key]

    x = np.asarray(inputs["x"], np.float32).reshape(B, T * C)
    in_maps = []
    for c in range(NCORES):
        m = dict(staged)
        m["x"] = x[c * SEQ_PER_CORE : (c + 1) * SEQ_PER_CORE].reshape(S, C)
        in_maps.append(m)

    res = bass_utils.run_bass_kernel_spmd(nc, in_maps, core_ids=list(range(NCORES)))
    out = np.concatenate([r["y"] for r in res.results], axis=0)
    return out.reshape(B, T, C).astype(np.float32)


# revision 11
# speedup vs baseline: 1.2710x; 1.2710x over previous
"""Fused transformer-block kernel for TRN2, 8-way data parallel over batch.

Layout strategy per core (128 sequences of 96 tokens = 12288 tokens):
  - Residual stream kept in N-layout [token_part, feature_free]; LayerNorm
    stats are free-dim reductions.
  - LN outputs written as bf16 and transposed to feature-major T-layout
    [feature_part, token_free] via DMA-xbar transposes; these feed the QKV
    and MLP1 matmuls (bf16).
  - Attention computed per (seq, head) with T=96 <= 128: scores in [t, s]
    layout (softmax over free dim), exp without max-subtraction (scores are
    bounded for this problem scale), 0/1 causal mask multiply, probs
    transposed on the PE, then attn@V gives head outputs directly in
    T-layout.
  - proj and MLP2 run in float32r (full PE speed at N=512, ~1e-4 rel err).
  - gamma/beta of both LNs and all biases are folded into the weight
    matrices / bias vectors on the host (exact algebra, see fold()).
"""

import sys

sys.path.insert(0, "/opt/trn_rl_repo")

from contextlib import ExitStack

import ml_dtypes
import numpy as np

import concourse.bass as bass  # noqa: F401  (registers AP types)
import concourse.tile as tile
from concourse import bacc, bass_utils, mybir

# Cache walrus-compiled NEFFs on disk keyed by BIR hash: re-running an
# unchanged program skips the multi-minute backend compile.
try:
    import hashlib
    import os as _os
    import shutil as _shutil

    import concourse.bass2jax as _b2j

    _orig_cbk = _b2j.compile_bir_kernel

    def _cached_cbk(bir_json, tmpdir, neff_name="file.neff"):
        try:
            raw = bir_json if isinstance(bir_json, bytes) else bir_json.encode()
            h = hashlib.sha256(raw).hexdigest()[:24]
            cdir = "/tmp/neff_cache"
            _os.makedirs(cdir, exist_ok=True)
            cpath = _os.path.join(cdir, h + ".neff")
            if _os.path.exists(cpath):
                return cpath
        except Exception:
            return _orig_cbk(bir_json, tmpdir, neff_name)
        p = _orig_cbk(bir_json, tmpdir, neff_name)
        try:
            _shutil.copy(p, cpath)
        except Exception:
            pass
        return p

    if _orig_cbk.__name__ != "_cached_cbk":
        _b2j.compile_bir_kernel = _cached_cbk
except Exception:
    pass

B, T, C = 1024, 96, 512
H, D = 4, 128
F = 4 * C
EPS = 1e-5
SCALE = D**-0.5

NCORES = 8
SEQ_PER_CORE = B // NCORES  # 128
S = SEQ_PER_CORE * T  # 12288 tokens per core
NB = 4  # sequences per block
TOK = NB * T  # 384 tokens per block
NBLK = SEQ_PER_CORE // NB  # 32 blocks
TCH = TOK // 128  # 3 token chunks per block
KC = C // 128  # 4 feature chunks of C
FM = F // 128  # 16 feature chunks of F

F32 = mybir.dt.float32
F32R = mybir.dt.float32r
BF16 = mybir.dt.bfloat16
AF = mybir.ActivationFunctionType
OP = mybir.AluOpType


def build(nblk=NBLK, has_bq=False, has_bk=False, has_bv=False, has_bp=False,
          has_b2=False):
    nc = bacc.Bacc("TRN2", target_bir_lowering=False, debug=False)

    def din(name, shape, dt):
        return nc.dram_tensor(name, shape, dt, kind="ExternalInput").ap()

    x_d = din("x", [S, C], F32)
    wq_d = din("wq", [C, C], BF16)
    wk_d = din("wk", [C, C], BF16)
    wv_d = din("wv", [C, C], BF16)
    wp_d = din("wp", [C, C], F32R)
    w1_d = din("w1", [C, F], BF16)
    w2_d = din("w2", [F, C], F32R)
    b1_d = din("b1", [F], F32)
    mask_d = din("mask", [T, T], BF16)
    ident_d = din("ident", [128, 128], BF16)
    bq_d = din("bq", [C], F32) if has_bq else None
    bk_d = din("bk", [C], F32) if has_bk else None
    bv_d = din("bv_b", [T, C], F32) if has_bv else None
    bp_d = din("bp_b", [128, C], F32) if has_bp else None
    b2_d = din("b2_b", [128, C], F32) if has_b2 else None
    y_d = nc.dram_tensor("y", [S, C], F32, kind="ExternalOutput").ap()

    with tile.TileContext(nc) as tc, ExitStack() as ctx:
        wp = ctx.enter_context(tc.tile_pool(name="wpool", bufs=1))
        ap_ = ctx.enter_context(tc.tile_pool(name="act", bufs=2))
        st = ctx.enter_context(tc.tile_pool(name="stat", bufs=3))
        hp = ctx.enter_context(tc.tile_pool(name="ht", bufs=1))
        ps = ctx.enter_context(tc.tile_pool(name="psum", bufs=1, space="PSUM"))

        # ---- resident weights ----
        def wload(name, d_ap, kchunks, fdim, dt):
            t = wp.tile([128, kchunks, fdim], dt, tag=name)
            nc.sync.dma_start(t[:], d_ap.rearrange("(kc p) f -> p kc f", p=128))
            return t

        wq_sb = wload("wq", wq_d, KC, C, BF16)
        wk_sb = wload("wk", wk_d, KC, C, BF16)
        wv_sb = wload("wv", wv_d, KC, C, BF16)
        wp_sb = wload("wp", wp_d, KC, C, F32R)
        w1_sb = wload("w1", w1_d, KC, F, BF16)
        w2_sb = wload("w2", w2_d, FM, C, F32R)

        b1_sb = wp.tile([128, FM], F32, tag="b1")
        nc.sync.dma_start(b1_sb[:], b1_d.rearrange("(fm p) -> p fm", p=128))
        mask_sb = wp.tile([T, T], BF16, tag="mask")
        nc.sync.dma_start(mask_sb[:], mask_d)
        ident_sb = wp.tile([128, 128], BF16, tag="ident")
        nc.sync.dma_start(ident_sb[:], ident_d)
        eps_sb = wp.tile([128, 1], F32, tag="eps")
        nc.vector.memset(eps_sb[:], EPS)
        if has_bq:
            bq_sb = wp.tile([128, H], F32, tag="bq")
            nc.sync.dma_start(bq_sb[:], bq_d.rearrange("(h d) -> d h", d=128))
        if has_bk:
            bk_sb = wp.tile([128, H], F32, tag="bk")
            nc.sync.dma_start(bk_sb[:], bk_d.rearrange("(h d) -> d h", d=128))
        if has_bv:
            bv_sb = wp.tile([T, C], F32, tag="bv")
            nc.sync.dma_start(bv_sb[:], bv_d)
        if has_bp:
            bp_sb = wp.tile([128, C], F32, tag="bp")
            nc.sync.dma_start(bp_sb[:], bp_d)
        if has_b2:
            b2_sb = wp.tile([128, C], F32, tag="b2")
            nc.sync.dma_start(b2_sb[:], b2_d)

        # ---- per-block helpers ----
        def layer_norm(src, pref):
            """src: [128, TCH, C] f32 -> xn bf16 [128, TCH, C]."""
            sums = st.tile([128, TCH], F32, tag=pref + "sums")
            nc.vector.tensor_reduce(sums[:], src[:], axis=mybir.AxisListType.X,
                                    op=OP.add)
            sumsq = st.tile([128, TCH], F32, tag=pref + "sumsq")
            for i in range(TCH):
                scr = st.tile([128, C], BF16, tag="scr")
                nc.vector.scalar_tensor_tensor(
                    scr[:], src[:, i, :], 1.0, src[:, i, :], OP.mult, OP.mult,
                    accum_out=sumsq[:, i : i + 1])
            mu = st.tile([128, TCH], F32, tag=pref + "mu")
            nc.vector.tensor_scalar_mul(mu[:], sums[:], 1.0 / C)
            msq = st.tile([128, TCH], F32, tag=pref + "msq")
            nc.vector.tensor_mul(out=msq[:], in0=mu[:], in1=mu[:])
            var = st.tile([128, TCH], F32, tag=pref + "var")
            nc.vector.scalar_tensor_tensor(var[:], sumsq[:], 1.0 / C, msq[:],
                                           OP.mult, OP.subtract)
            std = st.tile([128, TCH], F32, tag=pref + "std")
            nc.scalar.activation(std[:], var[:], AF.Sqrt, bias=eps_sb[:, 0:1])
            rstd = st.tile([128, TCH], F32, tag=pref + "rstd")
            nc.vector.reciprocal(rstd[:], std[:])
            nmr = st.tile([128, TCH], F32, tag=pref + "nmr")
            nc.vector.scalar_tensor_tensor(nmr[:], mu[:], -1.0, rstd[:],
                                           OP.mult, OP.mult)
            xn = ap_.tile([128, TCH, C], BF16, tag=pref + "xn")
            for i in range(TCH):
                nc.vector.scalar_tensor_tensor(
                    xn[:, i, :], src[:, i, :], rstd[:, i : i + 1],
                    nmr[:, i : i + 1].to_broadcast([128, C]), OP.mult, OP.add)
            return xn

        def transpose_xn(xn, pref):
            xnT = ap_.tile([128, KC, TOK], BF16, tag=pref + "xnT")
            for kc in range(KC):
                for mc in range(TCH):
                    nc.sync.dma_start_transpose(
                        out=xnT[:, kc, mc * 128 : (mc + 1) * 128],
                        in_=xn[:, mc, kc * 128 : (kc + 1) * 128])
            return xnT

        # ---- block loop ----
        for blk in range(nblk):
            row0 = blk * TOK
            x_sb = ap_.tile([128, TCH, C], F32, tag="x")
            nc.sync.dma_start(
                x_sb[:],
                x_d[row0 : row0 + TOK, :].rearrange("(ch p) c -> p ch c", p=128))

            xn = layer_norm(x_sb, "a")
            xnT = transpose_xn(xn, "a")

            # QKV projections (bf16)
            qt = ap_.tile([128, H, TOK], BF16, tag="qt")
            kt = ap_.tile([128, H, TOK], BF16, tag="kt")
            for dst, w_sb, bias_sb in ((qt, wq_sb, bq_sb if has_bq else None),
                                       (kt, wk_sb, bk_sb if has_bk else None)):
                for h in range(H):
                    p = ps.tile([128, TOK], F32, tag="ps", bufs=8)
                    for kc in range(KC):
                        nc.tensor.matmul(p[:], w_sb[:, kc, h * 128 : (h + 1) * 128],
                                         xnT[:, kc, :], start=(kc == 0),
                                         stop=(kc == KC - 1))
                    if bias_sb is not None:
                        nc.scalar.activation(dst[:, h, :], p[:], AF.Identity,
                                             bias=bias_sb[:, h : h + 1])
                    else:
                        nc.vector.tensor_copy(out=dst[:, h, :], in_=p[:])
            vt = ap_.tile([T, NB, C], BF16, tag="vt")
            for b in range(NB):
                p = ps.tile([T, C], F32, tag="ps", bufs=8)
                for kc in range(KC):
                    nc.tensor.matmul(p[:], xnT[:, kc, b * T : (b + 1) * T],
                                     wv_sb[:, kc, :], start=(kc == 0),
                                     stop=(kc == KC - 1))
                if has_bv:
                    nc.vector.tensor_add(out=vt[:, b, :], in0=p[:], in1=bv_sb[:])
                else:
                    nc.vector.tensor_copy(out=vt[:, b, :], in_=p[:])

            # attention: scores [t, s] per (h, b), exp, mask, row-normalize
            ee = ap_.tile([T, H * NB, T], BF16, tag="ee")
            for h in range(H):
                p = ps.tile([T, NB, T], F32, tag="ps", bufs=8)
                for b in range(NB):
                    nc.tensor.matmul(p[:, b, :], qt[:, h, b * T : (b + 1) * T],
                                     kt[:, h, b * T : (b + 1) * T],
                                     start=True, stop=True)
                nc.scalar.activation(ee[:, h * NB : (h + 1) * NB, :], p[:],
                                     AF.Exp, scale=SCALE)
            nc.vector.tensor_mul(
                out=ee[:], in0=ee[:],
                in1=mask_sb[:].unsqueeze(1).to_broadcast([T, H * NB, T]))
            dsum = st.tile([T, H * NB], F32, tag="dsum")
            nc.vector.tensor_reduce(dsum[:], ee[:], axis=mybir.AxisListType.X,
                                    op=OP.add)
            rr = st.tile([T, H * NB], F32, tag="rr")
            nc.vector.reciprocal(rr[:], dsum[:])
            nc.vector.tensor_mul(
                out=ee[:], in0=ee[:],
                in1=rr[:].unsqueeze(2).to_broadcast([T, H * NB, T]))

            # transpose probs on PE, then attn @ V -> OT (T-layout, f32r)
            pt = ap_.tile([T, H * NB, T], BF16, tag="pt")
            for h in range(H):
                p = ps.tile([T, NB, T], BF16, tag="ps", bufs=8)
                for b in range(NB):
                    nc.tensor.transpose(p[:, b, :], ee[:, h * NB + b, :],
                                        ident_sb[:T, :T])
                nc.vector.tensor_copy(out=pt[:, h * NB : (h + 1) * NB, :], in_=p[:])
            ot = ap_.tile([128, H, TOK], F32R, tag="ot")
            for h in range(H):
                p = ps.tile([128, NB, T], F32, tag="ps", bufs=8)
                for b in range(NB):
                    nc.tensor.matmul(p[:, b, :], vt[:, b, h * 128 : (h + 1) * 128],
                                     pt[:, h * NB + b, :], start=True, stop=True)
                nc.scalar.activation(ot[:, h, :], p[:], AF.Identity)

            # proj (f32r) + residual
            x2 = ap_.tile([128, TCH, C], F32, tag="x2")
            for mc in range(TCH):
                p = ps.tile([128, C], F32, tag="ps", bufs=8)
                for kc in range(H):
                    nc.tensor.matmul(p[:], ot[:, kc, mc * 128 : (mc + 1) * 128],
                                     wp_sb[:, kc, :], start=(kc == 0),
                                     stop=(kc == H - 1))
                if has_bp:
                    nc.vector.tensor_add(out=p[:], in0=p[:], in1=bp_sb[:])
                nc.vector.tensor_add(out=x2[:, mc, :], in0=p[:],
                                     in1=x_sb[:, mc, :])

            # MLP
            xn2 = layer_norm(x2, "b")
            xn2T = transpose_xn(xn2, "b")
            ht = hp.tile([128, FM, TOK], F32R, tag="ht")
            for fm in range(FM):
                p = ps.tile([128, TOK], F32, tag="ps", bufs=8)
                for kc in range(KC):
                    nc.tensor.matmul(p[:], w1_sb[:, kc, fm * 128 : (fm + 1) * 128],
                                     xn2T[:, kc, :], start=(kc == 0),
                                     stop=(kc == KC - 1))
                nc.scalar.activation(ht[:, fm, :], p[:], AF.Gelu,
                                     bias=b1_sb[:, fm : fm + 1])
            xo = ap_.tile([128, TCH, C], F32, tag="xo")
            for mc in range(TCH):
                p = ps.tile([128, C], F32, tag="ps", bufs=8)
                for fk in range(FM):
                    nc.tensor.matmul(p[:], ht[:, fk, mc * 128 : (mc + 1) * 128],
                                     w2_sb[:, fk, :], start=(fk == 0),
                                     stop=(fk == FM - 1))
                if has_b2:
                    nc.vector.tensor_add(out=p[:], in0=p[:], in1=b2_sb[:])
                nc.vector.tensor_add(out=xo[:, mc, :], in0=p[:],
                                     in1=x2[:, mc, :])
            nc.sync.dma_start(
                y_d[row0 : row0 + TOK, :].rearrange("(ch p) c -> p ch c", p=128),
                xo[:])

    nc.compile()
    return nc


def fold(inputs):
    """Host-side exact folding of LN affines and biases into weights.

    Returns dict of staged arrays for the device program + bias flags.
    """
    f32 = np.float32
    g1 = inputs["g1"].astype(f32)
    be1 = inputs["be1"].astype(f32)
    g2 = inputs["g2"].astype(f32)
    be2 = inputs["be2"].astype(f32)

    def headcat(w):  # [H, C, D] -> [C, H*D]
        return np.concatenate([w[h] for h in range(H)], axis=1)

    wq = headcat(np.asarray(inputs["wq"], f32))
    wk = headcat(np.asarray(inputs["wk"], f32))
    wv = headcat(np.asarray(inputs["wv"], f32))
    wp_ = np.asarray(inputs["w_proj"], f32)
    w1 = np.asarray(inputs["w1"], f32)
    w2 = np.asarray(inputs["w2"], f32)

    wq_f = g1[:, None] * wq
    wk_f = g1[:, None] * wk
    wv_f = g1[:, None] * wv
    bq = be1 @ wq
    bk = be1 @ wk
    bv = be1 @ wv
    bp = np.asarray(inputs["b_proj"], f32)
    w1_f = g2[:, None] * w1
    b1 = np.asarray(inputs["b1"], f32) + be2 @ w1
    b2 = np.asarray(inputs["b2"], f32)

    mask = np.tril(np.ones((T, T), np.float32)).astype(ml_dtypes.bfloat16)
    ident = np.eye(128, dtype=ml_dtypes.bfloat16)

    staged = {
        "wq": wq_f.astype(ml_dtypes.bfloat16),
        "wk": wk_f.astype(ml_dtypes.bfloat16),
        "wv": wv_f.astype(ml_dtypes.bfloat16),
        "wp": wp_.astype(f32),
        "w1": w1_f.astype(ml_dtypes.bfloat16),
        "w2": w2.astype(f32),
        "b1": b1,
        "mask": mask,
        "ident": ident,
    }
    flags = {
        "has_bq": bool(np.any(bq)),
        "has_bk": bool(np.any(bk)),
        "has_bv": bool(np.any(bv)),
        "has_bp": bool(np.any(bp)),
        "has_b2": bool(np.any(b2)),
    }
    if flags["has_bq"]:
        staged["bq"] = bq
    if flags["has_bk"]:
        staged["bk"] = bk
    if flags["has_bv"]:
        staged["bv_b"] = np.broadcast_to(bv, (T, C)).copy()
    if flags["has_bp"]:
        staged["bp_b"] = np.broadcast_to(bp, (128, C)).copy()
    if flags["has_b2"]:
        staged["b2_b"] = np.broadcast_to(b2, (128, C)).copy()
    return staged, flags


_CACHE = {}


def kernel(**inputs):
    staged, flags = fold(inputs)
    key = tuple(sorted(flags.items()))
    if key not in _CACHE:
        _CACHE[key] = build(**flags)
    nc = _CACHE[key]

    x = np.asarray(inputs["x"], np.float32).reshape(B, T * C)
    in_maps = []
    for c in range(NCORES):
        m = dict(staged)
        m["x"] = x[c * SEQ_PER_CORE : (c + 1) * SEQ_PER_CORE].reshape(S, C)
        in_maps.append(m)

    res = bass_utils.run_bass_kernel_spmd(nc, in_maps, core_ids=list(range(NCORES)))
    out = np.concatenate([r["y"] for r in res.results], axis=0)
    return out.reshape(B, T, C).astype(np.float32)


# revision 12
# speedup vs baseline: 1.5803x; 1.2434x over previous
"""Fused transformer-block kernel for TRN2, 8-way data parallel over batch.

Layout strategy per core (128 sequences of 96 tokens = 12288 tokens):
  - Residual stream kept in N-layout [token_part, feature_free]; LayerNorm
    stats are free-dim reductions.
  - LN outputs written as bf16 and transposed to feature-major T-layout
    [feature_part, token_free] via DMA-xbar transposes; these feed the QKV
    and MLP1 matmuls (bf16).
  - Attention computed per (seq, head) with T=96 <= 128: scores in [t, s]
    layout (softmax over free dim), exp without max-subtraction (scores are
    bounded for this problem scale), 0/1 causal mask multiply, probs
    transposed on the PE, then attn@V gives head outputs directly in
    T-layout.
  - proj and MLP2 run in float32r (full PE speed at N=512, ~1e-4 rel err).
  - gamma/beta of both LNs and all biases are folded into the weight
    matrices / bias vectors on the host (exact algebra, see fold()).
"""

import sys

sys.path.insert(0, "/opt/trn_rl_repo")

from contextlib import ExitStack

import ml_dtypes
import numpy as np

import concourse.bass as bass  # noqa: F401  (registers AP types)
import concourse.tile as tile
from concourse import bacc, bass_utils, mybir

# Cache walrus-compiled NEFFs on disk keyed by BIR hash: re-running an
# unchanged program skips the multi-minute backend compile.
try:
    import hashlib
    import os as _os
    import shutil as _shutil

    import concourse.bass2jax as _b2j

    _orig_cbk = _b2j.compile_bir_kernel

    def _cached_cbk(bir_json, tmpdir, neff_name="file.neff"):
        try:
            raw = bir_json if isinstance(bir_json, bytes) else bir_json.encode()
            h = hashlib.sha256(raw).hexdigest()[:24]
            cdir = "/tmp/neff_cache"
            _os.makedirs(cdir, exist_ok=True)
            cpath = _os.path.join(cdir, h + ".neff")
            if _os.path.exists(cpath):
                return cpath
        except Exception:
            return _orig_cbk(bir_json, tmpdir, neff_name)
        p = _orig_cbk(bir_json, tmpdir, neff_name)
        try:
            _shutil.copy(p, cpath)
        except Exception:
            pass
        return p

    if _orig_cbk.__name__ != "_cached_cbk":
        _b2j.compile_bir_kernel = _cached_cbk
except Exception:
    pass

B, T, C = 1024, 96, 512
H, D = 4, 128
F = 4 * C
EPS = 1e-5
SCALE = D**-0.5

NCORES = 8
SEQ_PER_CORE = B // NCORES  # 128
S = SEQ_PER_CORE * T  # 12288 tokens per core
NB = 4  # sequences per block
TOK = NB * T  # 384 tokens per block
NBLK = SEQ_PER_CORE // NB  # 32 blocks
TCH = TOK // 128  # 3 token chunks per block
KC = C // 128  # 4 feature chunks of C
FM = F // 128  # 16 feature chunks of F

F32 = mybir.dt.float32
F32R = mybir.dt.float32r
BF16 = mybir.dt.bfloat16
AF = mybir.ActivationFunctionType
OP = mybir.AluOpType


def build(nblk=NBLK, has_bq=False, has_bk=False, has_bv=False, has_bp=False,
          has_b2=False):
    nc = bacc.Bacc("TRN2", target_bir_lowering=False, debug=False)

    def din(name, shape, dt):
        return nc.dram_tensor(name, shape, dt, kind="ExternalInput").ap()

    x_d = din("x", [S, C], F32)
    wq_d = din("wq", [C, C], BF16)
    wk_d = din("wk", [C, C], BF16)
    wv_d = din("wv", [C, C], BF16)
    wp_d = din("wp", [C, C], F32R)
    w1_d = din("w1", [C, F], BF16)
    w2_d = din("w2", [F, C], F32R)
    b1_d = din("b1", [F], F32)
    mask_d = din("mask", [T, T], BF16)
    ident_d = din("ident", [128, 128], BF16)
    bq_d = din("bq", [C], F32) if has_bq else None
    bk_d = din("bk", [C], F32) if has_bk else None
    bv_d = din("bv_b", [T, C], F32) if has_bv else None
    bp_d = din("bp_b", [128, C], F32) if has_bp else None
    b2_d = din("b2_b", [128, C], F32) if has_b2 else None
    y_d = nc.dram_tensor("y", [S, C], F32, kind="ExternalOutput").ap()

    with tile.TileContext(nc) as tc, ExitStack() as ctx:
        wp = ctx.enter_context(tc.tile_pool(name="wpool", bufs=1))
        ap_ = ctx.enter_context(tc.tile_pool(name="act", bufs=2))
        st = ctx.enter_context(tc.tile_pool(name="stat", bufs=3))
        hp = ctx.enter_context(tc.tile_pool(name="ht", bufs=1))
        ps = ctx.enter_context(tc.tile_pool(name="psum", bufs=1, space="PSUM"))

        # ---- resident weights ----
        def wload(name, d_ap, kchunks, fdim, dt):
            t = wp.tile([128, kchunks, fdim], dt, tag=name)
            nc.sync.dma_start(t[:], d_ap.rearrange("(kc p) f -> p kc f", p=128))
            return t

        wq_sb = wload("wq", wq_d, KC, C, BF16)
        wk_sb = wload("wk", wk_d, KC, C, BF16)
        wv_sb = wload("wv", wv_d, KC, C, BF16)
        wp_sb = wload("wp", wp_d, KC, C, F32R)
        w1_sb = wload("w1", w1_d, KC, F, BF16)
        w2_sb = wload("w2", w2_d, FM, C, F32R)

        b1_sb = wp.tile([128, FM], F32, tag="b1")
        nc.sync.dma_start(b1_sb[:], b1_d.rearrange("(fm p) -> p fm", p=128))
        mask_sb = wp.tile([T, T], BF16, tag="mask")
        nc.sync.dma_start(mask_sb[:], mask_d)
        ident_sb = wp.tile([128, 128], BF16, tag="ident")
        nc.sync.dma_start(ident_sb[:], ident_d)
        eps_sb = wp.tile([128, 1], F32, tag="eps")
        nc.vector.memset(eps_sb[:], EPS)
        if has_bq:
            bq_sb = wp.tile([128, H], F32, tag="bq")
            nc.sync.dma_start(bq_sb[:], bq_d.rearrange("(h d) -> d h", d=128))
        if has_bk:
            bk_sb = wp.tile([128, H], F32, tag="bk")
            nc.sync.dma_start(bk_sb[:], bk_d.rearrange("(h d) -> d h", d=128))
        if has_bv:
            bv_sb = wp.tile([T, C], F32, tag="bv")
            nc.sync.dma_start(bv_sb[:], bv_d)
        if has_bp:
            bp_sb = wp.tile([128, C], F32, tag="bp")
            nc.sync.dma_start(bp_sb[:], bp_d)
        if has_b2:
            b2_sb = wp.tile([128, C], F32, tag="b2")
            nc.sync.dma_start(b2_sb[:], b2_d)

        # ---- per-block helpers ----
        def layer_norm(src, pref):
            """src: [128, TCH, C] f32 -> xn bf16 [128, TCH, C]."""
            sums = st.tile([128, TCH], F32, tag=pref + "sums")
            nc.vector.tensor_reduce(sums[:], src[:], axis=mybir.AxisListType.X,
                                    op=OP.add)
            sumsq = st.tile([128, TCH], F32, tag=pref + "sumsq")
            for i in range(TCH):
                scr = st.tile([128, C], BF16, tag="scr")
                nc.vector.scalar_tensor_tensor(
                    scr[:], src[:, i, :], 1.0, src[:, i, :], OP.mult, OP.mult,
                    accum_out=sumsq[:, i : i + 1])
            mu = st.tile([128, TCH], F32, tag=pref + "mu")
            nc.vector.tensor_scalar_mul(mu[:], sums[:], 1.0 / C)
            msq = st.tile([128, TCH], F32, tag=pref + "msq")
            nc.vector.tensor_mul(out=msq[:], in0=mu[:], in1=mu[:])
            var = st.tile([128, TCH], F32, tag=pref + "var")
            nc.vector.scalar_tensor_tensor(var[:], sumsq[:], 1.0 / C, msq[:],
                                           OP.mult, OP.subtract)
            std = st.tile([128, TCH], F32, tag=pref + "std")
            nc.scalar.activation(std[:], var[:], AF.Sqrt, bias=eps_sb[:, 0:1])
            rstd = st.tile([128, TCH], F32, tag=pref + "rstd")
            nc.vector.reciprocal(rstd[:], std[:])
            nmr = st.tile([128, TCH], F32, tag=pref + "nmr")
            nc.vector.scalar_tensor_tensor(nmr[:], mu[:], -1.0, rstd[:],
                                           OP.mult, OP.mult)
            xn = ap_.tile([128, TCH, C], BF16, tag=pref + "xn")
            for i in range(TCH):
                nc.vector.scalar_tensor_tensor(
                    xn[:, i, :], src[:, i, :], rstd[:, i : i + 1],
                    nmr[:, i : i + 1].to_broadcast([128, C]), OP.mult, OP.add)
            return xn

        def transpose_xn(xn, pref):
            """Transpose LN output to T-layout on the PE (bf16)."""
            xnT = ap_.tile([128, KC, TOK], BF16, tag=pref + "xnT")
            for kc in range(KC):
                p = ps.tile([128, TCH, 128], BF16, tag="ps", bufs=8, name="txp")
                for mc in range(TCH):
                    nc.tensor.transpose(p[:, mc, :],
                                        xn[:, mc, kc * 128 : (kc + 1) * 128],
                                        ident_sb[:])
                if kc % 2 == 0:
                    nc.scalar.activation(xnT[:, kc, :], p[:], AF.Identity)
                else:
                    nc.vector.tensor_copy(out=xnT[:, kc, :], in_=p[:])
            return xnT

        # ---- block loop ----
        for blk in range(nblk):
            row0 = blk * TOK
            x_sb = ap_.tile([128, TCH, C], F32, tag="x")
            nc.sync.dma_start(
                x_sb[:],
                x_d[row0 : row0 + TOK, :].rearrange("(ch p) c -> p ch c", p=128))

            xn = layer_norm(x_sb, "a")
            xnT = transpose_xn(xn, "a")

            # QKV projections (bf16)
            qt = ap_.tile([128, H, TOK], BF16, tag="qt")
            kt = ap_.tile([128, H, TOK], BF16, tag="kt")
            for dst, w_sb, bias_sb in ((qt, wq_sb, bq_sb if has_bq else None),
                                       (kt, wk_sb, bk_sb if has_bk else None)):
                for h in range(H):
                    p = ps.tile([128, TOK], F32, tag="ps", bufs=8)
                    for kc in range(KC):
                        nc.tensor.matmul(p[:], w_sb[:, kc, h * 128 : (h + 1) * 128],
                                         xnT[:, kc, :], start=(kc == 0),
                                         stop=(kc == KC - 1))
                    if bias_sb is not None:
                        nc.scalar.activation(dst[:, h, :], p[:], AF.Identity,
                                             bias=bias_sb[:, h : h + 1])
                    else:
                        nc.vector.tensor_copy(out=dst[:, h, :], in_=p[:])
            vt = ap_.tile([T, NB, C], BF16, tag="vt")
            for b in range(NB):
                p = ps.tile([T, C], F32, tag="ps", bufs=8)
                for kc in range(KC):
                    nc.tensor.matmul(p[:], xnT[:, kc, b * T : (b + 1) * T],
                                     wv_sb[:, kc, :], start=(kc == 0),
                                     stop=(kc == KC - 1))
                if has_bv:
                    nc.vector.tensor_add(out=vt[:, b, :], in0=p[:], in1=bv_sb[:])
                else:
                    nc.vector.tensor_copy(out=vt[:, b, :], in_=p[:])

            # attention: scores [t, s] per (h, b), exp, mask, row-normalize
            ee = ap_.tile([T, H * NB, T], BF16, tag="ee")
            for h in range(H):
                p = ps.tile([T, NB, T], F32, tag="ps", bufs=8)
                for b in range(NB):
                    nc.tensor.matmul(p[:, b, :], qt[:, h, b * T : (b + 1) * T],
                                     kt[:, h, b * T : (b + 1) * T],
                                     start=True, stop=True)
                nc.scalar.activation(ee[:, h * NB : (h + 1) * NB, :], p[:],
                                     AF.Exp, scale=SCALE)
            nc.vector.tensor_mul(
                out=ee[:], in0=ee[:],
                in1=mask_sb[:].unsqueeze(1).to_broadcast([T, H * NB, T]))
            dsum = st.tile([T, H * NB], F32, tag="dsum")
            nc.vector.tensor_reduce(dsum[:], ee[:], axis=mybir.AxisListType.X,
                                    op=OP.add)
            rr = st.tile([T, H * NB], F32, tag="rr")
            nc.vector.reciprocal(rr[:], dsum[:])
            nc.vector.tensor_mul(
                out=ee[:], in0=ee[:],
                in1=rr[:].unsqueeze(2).to_broadcast([T, H * NB, T]))

            # transpose probs on PE, then attn @ V -> OT (T-layout, f32r)
            pt = ap_.tile([T, H * NB, T], BF16, tag="pt")
            for h in range(H):
                p = ps.tile([T, NB, T], BF16, tag="ps", bufs=8)
                for b in range(NB):
                    nc.tensor.transpose(p[:, b, :], ee[:, h * NB + b, :],
                                        ident_sb[:T, :T])
                nc.vector.tensor_copy(out=pt[:, h * NB : (h + 1) * NB, :], in_=p[:])
            ot = ap_.tile([128, H, TOK], F32R, tag="ot")
            for h in range(H):
                p = ps.tile([128, NB, T], F32, tag="ps", bufs=8)
                for b in range(NB):
                    nc.tensor.matmul(p[:, b, :], vt[:, b, h * 128 : (h + 1) * 128],
                                     pt[:, h * NB + b, :], start=True, stop=True)
                nc.scalar.activation(ot[:, h, :], p[:], AF.Identity)

            # proj (f32r) + residual
            x2 = ap_.tile([128, TCH, C], F32, tag="x2")
            for mc in range(TCH):
                p = ps.tile([128, C], F32, tag="ps", bufs=8)
                for kc in range(H):
                    nc.tensor.matmul(p[:], ot[:, kc, mc * 128 : (mc + 1) * 128],
                                     wp_sb[:, kc, :], start=(kc == 0),
                                     stop=(kc == H - 1))
                if has_bp:
                    nc.vector.tensor_add(out=p[:], in0=p[:], in1=bp_sb[:])
                nc.vector.tensor_add(out=x2[:, mc, :], in0=p[:],
                                     in1=x_sb[:, mc, :])

            # MLP
            xn2 = layer_norm(x2, "b")
            xn2T = transpose_xn(xn2, "b")
            ht = hp.tile([128, FM, TOK], F32R, tag="ht")
            for fm in range(FM):
                p = ps.tile([128, TOK], F32, tag="ps", bufs=8)
                for kc in range(KC):
                    nc.tensor.matmul(p[:], w1_sb[:, kc, fm * 128 : (fm + 1) * 128],
                                     xn2T[:, kc, :], start=(kc == 0),
                                     stop=(kc == KC - 1))
                nc.scalar.activation(ht[:, fm, :], p[:], AF.Gelu,
                                     bias=b1_sb[:, fm : fm + 1])
            xo = ap_.tile([128, TCH, C], F32, tag="xo")
            for mc in range(TCH):
                p = ps.tile([128, C], F32, tag="ps", bufs=8)
                for fk in range(FM):
                    nc.tensor.matmul(p[:], ht[:, fk, mc * 128 : (mc + 1) * 128],
                                     w2_sb[:, fk, :], start=(fk == 0),
                                     stop=(fk == FM - 1))
                if has_b2:
                    nc.vector.tensor_add(out=p[:], in0=p[:], in1=b2_sb[:])
                nc.vector.tensor_add(out=xo[:, mc, :], in0=p[:],
                                     in1=x2[:, mc, :])
            nc.sync.dma_start(
                y_d[row0 : row0 + TOK, :].rearrange("(ch p) c -> p ch c", p=128),
                xo[:])

    nc.compile()
    return nc


def fold(inputs):
    """Host-side exact folding of LN affines and biases into weights.

    Returns dict of staged arrays for the device program + bias flags.
    """
    f32 = np.float32
    g1 = inputs["g1"].astype(f32)
    be1 = inputs["be1"].astype(f32)
    g2 = inputs["g2"].astype(f32)
    be2 = inputs["be2"].astype(f32)

    def headcat(w):  # [H, C, D] -> [C, H*D]
        return np.concatenate([w[h] for h in range(H)], axis=1)

    wq = headcat(np.asarray(inputs["wq"], f32))
    wk = headcat(np.asarray(inputs["wk"], f32))
    wv = headcat(np.asarray(inputs["wv"], f32))
    wp_ = np.asarray(inputs["w_proj"], f32)
    w1 = np.asarray(inputs["w1"], f32)
    w2 = np.asarray(inputs["w2"], f32)

    wq_f = g1[:, None] * wq
    wk_f = g1[:, None] * wk
    wv_f = g1[:, None] * wv
    bq = be1 @ wq
    bk = be1 @ wk
    bv = be1 @ wv
    bp = np.asarray(inputs["b_proj"], f32)
    w1_f = g2[:, None] * w1
    b1 = np.asarray(inputs["b1"], f32) + be2 @ w1
    b2 = np.asarray(inputs["b2"], f32)

    mask = np.tril(np.ones((T, T), np.float32)).astype(ml_dtypes.bfloat16)
    ident = np.eye(128, dtype=ml_dtypes.bfloat16)

    staged = {
        "wq": wq_f.astype(ml_dtypes.bfloat16),
        "wk": wk_f.astype(ml_dtypes.bfloat16),
        "wv": wv_f.astype(ml_dtypes.bfloat16),
        "wp": wp_.astype(f32),
        "w1": w1_f.astype(ml_dtypes.bfloat16),
        "w2": w2.astype(f32),
        "b1": b1,
        "mask": mask,
        "ident": ident,
    }
    flags = {
        "has_bq": bool(np.any(bq)),
        "has_bk": bool(np.any(bk)),
        "has_bv": bool(np.any(bv)),
        "has_bp": bool(np.any(bp)),
        "has_b2": bool(np.any(b2)),
    }
    if flags["has_bq"]:
        staged["bq"] = bq
    if flags["has_bk"]:
        staged["bk"] = bk
    if flags["has_bv"]:
        staged["bv_b"] = np.broadcast_to(bv, (T, C)).copy()
    if flags["has_bp"]:
        staged["bp_b"] = np.broadcast_to(bp, (128, C)).copy()
    if flags["has_b2"]:
        staged["b2_b"] = np.broadcast_to(b2, (128, C)).copy()
    return staged, flags


_CACHE = {}


def kernel(**inputs):
    staged, flags = fold(inputs)
    key = tuple(sorted(flags.items()))
    if key not in _CACHE:
        _CACHE[key] = build(**flags)
    nc = _CACHE[key]

    x = np.asarray(inputs["x"], np.float32).reshape(B, T * C)
    in_maps = []
    for c in range(NCORES):
        m = dict(staged)
        m["x"] = x[c * SEQ_PER_CORE : (c + 1) * SEQ_PER_CORE].reshape(S, C)
        in_maps.append(m)

    res = bass_utils.run_bass_kernel_spmd(nc, in_maps, core_ids=list(range(NCORES)))
    out = np.concatenate([r["y"] for r in res.results], axis=0)
    return out.reshape(B, T, C).astype(np.float32)


# revision 15
# speedup vs baseline: 1.5828x; 1.0016x over previous
"""Fused transformer-block kernel for TRN2, 8-way data parallel over batch.

Layout strategy per core (128 sequences of 96 tokens = 12288 tokens):
  - Residual stream kept in N-layout [token_part, feature_free]; LayerNorm
    stats are free-dim reductions.
  - LN outputs written as bf16 and transposed to feature-major T-layout
    [feature_part, token_free] via DMA-xbar transposes; these feed the QKV
    and MLP1 matmuls (bf16).
  - Attention computed per (seq, head) with T=96 <= 128: scores in [t, s]
    layout (softmax over free dim), exp without max-subtraction (scores are
    bounded for this problem scale), 0/1 causal mask multiply, probs
    transposed on the PE, then attn@V gives head outputs directly in
    T-layout.
  - proj and MLP2 run in float32r (full PE speed at N=512, ~1e-4 rel err).
  - gamma/beta of both LNs and all biases are folded into the weight
    matrices / bias vectors on the host (exact algebra, see fold()).
"""

import sys

sys.path.insert(0, "/opt/trn_rl_repo")

from contextlib import ExitStack

import ml_dtypes
import numpy as np

import concourse.bass as bass  # noqa: F401  (registers AP types)
import concourse.tile as tile
from concourse import bacc, bass_utils, mybir

# Cache walrus-compiled NEFFs on disk keyed by BIR hash: re-running an
# unchanged program skips the multi-minute backend compile.
try:
    import hashlib
    import os as _os
    import shutil as _shutil

    import concourse.bass2jax as _b2j

    _orig_cbk = _b2j.compile_bir_kernel

    def _cached_cbk(bir_json, tmpdir, neff_name="file.neff"):
        try:
            raw = bir_json if isinstance(bir_json, bytes) else bir_json.encode()
            h = hashlib.sha256(raw).hexdigest()[:24]
            cdir = "/tmp/neff_cache"
            _os.makedirs(cdir, exist_ok=True)
            cpath = _os.path.join(cdir, h + ".neff")
            if _os.path.exists(cpath):
                return cpath
        except Exception:
            return _orig_cbk(bir_json, tmpdir, neff_name)
        p = _orig_cbk(bir_json, tmpdir, neff_name)
        try:
            _shutil.copy(p, cpath)
        except Exception:
            pass
        return p

    if _orig_cbk.__name__ != "_cached_cbk":
        _b2j.compile_bir_kernel = _cached_cbk
except Exception:
    pass

B, T, C = 1024, 96, 512
H, D = 4, 128
F = 4 * C
EPS = 1e-5
SCALE = D**-0.5

NCORES = 8
SEQ_PER_CORE = B // NCORES  # 128
S = SEQ_PER_CORE * T  # 12288 tokens per core
NB = 4  # sequences per block
TOK = NB * T  # 384 tokens per block
NBLK = SEQ_PER_CORE // NB  # 32 blocks
TCH = TOK // 128  # 3 token chunks per block
KC = C // 128  # 4 feature chunks of C
FM = F // 128  # 16 feature chunks of F

F32 = mybir.dt.float32
F32R = mybir.dt.float32r
BF16 = mybir.dt.bfloat16
AF = mybir.ActivationFunctionType
OP = mybir.AluOpType


def build(nblk=NBLK, has_bq=False, has_bk=False, has_bv=False, has_bp=False,
          has_b2=False):
    nc = bacc.Bacc("TRN2", target_bir_lowering=False, debug=False)

    def din(name, shape, dt):
        return nc.dram_tensor(name, shape, dt, kind="ExternalInput").ap()

    x_d = din("x", [S, C], F32)
    wq_d = din("wq", [C, C], BF16)
    wk_d = din("wk", [C, C], BF16)
    wv_d = din("wv", [C, C], BF16)
    wp_d = din("wp", [C, C], F32R)
    w1_d = din("w1", [C, F], BF16)
    w2_d = din("w2", [F, C], F32R)
    b1_d = din("b1", [F], F32)
    mask_d = din("mask", [T, T], BF16)
    ident_d = din("ident", [128, 128], BF16)
    bq_d = din("bq", [C], F32) if has_bq else None
    bk_d = din("bk", [C], F32) if has_bk else None
    bv_d = din("bv_b", [T, C], F32) if has_bv else None
    bp_d = din("bp_b", [128, C], F32) if has_bp else None
    b2_d = din("b2_b", [128, C], F32) if has_b2 else None
    y_d = nc.dram_tensor("y", [S, C], F32, kind="ExternalOutput").ap()

    with tile.TileContext(nc) as tc, ExitStack() as ctx:
        wp = ctx.enter_context(tc.tile_pool(name="wpool", bufs=1))
        ap_ = ctx.enter_context(tc.tile_pool(name="act", bufs=2))
        st = ctx.enter_context(tc.tile_pool(name="stat", bufs=3))
        hp = ctx.enter_context(tc.tile_pool(name="ht", bufs=1))
        ps = ctx.enter_context(tc.tile_pool(name="psum", bufs=1, space="PSUM"))

        # ---- resident weights ----
        def wload(name, d_ap, kchunks, fdim, dt):
            t = wp.tile([128, kchunks, fdim], dt, tag=name)
            nc.sync.dma_start(t[:], d_ap.rearrange("(kc p) f -> p kc f", p=128))
            return t

        wq_sb = wload("wq", wq_d, KC, C, BF16)
        wk_sb = wload("wk", wk_d, KC, C, BF16)
        wv_sb = wload("wv", wv_d, KC, C, BF16)
        wp_sb = wload("wp", wp_d, KC, C, F32R)
        w1_sb = wload("w1", w1_d, KC, F, BF16)
        w2_sb = wload("w2", w2_d, FM, C, F32R)

        b1_sb = wp.tile([128, FM], F32, tag="b1")
        nc.sync.dma_start(b1_sb[:], b1_d.rearrange("(fm p) -> p fm", p=128))
        mask_sb = wp.tile([T, T], BF16, tag="mask")
        nc.sync.dma_start(mask_sb[:], mask_d)
        ident_sb = wp.tile([128, 128], BF16, tag="ident")
        nc.sync.dma_start(ident_sb[:], ident_d)
        eps_sb = wp.tile([128, 1], F32, tag="eps")
        nc.vector.memset(eps_sb[:], EPS)
        if has_bq:
            bq_sb = wp.tile([128, H], F32, tag="bq")
            nc.sync.dma_start(bq_sb[:], bq_d.rearrange("(h d) -> d h", d=128))
        if has_bk:
            bk_sb = wp.tile([128, H], F32, tag="bk")
            nc.sync.dma_start(bk_sb[:], bk_d.rearrange("(h d) -> d h", d=128))
        if has_bv:
            bv_sb = wp.tile([T, C], F32, tag="bv")
            nc.sync.dma_start(bv_sb[:], bv_d)
        if has_bp:
            bp_sb = wp.tile([128, C], F32, tag="bp")
            nc.sync.dma_start(bp_sb[:], bp_d)
        if has_b2:
            b2_sb = wp.tile([128, C], F32, tag="b2")
            nc.sync.dma_start(b2_sb[:], b2_d)

        # ---- per-block helpers ----
        def layer_norm(src, pref):
            """src: [128, TCH, C] f32 -> xn bf16 [128, TCH, C]."""
            sums = st.tile([128, TCH], F32, tag=pref + "sums")
            nc.vector.tensor_reduce(sums[:], src[:], axis=mybir.AxisListType.X,
                                    op=OP.add)
            sumsq = st.tile([128, TCH], F32, tag=pref + "sumsq")
            for i in range(TCH):
                scr = st.tile([128, C], BF16, tag="scr")
                nc.vector.scalar_tensor_tensor(
                    scr[:], src[:, i, :], 1.0, src[:, i, :], OP.mult, OP.mult,
                    accum_out=sumsq[:, i : i + 1])
            mu = st.tile([128, TCH], F32, tag=pref + "mu")
            nc.vector.tensor_scalar_mul(mu[:], sums[:], 1.0 / C)
            msq = st.tile([128, TCH], F32, tag=pref + "msq")
            nc.vector.tensor_mul(out=msq[:], in0=mu[:], in1=mu[:])
            var = st.tile([128, TCH], F32, tag=pref + "var")
            nc.vector.scalar_tensor_tensor(var[:], sumsq[:], 1.0 / C, msq[:],
                                           OP.mult, OP.subtract)
            std = st.tile([128, TCH], F32, tag=pref + "std")
            nc.scalar.activation(std[:], var[:], AF.Sqrt, bias=eps_sb[:, 0:1])
            rstd = st.tile([128, TCH], F32, tag=pref + "rstd")
            nc.vector.reciprocal(rstd[:], std[:])
            nmr = st.tile([128, TCH], F32, tag=pref + "nmr")
            nc.vector.scalar_tensor_tensor(nmr[:], mu[:], -1.0, rstd[:],
                                           OP.mult, OP.mult)
            xn = ap_.tile([128, TCH, C], BF16, tag=pref + "xn")
            for i in range(TCH):
                nc.vector.scalar_tensor_tensor(
                    xn[:, i, :], src[:, i, :], rstd[:, i : i + 1],
                    nmr[:, i : i + 1].to_broadcast([128, C]), OP.mult, OP.add)
            return xn

        def transpose_xn(xn, pref):
            """Transpose LN output to T-layout on the PE (bf16)."""
            xnT = ap_.tile([128, KC, TOK], BF16, tag=pref + "xnT")
            for kc in range(KC):
                p = ps.tile([128, TCH, 128], BF16, tag="ps", bufs=8, name="txp")
                for mc in range(TCH):
                    nc.tensor.transpose(p[:, mc, :],
                                        xn[:, mc, kc * 128 : (kc + 1) * 128],
                                        ident_sb[:])
                if kc % 2 == 0:
                    nc.scalar.activation(xnT[:, kc, :], p[:], AF.Identity)
                else:
                    nc.vector.tensor_copy(out=xnT[:, kc, :], in_=p[:])
            return xnT

        # ---- block stages ----
        def stage_a(blk):
            """x load, LN1, QKV, attention -> returns (x_sb, ot)."""
            row0 = blk * TOK
            x_sb = ap_.tile([128, TCH, C], F32, tag="x", bufs=3)
            nc.sync.dma_start(
                x_sb[:],
                x_d[row0 : row0 + TOK, :].rearrange("(ch p) c -> p ch c", p=128))

            xn = layer_norm(x_sb, "a")
            xnT = transpose_xn(xn, "a")

            # QKV projections (bf16)
            qt = ap_.tile([128, H, TOK], BF16, tag="qt")
            kt = ap_.tile([128, H, TOK], BF16, tag="kt")
            for dst, w_sb, bias_sb in ((qt, wq_sb, bq_sb if has_bq else None),
                                       (kt, wk_sb, bk_sb if has_bk else None)):
                for h in range(H):
                    p = ps.tile([128, TOK], F32, tag="ps", bufs=8)
                    for kc in range(KC):
                        nc.tensor.matmul(p[:], w_sb[:, kc, h * 128 : (h + 1) * 128],
                                         xnT[:, kc, :], start=(kc == 0),
                                         stop=(kc == KC - 1))
                    if bias_sb is not None:
                        nc.scalar.activation(dst[:, h, :], p[:], AF.Identity,
                                             bias=bias_sb[:, h : h + 1])
                    else:
                        nc.vector.tensor_copy(out=dst[:, h, :], in_=p[:])
            vt = ap_.tile([T, NB, C], BF16, tag="vt")
            for b in range(NB):
                p = ps.tile([T, C], F32, tag="ps", bufs=8)
                for kc in range(KC):
                    nc.tensor.matmul(p[:], xnT[:, kc, b * T : (b + 1) * T],
                                     wv_sb[:, kc, :], start=(kc == 0),
                                     stop=(kc == KC - 1))
                if has_bv:
                    nc.vector.tensor_add(out=vt[:, b, :], in0=p[:], in1=bv_sb[:])
                else:
                    nc.vector.tensor_copy(out=vt[:, b, :], in_=p[:])

            # attention: scores [t, s] per (h, b), exp, mask, row-normalize
            ee = ap_.tile([T, H * NB, T], BF16, tag="ee")
            for h in range(H):
                p = ps.tile([T, NB, T], F32, tag="ps", bufs=8)
                for b in range(NB):
                    nc.tensor.matmul(p[:, b, :], qt[:, h, b * T : (b + 1) * T],
                                     kt[:, h, b * T : (b + 1) * T],
                                     start=True, stop=True)
                nc.scalar.activation(ee[:, h * NB : (h + 1) * NB, :], p[:],
                                     AF.Exp, scale=SCALE)
            nc.vector.tensor_mul(
                out=ee[:], in0=ee[:],
                in1=mask_sb[:].unsqueeze(1).to_broadcast([T, H * NB, T]))
            dsum = st.tile([T, H * NB], F32, tag="dsum")
            nc.vector.tensor_reduce(dsum[:], ee[:], axis=mybir.AxisListType.X,
                                    op=OP.add)
            rr = st.tile([T, H * NB], F32, tag="rr")
            nc.vector.reciprocal(rr[:], dsum[:])
            nc.vector.tensor_mul(
                out=ee[:], in0=ee[:],
                in1=rr[:].unsqueeze(2).to_broadcast([T, H * NB, T]))

            # transpose probs on PE, then attn @ V -> OT (T-layout, f32r)
            pt = ap_.tile([T, H * NB, T], BF16, tag="pt")
            for h in range(H):
                p = ps.tile([T, NB, T], BF16, tag="ps", bufs=8)
                for b in range(NB):
                    nc.tensor.transpose(p[:, b, :], ee[:, h * NB + b, :],
                                        ident_sb[:T, :T])
                nc.vector.tensor_copy(out=pt[:, h * NB : (h + 1) * NB, :], in_=p[:])
            ot = ap_.tile([128, H, TOK], F32R, tag="ot")
            for h in range(H):
                p = ps.tile([128, NB, T], F32, tag="ps", bufs=8)
                for b in range(NB):
                    nc.tensor.matmul(p[:, b, :], vt[:, b, h * 128 : (h + 1) * 128],
                                     pt[:, h * NB + b, :], start=True, stop=True)
                nc.scalar.activation(ot[:, h, :], p[:], AF.Identity)
            return x_sb, ot

        def stage_b(blk, x_sb, ot):
            """proj + residual, LN2, MLP, store."""
            row0 = blk * TOK
            x2 = ap_.tile([128, TCH, C], F32, tag="x2")
            for mc in range(TCH):
                p = ps.tile([128, C], F32, tag="ps", bufs=8)
                for kc in range(H):
                    nc.tensor.matmul(p[:], ot[:, kc, mc * 128 : (mc + 1) * 128],
                                     wp_sb[:, kc, :], start=(kc == 0),
                                     stop=(kc == H - 1))
                if has_bp:
                    nc.vector.tensor_add(out=p[:], in0=p[:], in1=bp_sb[:])
                nc.vector.tensor_add(out=x2[:, mc, :], in0=p[:],
                                     in1=x_sb[:, mc, :])

            # MLP
            xn2 = layer_norm(x2, "b")
            xn2T = transpose_xn(xn2, "b")
            ht = hp.tile([128, FM, TOK], F32R, tag="ht")
            for fm in range(FM):
                p = ps.tile([128, TOK], F32, tag="ps", bufs=8)
                for kc in range(KC):
                    nc.tensor.matmul(p[:], w1_sb[:, kc, fm * 128 : (fm + 1) * 128],
                                     xn2T[:, kc, :], start=(kc == 0),
                                     stop=(kc == KC - 1))
                nc.scalar.activation(ht[:, fm, :], p[:], AF.Gelu,
                                     bias=b1_sb[:, fm : fm + 1])
            xo = ap_.tile([128, TCH, C], F32, tag="xo")
            for mc in range(TCH):
                p = ps.tile([128, C], F32, tag="ps", bufs=8)
                for fk in range(FM):
                    nc.tensor.matmul(p[:], ht[:, fk, mc * 128 : (mc + 1) * 128],
                                     w2_sb[:, fk, :], start=(fk == 0),
                                     stop=(fk == FM - 1))
                if has_b2:
                    nc.vector.tensor_add(out=p[:], in0=p[:], in1=b2_sb[:])
                nc.vector.tensor_add(out=xo[:, mc, :], in0=p[:],
                                     in1=x2[:, mc, :])
            nc.sync.dma_start(
                y_d[row0 : row0 + TOK, :].rearrange("(ch p) c -> p ch c", p=128),
                xo[:])

        # Software-pipelined emission: stage A of block i+1 is emitted before
        # stage B of block i so each engine's (FIFO) instruction stream can
        # start the next block's independent front half while the current
        # block's back half waits on the proj dependency.
        pend = stage_a(0)
        for blk in range(1, nblk):
            nxt = stage_a(blk)
            stage_b(blk - 1, *pend)
            pend = nxt
        stage_b(nblk - 1, *pend)

    nc.compile()
    return nc


def fold(inputs):
    """Host-side exact folding of LN affines and biases into weights.

    Returns dict of staged arrays for the device program + bias flags.
    """
    f32 = np.float32
    g1 = inputs["g1"].astype(f32)
    be1 = inputs["be1"].astype(f32)
    g2 = inputs["g2"].astype(f32)
    be2 = inputs["be2"].astype(f32)

    def headcat(w):  # [H, C, D] -> [C, H*D]
        return np.concatenate([w[h] for h in range(H)], axis=1)

    wq = headcat(np.asarray(inputs["wq"], f32))
    wk = headcat(np.asarray(inputs["wk"], f32))
    wv = headcat(np.asarray(inputs["wv"], f32))
    wp_ = np.asarray(inputs["w_proj"], f32)
    w1 = np.asarray(inputs["w1"], f32)
    w2 = np.asarray(inputs["w2"], f32)

    wq_f = g1[:, None] * wq
    wk_f = g1[:, None] * wk
    wv_f = g1[:, None] * wv
    bq = be1 @ wq
    bk = be1 @ wk
    bv = be1 @ wv
    bp = np.asarray(inputs["b_proj"], f32)
    w1_f = g2[:, None] * w1
    b1 = np.asarray(inputs["b1"], f32) + be2 @ w1
    b2 = np.asarray(inputs["b2"], f32)

    mask = np.tril(np.ones((T, T), np.float32)).astype(ml_dtypes.bfloat16)
    ident = np.eye(128, dtype=ml_dtypes.bfloat16)

    staged = {
        "wq": wq_f.astype(ml_dtypes.bfloat16),
        "wk": wk_f.astype(ml_dtypes.bfloat16),
        "wv": wv_f.astype(ml_dtypes.bfloat16),
        "wp": wp_.astype(f32),
        "w1": w1_f.astype(ml_dtypes.bfloat16),
        "w2": w2.astype(f32),
        "b1": b1,
        "mask": mask,
        "ident": ident,
    }
    flags = {
        "has_bq": bool(np.any(bq)),
        "has_bk": bool(np.any(bk)),
        "has_bv": bool(np.any(bv)),
        "has_bp": bool(np.any(bp)),
        "has_b2": bool(np.any(b2)),
    }
    if flags["has_bq"]:
        staged["bq"] = bq
    if flags["has_bk"]:
        staged["bk"] = bk
    if flags["has_bv"]:
        staged["bv_b"] = np.broadcast_to(bv, (T, C)).copy()
    if flags["has_bp"]:
        staged["bp_b"] = np.broadcast_to(bp, (128, C)).copy()
    if flags["has_b2"]:
        staged["b2_b"] = np.broadcast_to(b2, (128, C)).copy()
    return staged, flags


_CACHE = {}


def kernel(**inputs):
    staged, flags = fold(inputs)
    key = tuple(sorted(flags.items()))
    if key not in _CACHE:
        _CACHE[key] = build(**flags)
    nc = _CACHE[key]

    x = np.asarray(inputs["x"], np.float32).reshape(B, T * C)
    in_maps = []
    for c in range(NCORES):
        m = dict(staged)
        m["x"] = x[c * SEQ_PER_CORE : (c + 1) * SEQ_PER_CORE].reshape(S, C)
        in_maps.append(m)

    res = bass_utils.run_bass_kernel_spmd(nc, in_maps, core_ids=list(range(NCORES)))
    out = np.concatenate([r["y"] for r in res.results], axis=0)
    return out.reshape(B, T, C).astype(np.float32)


# revision 18
# speedup vs baseline: 1.6606x; 1.0491x over previous
"""Fused transformer-block kernel for TRN2, 8-way data parallel over batch.

Layout strategy per core (128 sequences of 96 tokens = 12288 tokens):
  - Residual stream kept in N-layout [token_part, feature_free]; LayerNorm
    stats are free-dim reductions.
  - LN outputs written as bf16 and transposed to feature-major T-layout
    [feature_part, token_free] via DMA-xbar transposes; these feed the QKV
    and MLP1 matmuls (bf16).
  - Attention computed per (seq, head) with T=96 <= 128: scores in [t, s]
    layout (softmax over free dim), exp without max-subtraction (scores are
    bounded for this problem scale), 0/1 causal mask multiply, probs
    transposed on the PE, then attn@V gives head outputs directly in
    T-layout.
  - proj and MLP2 run in float32r (full PE speed at N=512, ~1e-4 rel err).
  - gamma/beta of both LNs and all biases are folded into the weight
    matrices / bias vectors on the host (exact algebra, see fold()).
"""

import sys

sys.path.insert(0, "/opt/trn_rl_repo")

from contextlib import ExitStack

import ml_dtypes
import numpy as np

import concourse.bass as bass  # noqa: F401  (registers AP types)
import concourse.tile as tile
from concourse import bacc, bass_utils, mybir

# Cache walrus-compiled NEFFs on disk keyed by BIR hash: re-running an
# unchanged program skips the multi-minute backend compile.
try:
    import hashlib
    import os as _os
    import shutil as _shutil

    import concourse.bass2jax as _b2j

    _orig_cbk = _b2j.compile_bir_kernel

    def _cached_cbk(bir_json, tmpdir, neff_name="file.neff"):
        try:
            raw = bir_json if isinstance(bir_json, bytes) else bir_json.encode()
            h = hashlib.sha256(raw).hexdigest()[:24]
            cdir = "/tmp/neff_cache"
            _os.makedirs(cdir, exist_ok=True)
            cpath = _os.path.join(cdir, h + ".neff")
            if _os.path.exists(cpath):
                return cpath
        except Exception:
            return _orig_cbk(bir_json, tmpdir, neff_name)
        p = _orig_cbk(bir_json, tmpdir, neff_name)
        try:
            _shutil.copy(p, cpath)
        except Exception:
            pass
        return p

    if _orig_cbk.__name__ != "_cached_cbk":
        _b2j.compile_bir_kernel = _cached_cbk
except Exception:
    pass

B, T, C = 1024, 96, 512
H, D = 4, 128
F = 4 * C
EPS = 1e-5
SCALE = D**-0.5

NCORES = 8
SEQ_PER_CORE = B // NCORES  # 128
S = SEQ_PER_CORE * T  # 12288 tokens per core
NB = 4  # sequences per block
TOK = NB * T  # 384 tokens per block
NBLK = SEQ_PER_CORE // NB  # 32 blocks
TCH = TOK // 128  # 3 token chunks per block
KC = C // 128  # 4 feature chunks of C
FM = F // 128  # 16 feature chunks of F

F32 = mybir.dt.float32
F32R = mybir.dt.float32r
BF16 = mybir.dt.bfloat16
AF = mybir.ActivationFunctionType
OP = mybir.AluOpType


def build(nblk=NBLK, has_bq=False, has_bk=False, has_bv=False, has_bp=False,
          has_b2=False):
    nc = bacc.Bacc("TRN2", target_bir_lowering=False, debug=False)

    def din(name, shape, dt):
        return nc.dram_tensor(name, shape, dt, kind="ExternalInput").ap()

    x_d = din("x", [S, C], F32)
    wq_d = din("wq", [C, C], BF16)
    wk_d = din("wk", [C, C], BF16)
    wv_d = din("wv", [C, C], BF16)
    wp_d = din("wp", [C, C], F32R)
    w1_d = din("w1", [C, F], BF16)
    w2_d = din("w2", [F, C], F32R)
    b1_d = din("b1", [F], F32)
    mask_d = din("mask", [T, T], BF16)
    ident_d = din("ident", [128, 128], BF16)
    bq_d = din("bq", [C], F32) if has_bq else None
    bk_d = din("bk", [C], F32) if has_bk else None
    bv_d = din("bv_b", [T, C], F32) if has_bv else None
    bp_d = din("bp_b", [128, C], F32) if has_bp else None
    b2_d = din("b2_b", [128, C], F32) if has_b2 else None
    y_d = nc.dram_tensor("y", [S, C], F32, kind="ExternalOutput").ap()

    with tile.TileContext(nc) as tc, ExitStack() as ctx:
        wp = ctx.enter_context(tc.tile_pool(name="wpool", bufs=1))
        ap_ = ctx.enter_context(tc.tile_pool(name="act", bufs=2))
        st = ctx.enter_context(tc.tile_pool(name="stat", bufs=3))
        hp = ctx.enter_context(tc.tile_pool(name="ht", bufs=1))
        ps = ctx.enter_context(tc.tile_pool(name="psum", bufs=1, space="PSUM"))

        # ---- resident weights ----
        def wload(name, d_ap, kchunks, fdim, dt):
            t = wp.tile([128, kchunks, fdim], dt, tag=name)
            nc.sync.dma_start(t[:], d_ap.rearrange("(kc p) f -> p kc f", p=128))
            return t

        wq_sb = wload("wq", wq_d, KC, C, BF16)
        wk_sb = wload("wk", wk_d, KC, C, BF16)
        wv_sb = wload("wv", wv_d, KC, C, BF16)
        wp_sb = wload("wp", wp_d, KC, C, F32R)
        w1_sb = wload("w1", w1_d, KC, F, BF16)
        w2_sb = wload("w2", w2_d, FM, C, F32R)

        b1_sb = wp.tile([128, FM], F32, tag="b1")
        nc.sync.dma_start(b1_sb[:], b1_d.rearrange("(fm p) -> p fm", p=128))
        mask_sb = wp.tile([T, T], BF16, tag="mask")
        nc.sync.dma_start(mask_sb[:], mask_d)
        ident_sb = wp.tile([128, 128], BF16, tag="ident")
        nc.sync.dma_start(ident_sb[:], ident_d)
        eps_sb = wp.tile([128, 1], F32, tag="eps")
        nc.vector.memset(eps_sb[:], EPS)
        if has_bq:
            bq_sb = wp.tile([128, H], F32, tag="bq")
            nc.sync.dma_start(bq_sb[:], bq_d.rearrange("(h d) -> d h", d=128))
        if has_bk:
            bk_sb = wp.tile([128, H], F32, tag="bk")
            nc.sync.dma_start(bk_sb[:], bk_d.rearrange("(h d) -> d h", d=128))
        if has_bv:
            bv_sb = wp.tile([T, C], F32, tag="bv")
            nc.sync.dma_start(bv_sb[:], bv_d)
        if has_bp:
            bp_sb = wp.tile([128, C], F32, tag="bp")
            nc.sync.dma_start(bp_sb[:], bp_d)
        if has_b2:
            b2_sb = wp.tile([128, C], F32, tag="b2")
            nc.sync.dma_start(b2_sb[:], b2_d)

        # ---- per-block helpers ----
        def ln_stats_apply(src, pref, sums, sumsq):
            """Finish LN given per-chunk sums/sumsq [128, TCH]; apply on ACT."""
            mu = st.tile([128, TCH], F32, tag=pref + "mu")
            nc.vector.tensor_scalar_mul(mu[:], sums[:], 1.0 / C)
            msq = st.tile([128, TCH], F32, tag=pref + "msq")
            nc.vector.tensor_mul(out=msq[:], in0=mu[:], in1=mu[:])
            var = st.tile([128, TCH], F32, tag=pref + "var")
            nc.vector.scalar_tensor_tensor(var[:], sumsq[:], 1.0 / C, msq[:],
                                           OP.mult, OP.subtract)
            std = st.tile([128, TCH], F32, tag=pref + "std")
            nc.scalar.activation(std[:], var[:], AF.Sqrt, bias=eps_sb[:, 0:1])
            rstd = st.tile([128, TCH], F32, tag=pref + "rstd")
            nc.vector.reciprocal(rstd[:], std[:])
            nmr = st.tile([128, TCH], F32, tag=pref + "nmr")
            nc.vector.scalar_tensor_tensor(nmr[:], mu[:], -1.0, rstd[:],
                                           OP.mult, OP.mult)
            xn = ap_.tile([128, TCH, C], BF16, tag=pref + "xn")
            for i in range(TCH):
                nc.scalar.activation(xn[:, i, :], src[:, i, :], AF.Identity,
                                     scale=rstd[:, i : i + 1],
                                     bias=nmr[:, i : i + 1])
            return xn

        def layer_norm(src, pref):
            """src: [128, TCH, C] f32 -> xn bf16 [128, TCH, C]."""
            sums = st.tile([128, TCH], F32, tag=pref + "sums")
            sumsq = st.tile([128, TCH], F32, tag=pref + "sumsq")
            for i in range(TCH):
                scr = st.tile([128, C], BF16, tag="scr", bufs=2)
                nc.vector.scalar_tensor_tensor(
                    scr[:], src[:, i, :], 1.0, src[:, i, :], OP.mult, OP.mult,
                    accum_out=sumsq[:, i : i + 1])
                scr2 = st.tile([128, C], BF16, tag="scr2", bufs=2)
                nc.scalar.activation(scr2[:], src[:, i, :], AF.Copy,
                                     accum_out=sums[:, i : i + 1])
            return ln_stats_apply(src, pref, sums, sumsq)

        def transpose_xn(xn, pref):
            """Transpose LN output to T-layout on the PE (bf16)."""
            xnT = ap_.tile([128, KC, TOK], BF16, tag=pref + "xnT")
            for kc in range(KC):
                p = ps.tile([128, TCH, 128], BF16, tag="ps", bufs=8, name="txp")
                for mc in range(TCH):
                    nc.tensor.transpose(p[:, mc, :],
                                        xn[:, mc, kc * 128 : (kc + 1) * 128],
                                        ident_sb[:])
                if kc % 2 == 0:
                    nc.scalar.activation(xnT[:, kc, :], p[:], AF.Identity)
                else:
                    nc.vector.tensor_copy(out=xnT[:, kc, :], in_=p[:])
            return xnT

        # ---- block stages ----
        def stage_a(blk):
            """x load, LN1, QKV, attention -> returns (x_sb, ot)."""
            row0 = blk * TOK
            x_sb = ap_.tile([128, TCH, C], F32, tag="x", bufs=2)
            nc.sync.dma_start(
                x_sb[:],
                x_d[row0 : row0 + TOK, :].rearrange("(ch p) c -> p ch c", p=128))

            xn = layer_norm(x_sb, "a")
            xnT = transpose_xn(xn, "a")

            # QKV projections (bf16)
            qt = ap_.tile([128, H, TOK], BF16, tag="qt")
            kt = ap_.tile([128, H, TOK], BF16, tag="kt")
            for dst, w_sb, bias_sb in ((qt, wq_sb, bq_sb if has_bq else None),
                                       (kt, wk_sb, bk_sb if has_bk else None)):
                for h in range(H):
                    p = ps.tile([128, TOK], F32, tag="ps", bufs=8)
                    for kc in range(KC):
                        nc.tensor.matmul(p[:], w_sb[:, kc, h * 128 : (h + 1) * 128],
                                         xnT[:, kc, :], start=(kc == 0),
                                         stop=(kc == KC - 1))
                    if bias_sb is not None:
                        nc.scalar.activation(dst[:, h, :], p[:], AF.Identity,
                                             bias=bias_sb[:, h : h + 1])
                    else:
                        nc.vector.tensor_copy(out=dst[:, h, :], in_=p[:])
            vt = ap_.tile([T, NB, C], BF16, tag="vt")
            for b in range(NB):
                p = ps.tile([T, C], F32, tag="ps", bufs=8)
                for kc in range(KC):
                    nc.tensor.matmul(p[:], xnT[:, kc, b * T : (b + 1) * T],
                                     wv_sb[:, kc, :], start=(kc == 0),
                                     stop=(kc == KC - 1))
                if has_bv:
                    nc.vector.tensor_add(out=vt[:, b, :], in0=p[:], in1=bv_sb[:])
                else:
                    nc.vector.tensor_copy(out=vt[:, b, :], in_=p[:])

            # attention: scores [t, s] per (h, b), exp, mask, row-normalize
            ee = ap_.tile([T, H * NB, T], BF16, tag="ee")
            for h in range(H):
                p = ps.tile([T, NB, T], F32, tag="ps", bufs=8)
                for b in range(NB):
                    nc.tensor.matmul(p[:, b, :], qt[:, h, b * T : (b + 1) * T],
                                     kt[:, h, b * T : (b + 1) * T],
                                     start=True, stop=True)
                nc.scalar.activation(ee[:, h * NB : (h + 1) * NB, :], p[:],
                                     AF.Exp, scale=SCALE)
            nc.vector.tensor_mul(
                out=ee[:], in0=ee[:],
                in1=mask_sb[:].unsqueeze(1).to_broadcast([T, H * NB, T]))
            dsum = st.tile([T, H * NB], F32, tag="dsum")
            nc.vector.tensor_reduce(dsum[:], ee[:], axis=mybir.AxisListType.X,
                                    op=OP.add)
            rr = st.tile([T, H * NB], F32, tag="rr")
            nc.vector.reciprocal(rr[:], dsum[:])
            nc.vector.tensor_mul(
                out=ee[:], in0=ee[:],
                in1=rr[:].unsqueeze(2).to_broadcast([T, H * NB, T]))

            # transpose probs on PE, then attn @ V -> OT (T-layout, f32r)
            pt = ap_.tile([T, H * NB, T], BF16, tag="pt")
            for h in range(H):
                p = ps.tile([T, NB, T], BF16, tag="ps", bufs=8)
                for b in range(NB):
                    nc.tensor.transpose(p[:, b, :], ee[:, h * NB + b, :],
                                        ident_sb[:T, :T])
                nc.vector.tensor_copy(out=pt[:, h * NB : (h + 1) * NB, :], in_=p[:])
            ot = ap_.tile([128, H, TOK], F32R, tag="ot")
            for h in range(H):
                p = ps.tile([128, NB, T], F32, tag="ps", bufs=8)
                for b in range(NB):
                    nc.tensor.matmul(p[:, b, :], vt[:, b, h * 128 : (h + 1) * 128],
                                     pt[:, h * NB + b, :], start=True, stop=True)
                nc.scalar.activation(ot[:, h, :], p[:], AF.Identity)
            return x_sb, ot

        def stage_b(blk, x_sb, ot):
            """proj + residual, LN2, MLP, store."""
            row0 = blk * TOK
            x2 = ap_.tile([128, TCH, C], F32, tag="x2")
            sums2 = st.tile([128, TCH], F32, tag="bsums")
            sumsq2 = st.tile([128, TCH], F32, tag="bsumsq")
            for mc in range(TCH):
                p = ps.tile([128, C], F32, tag="ps", bufs=8)
                for kc in range(H):
                    nc.tensor.matmul(p[:], ot[:, kc, mc * 128 : (mc + 1) * 128],
                                     wp_sb[:, kc, :], start=(kc == 0),
                                     stop=(kc == H - 1))
                if has_bp:
                    nc.vector.tensor_add(out=p[:], in0=p[:], in1=bp_sb[:])
                # x2 = sa + x, with the LN2 row-sum accumulated for free
                nc.vector.scalar_tensor_tensor(
                    x2[:, mc, :], p[:], 1.0, x_sb[:, mc, :], OP.mult, OP.add,
                    accum_out=sums2[:, mc : mc + 1])
                scr2 = st.tile([128, C], BF16, tag="scr2", bufs=2)
                nc.scalar.activation(scr2[:], x2[:, mc, :], AF.Square,
                                     accum_out=sumsq2[:, mc : mc + 1])

            # MLP
            xn2 = ln_stats_apply(x2, "b", sums2, sumsq2)
            xn2T = transpose_xn(xn2, "b")
            ht = hp.tile([128, FM, TOK], F32R, tag="ht")
            for fm in range(FM):
                p = ps.tile([128, TOK], F32, tag="ps", bufs=8)
                for kc in range(KC):
                    nc.tensor.matmul(p[:], w1_sb[:, kc, fm * 128 : (fm + 1) * 128],
                                     xn2T[:, kc, :], start=(kc == 0),
                                     stop=(kc == KC - 1))
                nc.scalar.activation(ht[:, fm, :], p[:], AF.Gelu,
                                     bias=b1_sb[:, fm : fm + 1])
            xo = ap_.tile([128, TCH, C], F32, tag="xo")
            for mc in range(TCH):
                p = ps.tile([128, C], F32, tag="ps", bufs=8)
                for fk in range(FM):
                    nc.tensor.matmul(p[:], ht[:, fk, mc * 128 : (mc + 1) * 128],
                                     w2_sb[:, fk, :], start=(fk == 0),
                                     stop=(fk == FM - 1))
                if has_b2:
                    nc.vector.tensor_add(out=p[:], in0=p[:], in1=b2_sb[:])
                nc.vector.tensor_add(out=xo[:, mc, :], in0=p[:],
                                     in1=x2[:, mc, :])
            nc.sync.dma_start(
                y_d[row0 : row0 + TOK, :].rearrange("(ch p) c -> p ch c", p=128),
                xo[:])

        # Software-pipelined emission: stage A of block i+1 is emitted before
        # stage B of block i so each engine's (FIFO) instruction stream can
        # start the next block's independent front half while the current
        # block's back half waits on the proj dependency.
        pend = stage_a(0)
        for blk in range(1, nblk):
            nxt = stage_a(blk)
            stage_b(blk - 1, *pend)
            pend = nxt
        stage_b(nblk - 1, *pend)

    nc.compile()
    return nc


def fold(inputs):
    """Host-side exact folding of LN affines and biases into weights.

    Returns dict of staged arrays for the device program + bias flags.
    """
    f32 = np.float32
    g1 = inputs["g1"].astype(f32)
    be1 = inputs["be1"].astype(f32)
    g2 = inputs["g2"].astype(f32)
    be2 = inputs["be2"].astype(f32)

    def headcat(w):  # [H, C, D] -> [C, H*D]
        return np.concatenate([w[h] for h in range(H)], axis=1)

    wq = headcat(np.asarray(inputs["wq"], f32))
    wk = headcat(np.asarray(inputs["wk"], f32))
    wv = headcat(np.asarray(inputs["wv"], f32))
    wp_ = np.asarray(inputs["w_proj"], f32)
    w1 = np.asarray(inputs["w1"], f32)
    w2 = np.asarray(inputs["w2"], f32)

    wq_f = g1[:, None] * wq
    wk_f = g1[:, None] * wk
    wv_f = g1[:, None] * wv
    bq = be1 @ wq
    bk = be1 @ wk
    bv = be1 @ wv
    bp = np.asarray(inputs["b_proj"], f32)
    w1_f = g2[:, None] * w1
    b1 = np.asarray(inputs["b1"], f32) + be2 @ w1
    b2 = np.asarray(inputs["b2"], f32)

    mask = np.tril(np.ones((T, T), np.float32)).astype(ml_dtypes.bfloat16)
    ident = np.eye(128, dtype=ml_dtypes.bfloat16)

    staged = {
        "wq": wq_f.astype(ml_dtypes.bfloat16),
        "wk": wk_f.astype(ml_dtypes.bfloat16),
        "wv": wv_f.astype(ml_dtypes.bfloat16),
        "wp": wp_.astype(f32),
        "w1": w1_f.astype(ml_dtypes.bfloat16),
        "w2": w2.astype(f32),
        "b1": b1,
        "mask": mask,
        "ident": ident,
    }
    flags = {
        "has_bq": bool(np.any(bq)),
        "has_bk": bool(np.any(bk)),
        "has_bv": bool(np.any(bv)),
        "has_bp": bool(np.any(bp)),
        "has_b2": bool(np.any(b2)),
    }
    if flags["has_bq"]:
        staged["bq"] = bq
    if flags["has_bk"]:
        staged["bk"] = bk
    if flags["has_bv"]:
        staged["bv_b"] = np.broadcast_to(bv, (T, C)).copy()
    if flags["has_bp"]:
        staged["bp_b"] = np.broadcast_to(bp, (128, C)).copy()
    if flags["has_b2"]:
        staged["b2_b"] = np.broadcast_to(b2, (128, C)).copy()
    return staged, flags


_CACHE = {}


def kernel(**inputs):
    staged, flags = fold(inputs)
    key = tuple(sorted(flags.items()))
    if key not in _CACHE:
        _CACHE[key] = build(**flags)
    nc = _CACHE[key]

    x = np.asarray(inputs["x"], np.float32).reshape(B, T * C)
    in_maps = []
    for c in range(NCORES):
        m = dict(staged)
        m["x"] = x[c * SEQ_PER_CORE : (c + 1) * SEQ_PER_CORE].reshape(S, C)
        in_maps.append(m)

    res = bass_utils.run_bass_kernel_spmd(nc, in_maps, core_ids=list(range(NCORES)))
    out = np.concatenate([r["y"] for r in res.results], axis=0)
    return out.reshape(B, T, C).astype(np.float32)


# revision 19
# speedup vs baseline: 1.6624x; 1.0011x over previous
"""Fused transformer-block kernel for TRN2, 8-way data parallel over batch.

Layout strategy per core (128 sequences of 96 tokens = 12288 tokens):
  - Residual stream kept in N-layout [token_part, feature_free]; LayerNorm
    stats are free-dim reductions.
  - LN outputs written as bf16 and transposed to feature-major T-layout
    [feature_part, token_free] via DMA-xbar transposes; these feed the QKV
    and MLP1 matmuls (bf16).
  - Attention computed per (seq, head) with T=96 <= 128: scores in [t, s]
    layout (softmax over free dim), exp without max-subtraction (scores are
    bounded for this problem scale), 0/1 causal mask multiply, probs
    transposed on the PE, then attn@V gives head outputs directly in
    T-layout.
  - proj and MLP2 run in float32r (full PE speed at N=512, ~1e-4 rel err).
  - gamma/beta of both LNs and all biases are folded into the weight
    matrices / bias vectors on the host (exact algebra, see fold()).
"""

import sys

sys.path.insert(0, "/opt/trn_rl_repo")

from contextlib import ExitStack

import ml_dtypes
import numpy as np

import concourse.bass as bass  # noqa: F401  (registers AP types)
import concourse.tile as tile
from concourse import bacc, bass_utils, mybir

# Cache walrus-compiled NEFFs on disk keyed by BIR hash: re-running an
# unchanged program skips the multi-minute backend compile.
try:
    import hashlib
    import os as _os
    import shutil as _shutil

    import concourse.bass2jax as _b2j

    _orig_cbk = _b2j.compile_bir_kernel

    def _cached_cbk(bir_json, tmpdir, neff_name="file.neff"):
        try:
            raw = bir_json if isinstance(bir_json, bytes) else bir_json.encode()
            h = hashlib.sha256(raw).hexdigest()[:24]
            cdir = "/tmp/neff_cache"
            _os.makedirs(cdir, exist_ok=True)
            cpath = _os.path.join(cdir, h + ".neff")
            if _os.path.exists(cpath):
                return cpath
        except Exception:
            return _orig_cbk(bir_json, tmpdir, neff_name)
        p = _orig_cbk(bir_json, tmpdir, neff_name)
        try:
            _shutil.copy(p, cpath)
        except Exception:
            pass
        return p

    if _orig_cbk.__name__ != "_cached_cbk":
        _b2j.compile_bir_kernel = _cached_cbk
except Exception:
    pass

B, T, C = 1024, 96, 512
H, D = 4, 128
F = 4 * C
EPS = 1e-5
SCALE = D**-0.5

NCORES = 8
SEQ_PER_CORE = B // NCORES  # 128
S = SEQ_PER_CORE * T  # 12288 tokens per core
NB = 4  # sequences per block
TOK = NB * T  # 384 tokens per block
NBLK = SEQ_PER_CORE // NB  # 32 blocks
TCH = TOK // 128  # 3 token chunks per block
KC = C // 128  # 4 feature chunks of C
FM = F // 128  # 16 feature chunks of F

F32 = mybir.dt.float32
F32R = mybir.dt.float32r
BF16 = mybir.dt.bfloat16
AF = mybir.ActivationFunctionType
OP = mybir.AluOpType


def build(nblk=NBLK, has_bq=False, has_bk=False, has_bv=False, has_bp=False,
          has_b2=False):
    nc = bacc.Bacc("TRN2", target_bir_lowering=False, debug=False)

    def din(name, shape, dt):
        return nc.dram_tensor(name, shape, dt, kind="ExternalInput").ap()

    x_d = din("x", [S, C], F32)
    wq_d = din("wq", [C, C], BF16)
    wk_d = din("wk", [C, C], BF16)
    wv_d = din("wv", [C, C], BF16)
    wp_d = din("wp", [C, C], F32R)
    w1_d = din("w1", [C, F], BF16)
    w2_d = din("w2", [F, C], F32R)
    b1_d = din("b1", [F], F32)
    mask_d = din("mask", [T, T], BF16)
    ident_d = din("ident", [128, 128], BF16)
    bq_d = din("bq", [C], F32) if has_bq else None
    bk_d = din("bk", [C], F32) if has_bk else None
    bv_d = din("bv_b", [T, C], F32) if has_bv else None
    bp_d = din("bp_b", [128, C], F32) if has_bp else None
    b2_d = din("b2_b", [128, C], F32) if has_b2 else None
    y_d = nc.dram_tensor("y", [S, C], F32, kind="ExternalOutput").ap()

    with tile.TileContext(nc) as tc, ExitStack() as ctx:
        wp = ctx.enter_context(tc.tile_pool(name="wpool", bufs=1))
        ap_ = ctx.enter_context(tc.tile_pool(name="act", bufs=2))
        st = ctx.enter_context(tc.tile_pool(name="stat", bufs=3))
        hp = ctx.enter_context(tc.tile_pool(name="ht", bufs=1))
        ps = ctx.enter_context(tc.tile_pool(name="psum", bufs=1, space="PSUM"))

        # ---- resident weights ----
        def wload(name, d_ap, kchunks, fdim, dt):
            t = wp.tile([128, kchunks, fdim], dt, tag=name)
            nc.sync.dma_start(t[:], d_ap.rearrange("(kc p) f -> p kc f", p=128))
            return t

        wq_sb = wload("wq", wq_d, KC, C, BF16)
        wk_sb = wload("wk", wk_d, KC, C, BF16)
        wv_sb = wload("wv", wv_d, KC, C, BF16)
        wp_sb = wload("wp", wp_d, KC, C, F32R)
        w1_sb = wload("w1", w1_d, KC, F, BF16)
        w2_sb = wload("w2", w2_d, FM, C, F32R)

        b1_sb = wp.tile([128, FM], F32, tag="b1")
        nc.sync.dma_start(b1_sb[:], b1_d.rearrange("(fm p) -> p fm", p=128))
        mask_sb = wp.tile([T, T], BF16, tag="mask")
        nc.sync.dma_start(mask_sb[:], mask_d)
        ident_sb = wp.tile([128, 128], BF16, tag="ident")
        nc.sync.dma_start(ident_sb[:], ident_d)
        eps_sb = wp.tile([128, 1], F32, tag="eps")
        nc.vector.memset(eps_sb[:], EPS)
        if has_bq:
            bq_sb = wp.tile([128, H], F32, tag="bq")
            nc.sync.dma_start(bq_sb[:], bq_d.rearrange("(h d) -> d h", d=128))
        if has_bk:
            bk_sb = wp.tile([128, H], F32, tag="bk")
            nc.sync.dma_start(bk_sb[:], bk_d.rearrange("(h d) -> d h", d=128))
        if has_bv:
            bv_sb = wp.tile([T, C], F32, tag="bv")
            nc.sync.dma_start(bv_sb[:], bv_d)
        if has_bp:
            bp_sb = wp.tile([128, C], F32, tag="bp")
            nc.sync.dma_start(bp_sb[:], bp_d)
        if has_b2:
            b2_sb = wp.tile([128, C], F32, tag="b2")
            nc.sync.dma_start(b2_sb[:], b2_d)

        # ---- per-block helpers ----
        def ln_stats_apply(src, pref, sums, sumsq):
            """Finish LN given per-chunk sums/sumsq [128, TCH]; apply on ACT."""
            mu = st.tile([128, TCH], F32, tag=pref + "mu")
            nc.vector.tensor_scalar_mul(mu[:], sums[:], 1.0 / C)
            msq = st.tile([128, TCH], F32, tag=pref + "msq")
            nc.vector.tensor_mul(out=msq[:], in0=mu[:], in1=mu[:])
            var = st.tile([128, TCH], F32, tag=pref + "var")
            nc.vector.scalar_tensor_tensor(var[:], sumsq[:], 1.0 / C, msq[:],
                                           OP.mult, OP.subtract)
            std = st.tile([128, TCH], F32, tag=pref + "std")
            nc.scalar.activation(std[:], var[:], AF.Sqrt, bias=eps_sb[:, 0:1])
            rstd = st.tile([128, TCH], F32, tag=pref + "rstd")
            nc.vector.reciprocal(rstd[:], std[:])
            nmr = st.tile([128, TCH], F32, tag=pref + "nmr")
            nc.vector.scalar_tensor_tensor(nmr[:], mu[:], -1.0, rstd[:],
                                           OP.mult, OP.mult)
            xn = ap_.tile([128, TCH, C], BF16, tag=pref + "xn")
            for i in range(TCH):
                nc.scalar.activation(xn[:, i, :], src[:, i, :], AF.Identity,
                                     scale=rstd[:, i : i + 1],
                                     bias=nmr[:, i : i + 1])
            return xn

        def layer_norm(src, pref):
            """src: [128, TCH, C] f32 -> xn bf16 [128, TCH, C]."""
            sums = st.tile([128, TCH], F32, tag=pref + "sums")
            sumsq = st.tile([128, TCH], F32, tag=pref + "sumsq")
            nc.vector.tensor_reduce(sums[:], src[:], axis=mybir.AxisListType.X,
                                    op=OP.add)
            for i in range(TCH):
                scr = st.tile([128, C], BF16, tag="scr", bufs=2)
                nc.vector.scalar_tensor_tensor(
                    scr[:], src[:, i, :], 1.0, src[:, i, :], OP.mult, OP.mult,
                    accum_out=sumsq[:, i : i + 1])
            return ln_stats_apply(src, pref, sums, sumsq)

        def transpose_xn(xn, pref):
            """Transpose LN output to T-layout on the PE (bf16)."""
            xnT = ap_.tile([128, KC, TOK], BF16, tag=pref + "xnT")
            for kc in range(KC):
                p = ps.tile([128, TCH, 128], BF16, tag="ps", bufs=8, name="txp")
                for mc in range(TCH):
                    nc.tensor.transpose(p[:, mc, :],
                                        xn[:, mc, kc * 128 : (kc + 1) * 128],
                                        ident_sb[:])
                if kc % 2 == 0:
                    nc.scalar.activation(xnT[:, kc, :], p[:], AF.Identity)
                else:
                    nc.vector.tensor_copy(out=xnT[:, kc, :], in_=p[:])
            return xnT

        # ---- block stages ----
        def stage_a(blk):
            """x load, LN1, QKV, attention -> returns (x_sb, ot)."""
            row0 = blk * TOK
            x_sb = ap_.tile([128, TCH, C], F32, tag="x", bufs=2)
            nc.sync.dma_start(
                x_sb[:],
                x_d[row0 : row0 + TOK, :].rearrange("(ch p) c -> p ch c", p=128))

            xn = layer_norm(x_sb, "a")
            xnT = transpose_xn(xn, "a")

            # QKV projections (bf16)
            qt = ap_.tile([128, H, TOK], BF16, tag="qt")
            kt = ap_.tile([128, H, TOK], BF16, tag="kt")
            for dst, w_sb, bias_sb in ((qt, wq_sb, bq_sb if has_bq else None),
                                       (kt, wk_sb, bk_sb if has_bk else None)):
                for h in range(H):
                    p = ps.tile([128, TOK], F32, tag="ps", bufs=8)
                    for kc in range(KC):
                        nc.tensor.matmul(p[:], w_sb[:, kc, h * 128 : (h + 1) * 128],
                                         xnT[:, kc, :], start=(kc == 0),
                                         stop=(kc == KC - 1))
                    if bias_sb is not None:
                        nc.scalar.activation(dst[:, h, :], p[:], AF.Identity,
                                             bias=bias_sb[:, h : h + 1])
                    else:
                        nc.vector.tensor_copy(out=dst[:, h, :], in_=p[:])
            vt = ap_.tile([T, NB, C], BF16, tag="vt")
            for b in range(NB):
                p = ps.tile([T, C], F32, tag="ps", bufs=8)
                for kc in range(KC):
                    nc.tensor.matmul(p[:], xnT[:, kc, b * T : (b + 1) * T],
                                     wv_sb[:, kc, :], start=(kc == 0),
                                     stop=(kc == KC - 1))
                if has_bv:
                    nc.vector.tensor_add(out=vt[:, b, :], in0=p[:], in1=bv_sb[:])
                else:
                    nc.vector.tensor_copy(out=vt[:, b, :], in_=p[:])

            # attention: scores [t, s] per (h, b), exp, mask, row-normalize
            ee = ap_.tile([T, H * NB, T], BF16, tag="ee")
            for h in range(H):
                p = ps.tile([T, NB, T], F32, tag="ps", bufs=8)
                for b in range(NB):
                    nc.tensor.matmul(p[:, b, :], qt[:, h, b * T : (b + 1) * T],
                                     kt[:, h, b * T : (b + 1) * T],
                                     start=True, stop=True)
                nc.scalar.activation(ee[:, h * NB : (h + 1) * NB, :], p[:],
                                     AF.Exp, scale=SCALE)
            nc.vector.tensor_mul(
                out=ee[:], in0=ee[:],
                in1=mask_sb[:].unsqueeze(1).to_broadcast([T, H * NB, T]))
            dsum = st.tile([T, H * NB], F32, tag="dsum")
            nc.vector.tensor_reduce(dsum[:], ee[:], axis=mybir.AxisListType.X,
                                    op=OP.add)
            rr = st.tile([T, H * NB], F32, tag="rr")
            nc.vector.reciprocal(rr[:], dsum[:])
            nc.vector.tensor_mul(
                out=ee[:], in0=ee[:],
                in1=rr[:].unsqueeze(2).to_broadcast([T, H * NB, T]))

            # transpose probs on PE, then attn @ V -> OT (T-layout, f32r)
            pt = ap_.tile([T, H * NB, T], BF16, tag="pt")
            for h in range(H):
                p = ps.tile([T, NB, T], BF16, tag="ps", bufs=8)
                for b in range(NB):
                    nc.tensor.transpose(p[:, b, :], ee[:, h * NB + b, :],
                                        ident_sb[:T, :T])
                nc.vector.tensor_copy(out=pt[:, h * NB : (h + 1) * NB, :], in_=p[:])
            ot = ap_.tile([128, H, TOK], F32R, tag="ot")
            for h in range(H):
                p = ps.tile([128, NB, T], F32, tag="ps", bufs=8)
                for b in range(NB):
                    nc.tensor.matmul(p[:, b, :], vt[:, b, h * 128 : (h + 1) * 128],
                                     pt[:, h * NB + b, :], start=True, stop=True)
                nc.vector.tensor_copy(out=ot[:, h, :], in_=p[:])
            return x_sb, ot

        def stage_b(blk, x_sb, ot):
            """proj + residual, LN2, MLP, store."""
            row0 = blk * TOK
            x2 = ap_.tile([128, TCH, C], F32, tag="x2")
            sums2 = st.tile([128, TCH], F32, tag="bsums")
            sumsq2 = st.tile([128, TCH], F32, tag="bsumsq")
            for mc in range(TCH):
                p = ps.tile([128, C], F32, tag="ps", bufs=8)
                for kc in range(H):
                    nc.tensor.matmul(p[:], ot[:, kc, mc * 128 : (mc + 1) * 128],
                                     wp_sb[:, kc, :], start=(kc == 0),
                                     stop=(kc == H - 1))
                if has_bp:
                    nc.vector.tensor_add(out=p[:], in0=p[:], in1=bp_sb[:])
                # x2 = sa + x, with the LN2 row-sum accumulated for free
                nc.vector.scalar_tensor_tensor(
                    x2[:, mc, :], p[:], 1.0, x_sb[:, mc, :], OP.mult, OP.add,
                    accum_out=sums2[:, mc : mc + 1])
                scr2 = st.tile([128, C], BF16, tag="scr2", bufs=2)
                nc.vector.scalar_tensor_tensor(
                    scr2[:], x2[:, mc, :], 1.0, x2[:, mc, :], OP.mult, OP.mult,
                    accum_out=sumsq2[:, mc : mc + 1])

            # MLP
            xn2 = ln_stats_apply(x2, "b", sums2, sumsq2)
            xn2T = transpose_xn(xn2, "b")
            ht = hp.tile([128, FM, TOK], F32R, tag="ht")
            for fm in range(FM):
                p = ps.tile([128, TOK], F32, tag="ps", bufs=8)
                for kc in range(KC):
                    nc.tensor.matmul(p[:], w1_sb[:, kc, fm * 128 : (fm + 1) * 128],
                                     xn2T[:, kc, :], start=(kc == 0),
                                     stop=(kc == KC - 1))
                nc.scalar.activation(ht[:, fm, :], p[:], AF.Gelu,
                                     bias=b1_sb[:, fm : fm + 1])
            xo = ap_.tile([128, TCH, C], F32, tag="xo")
            for mc in range(TCH):
                p = ps.tile([128, C], F32, tag="ps", bufs=8)
                for fk in range(FM):
                    nc.tensor.matmul(p[:], ht[:, fk, mc * 128 : (mc + 1) * 128],
                                     w2_sb[:, fk, :], start=(fk == 0),
                                     stop=(fk == FM - 1))
                if has_b2:
                    nc.vector.tensor_add(out=p[:], in0=p[:], in1=b2_sb[:])
                nc.vector.tensor_add(out=xo[:, mc, :], in0=p[:],
                                     in1=x2[:, mc, :])
            nc.sync.dma_start(
                y_d[row0 : row0 + TOK, :].rearrange("(ch p) c -> p ch c", p=128),
                xo[:])

        # Software-pipelined emission: stage A of block i+1 is emitted before
        # stage B of block i so each engine's (FIFO) instruction stream can
        # start the next block's independent front half while the current
        # block's back half waits on the proj dependency.
        pend = stage_a(0)
        for blk in range(1, nblk):
            nxt = stage_a(blk)
            stage_b(blk - 1, *pend)
            pend = nxt
        stage_b(nblk - 1, *pend)

    nc.compile()
    return nc


def fold(inputs):
    """Host-side exact folding of LN affines and biases into weights.

    Returns dict of staged arrays for the device program + bias flags.
    """
    f32 = np.float32
    g1 = inputs["g1"].astype(f32)
    be1 = inputs["be1"].astype(f32)
    g2 = inputs["g2"].astype(f32)
    be2 = inputs["be2"].astype(f32)

    def headcat(w):  # [H, C, D] -> [C, H*D]
        return np.concatenate([w[h] for h in range(H)], axis=1)

    wq = headcat(np.asarray(inputs["wq"], f32))
    wk = headcat(np.asarray(inputs["wk"], f32))
    wv = headcat(np.asarray(inputs["wv"], f32))
    wp_ = np.asarray(inputs["w_proj"], f32)
    w1 = np.asarray(inputs["w1"], f32)
    w2 = np.asarray(inputs["w2"], f32)

    wq_f = g1[:, None] * wq
    wk_f = g1[:, None] * wk
    wv_f = g1[:, None] * wv
    bq = be1 @ wq
    bk = be1 @ wk
    bv = be1 @ wv
    bp = np.asarray(inputs["b_proj"], f32)
    w1_f = g2[:, None] * w1
    b1 = np.asarray(inputs["b1"], f32) + be2 @ w1
    b2 = np.asarray(inputs["b2"], f32)

    mask = np.tril(np.ones((T, T), np.float32)).astype(ml_dtypes.bfloat16)
    ident = np.eye(128, dtype=ml_dtypes.bfloat16)

    staged = {
        "wq": wq_f.astype(ml_dtypes.bfloat16),
        "wk": wk_f.astype(ml_dtypes.bfloat16),
        "wv": wv_f.astype(ml_dtypes.bfloat16),
        "wp": wp_.astype(f32),
        "w1": w1_f.astype(ml_dtypes.bfloat16),
        "w2": w2.astype(f32),
        "b1": b1,
        "mask": mask,
        "ident": ident,
    }
    flags = {
        "has_bq": bool(np.any(bq)),
        "has_bk": bool(np.any(bk)),
        "has_bv": bool(np.any(bv)),
        "has_bp": bool(np.any(bp)),
        "has_b2": bool(np.any(b2)),
    }
    if flags["has_bq"]:
        staged["bq"] = bq
    if flags["has_bk"]:
        staged["bk"] = bk
    if flags["has_bv"]:
        staged["bv_b"] = np.broadcast_to(bv, (T, C)).copy()
    if flags["has_bp"]:
        staged["bp_b"] = np.broadcast_to(bp, (128, C)).copy()
    if flags["has_b2"]:
        staged["b2_b"] = np.broadcast_to(b2, (128, C)).copy()
    return staged, flags


_CACHE = {}


def kernel(**inputs):
    staged, flags = fold(inputs)
    key = tuple(sorted(flags.items()))
    if key not in _CACHE:
        _CACHE[key] = build(**flags)
    nc = _CACHE[key]

    x = np.asarray(inputs["x"], np.float32).reshape(B, T * C)
    in_maps = []
    for c in range(NCORES):
        m = dict(staged)
        m["x"] = x[c * SEQ_PER_CORE : (c + 1) * SEQ_PER_CORE].reshape(S, C)
        in_maps.append(m)

    res = bass_utils.run_bass_kernel_spmd(nc, in_maps, core_ids=list(range(NCORES)))
    out = np.concatenate([r["y"] for r in res.results], axis=0)
    return out.reshape(B, T, C).astype(np.float32)


# revision 20
# speedup vs baseline: 1.7615x; 1.0596x over previous
"""Fused transformer-block kernel for TRN2, 8-way data parallel over batch.

Layout strategy per core (128 sequences of 96 tokens = 12288 tokens):
  - Residual stream kept in N-layout [token_part, feature_free]; LayerNorm
    stats are free-dim reductions.
  - LN outputs written as bf16 and transposed to feature-major T-layout
    [feature_part, token_free] via DMA-xbar transposes; these feed the QKV
    and MLP1 matmuls (bf16).
  - Attention computed per (seq, head) with T=96 <= 128: scores in [t, s]
    layout (softmax over free dim), exp without max-subtraction (scores are
    bounded for this problem scale), 0/1 causal mask multiply, probs
    transposed on the PE, then attn@V gives head outputs directly in
    T-layout.
  - proj and MLP2 run in float32r (full PE speed at N=512, ~1e-4 rel err).
  - gamma/beta of both LNs and all biases are folded into the weight
    matrices / bias vectors on the host (exact algebra, see fold()).
"""

import sys

sys.path.insert(0, "/opt/trn_rl_repo")

from contextlib import ExitStack

import ml_dtypes
import numpy as np

import concourse.bass as bass  # noqa: F401  (registers AP types)
import concourse.tile as tile
from concourse import bacc, bass_utils, mybir

# Cache walrus-compiled NEFFs on disk keyed by BIR hash: re-running an
# unchanged program skips the multi-minute backend compile.
try:
    import hashlib
    import os as _os
    import shutil as _shutil

    import concourse.bass2jax as _b2j

    _orig_cbk = _b2j.compile_bir_kernel

    def _cached_cbk(bir_json, tmpdir, neff_name="file.neff"):
        try:
            raw = bir_json if isinstance(bir_json, bytes) else bir_json.encode()
            h = hashlib.sha256(raw).hexdigest()[:24]
            cdir = "/tmp/neff_cache"
            _os.makedirs(cdir, exist_ok=True)
            cpath = _os.path.join(cdir, h + ".neff")
            if _os.path.exists(cpath):
                return cpath
        except Exception:
            return _orig_cbk(bir_json, tmpdir, neff_name)
        p = _orig_cbk(bir_json, tmpdir, neff_name)
        try:
            _shutil.copy(p, cpath)
        except Exception:
            pass
        return p

    if _orig_cbk.__name__ != "_cached_cbk":
        _b2j.compile_bir_kernel = _cached_cbk
except Exception:
    pass

B, T, C = 1024, 96, 512
H, D = 4, 128
F = 4 * C
EPS = 1e-5
SCALE = D**-0.5

NCORES = 8
SEQ_PER_CORE = B // NCORES  # 128
S = SEQ_PER_CORE * T  # 12288 tokens per core
NB = 4  # sequences per block
TOK = NB * T  # 384 tokens per block
NBLK = SEQ_PER_CORE // NB  # 32 blocks
TCH = TOK // 128  # 3 token chunks per block
KC = C // 128  # 4 feature chunks of C
FM = F // 128  # 16 feature chunks of F

F32 = mybir.dt.float32
F32R = mybir.dt.float32r
BF16 = mybir.dt.bfloat16
AF = mybir.ActivationFunctionType
OP = mybir.AluOpType


def build(nblk=NBLK, has_bq=False, has_bk=False, has_bv=False, has_bp=False,
          has_b2=False):
    nc = bacc.Bacc("TRN2", target_bir_lowering=False, debug=False)

    def din(name, shape, dt):
        return nc.dram_tensor(name, shape, dt, kind="ExternalInput").ap()

    x_d = din("x", [S, C], F32)
    wq_d = din("wq", [C, C], BF16)
    wk_d = din("wk", [C, C], BF16)
    wv_d = din("wv", [C, C], BF16)
    wp_d = din("wp", [C, C], F32R)
    w1_d = din("w1", [C, F], BF16)
    w2_d = din("w2", [F, C], F32R)
    b1_d = din("b1", [F], F32)
    mask_d = din("mask", [T, T], BF16)
    ident_d = din("ident", [128, 128], BF16)
    bq_d = din("bq", [C], F32) if has_bq else None
    bk_d = din("bk", [C], F32) if has_bk else None
    bv_d = din("bv_b", [T, C], F32) if has_bv else None
    bp_d = din("bp_b", [128, C], F32) if has_bp else None
    b2_d = din("b2_b", [128, C], F32) if has_b2 else None
    y_d = nc.dram_tensor("y", [S, C], F32, kind="ExternalOutput").ap()

    with tile.TileContext(nc) as tc, ExitStack() as ctx:
        wp = ctx.enter_context(tc.tile_pool(name="wpool", bufs=1))
        ap_ = ctx.enter_context(tc.tile_pool(name="act", bufs=2))
        st = ctx.enter_context(tc.tile_pool(name="stat", bufs=3))
        hp = ctx.enter_context(tc.tile_pool(name="ht", bufs=1))
        ps = ctx.enter_context(tc.tile_pool(name="psum", bufs=1, space="PSUM"))

        # ---- resident weights ----
        def wload(name, d_ap, kchunks, fdim, dt):
            t = wp.tile([128, kchunks, fdim], dt, tag=name)
            nc.sync.dma_start(t[:], d_ap.rearrange("(kc p) f -> p kc f", p=128))
            return t

        wq_sb = wload("wq", wq_d, KC, C, BF16)
        wk_sb = wload("wk", wk_d, KC, C, BF16)
        wv_sb = wload("wv", wv_d, KC, C, BF16)
        wp_sb = wload("wp", wp_d, KC, C, F32R)
        w1_sb = wload("w1", w1_d, KC, F, BF16)
        w2_sb = wload("w2", w2_d, FM, C, F32R)

        b1_sb = wp.tile([128, FM], F32, tag="b1")
        nc.sync.dma_start(b1_sb[:], b1_d.rearrange("(fm p) -> p fm", p=128))
        mask_sb = wp.tile([T, T], BF16, tag="mask")
        nc.sync.dma_start(mask_sb[:], mask_d)
        ident_sb = wp.tile([128, 128], BF16, tag="ident")
        nc.sync.dma_start(ident_sb[:], ident_d)
        eps_sb = wp.tile([128, 1], F32, tag="eps")
        nc.vector.memset(eps_sb[:], EPS)
        if has_bq:
            bq_sb = wp.tile([128, H], F32, tag="bq")
            nc.sync.dma_start(bq_sb[:], bq_d.rearrange("(h d) -> d h", d=128))
        if has_bk:
            bk_sb = wp.tile([128, H], F32, tag="bk")
            nc.sync.dma_start(bk_sb[:], bk_d.rearrange("(h d) -> d h", d=128))
        if has_bv:
            bv_sb = wp.tile([T, C], F32, tag="bv")
            nc.sync.dma_start(bv_sb[:], bv_d)
        if has_bp:
            bp_sb = wp.tile([128, C], F32, tag="bp")
            nc.sync.dma_start(bp_sb[:], bp_d)
        if has_b2:
            b2_sb = wp.tile([128, C], F32, tag="b2")
            nc.sync.dma_start(b2_sb[:], b2_d)

        # ---- per-block helpers ----
        def ln_stats_apply(src, pref, sums, sumsq):
            """Finish LN given per-chunk sums/sumsq [128, TCH]; apply on ACT."""
            mu = st.tile([128, TCH], F32, tag=pref + "mu")
            nc.vector.tensor_scalar_mul(mu[:], sums[:], 1.0 / C)
            msq = st.tile([128, TCH], F32, tag=pref + "msq")
            nc.vector.tensor_mul(out=msq[:], in0=mu[:], in1=mu[:])
            var = st.tile([128, TCH], F32, tag=pref + "var")
            nc.vector.scalar_tensor_tensor(var[:], sumsq[:], 1.0 / C, msq[:],
                                           OP.mult, OP.subtract)
            std = st.tile([128, TCH], F32, tag=pref + "std")
            nc.scalar.activation(std[:], var[:], AF.Sqrt, bias=eps_sb[:, 0:1])
            rstd = st.tile([128, TCH], F32, tag=pref + "rstd")
            nc.vector.reciprocal(rstd[:], std[:])
            nmr = st.tile([128, TCH], F32, tag=pref + "nmr")
            nc.vector.scalar_tensor_tensor(nmr[:], mu[:], -1.0, rstd[:],
                                           OP.mult, OP.mult)
            xn = ap_.tile([128, TCH, C], BF16, tag=pref + "xn")
            for i in range(TCH):
                nc.scalar.activation(xn[:, i, :], src[:, i, :], AF.Identity,
                                     scale=rstd[:, i : i + 1],
                                     bias=nmr[:, i : i + 1])
            return xn

        def layer_norm(src, pref):
            """src: [128, TCH, C] f32 -> xn bf16 [128, TCH, C]."""
            sums = st.tile([128, TCH], F32, tag=pref + "sums")
            sumsq = st.tile([128, TCH], F32, tag=pref + "sumsq")
            nc.vector.tensor_reduce(sums[:], src[:], axis=mybir.AxisListType.X,
                                    op=OP.add)
            for i in range(TCH):
                scr = st.tile([128, C], BF16, tag="scr", bufs=2)
                nc.vector.scalar_tensor_tensor(
                    scr[:], src[:, i, :], 1.0, src[:, i, :], OP.mult, OP.mult,
                    accum_out=sumsq[:, i : i + 1])
            return ln_stats_apply(src, pref, sums, sumsq)

        def transpose_xn(xn, pref, ptag):
            """Transpose LN output to T-layout on the PE (bf16)."""
            xnT = ap_.tile([128, KC, TOK], BF16, tag=pref + "xnT")
            for kc in range(KC):
                p = ps.tile([128, TCH, 128], BF16, tag=ptag, bufs=4, name="txp")
                for mc in range(TCH):
                    nc.tensor.transpose(p[:, mc, :],
                                        xn[:, mc, kc * 128 : (kc + 1) * 128],
                                        ident_sb[:])
                if kc % 2 == 0:
                    nc.scalar.activation(xnT[:, kc, :], p[:], AF.Identity)
                else:
                    nc.vector.tensor_copy(out=xnT[:, kc, :], in_=p[:])
            return xnT

        # ---- block stages ----
        def stage_a1(blk):
            """x load, LN1, transpose -> (x_sb, xnT)."""
            row0 = blk * TOK
            x_sb = ap_.tile([128, TCH, C], F32, tag="x", bufs=3)
            nc.sync.dma_start(
                x_sb[:],
                x_d[row0 : row0 + TOK, :].rearrange("(ch p) c -> p ch c", p=128))
            xn = layer_norm(x_sb, "a")
            xnT = transpose_xn(xn, "a", "pa")
            return x_sb, xnT

        def stage_a2(blk, xnT):
            """QKV + attention -> ot."""
            # QKV projections (bf16)
            qt = ap_.tile([128, H, TOK], BF16, tag="qt")
            kt = ap_.tile([128, H, TOK], BF16, tag="kt")
            for dst, w_sb, bias_sb in ((qt, wq_sb, bq_sb if has_bq else None),
                                       (kt, wk_sb, bk_sb if has_bk else None)):
                for h in range(H):
                    p = ps.tile([128, TOK], F32, tag="pa", bufs=4)
                    for kc in range(KC):
                        nc.tensor.matmul(p[:], w_sb[:, kc, h * 128 : (h + 1) * 128],
                                         xnT[:, kc, :], start=(kc == 0),
                                         stop=(kc == KC - 1))
                    if bias_sb is not None:
                        nc.scalar.activation(dst[:, h, :], p[:], AF.Identity,
                                             bias=bias_sb[:, h : h + 1])
                    else:
                        nc.vector.tensor_copy(out=dst[:, h, :], in_=p[:])
            vt = ap_.tile([T, NB, C], BF16, tag="vt")
            for b in range(NB):
                p = ps.tile([T, C], F32, tag="pa", bufs=4)
                for kc in range(KC):
                    nc.tensor.matmul(p[:], xnT[:, kc, b * T : (b + 1) * T],
                                     wv_sb[:, kc, :], start=(kc == 0),
                                     stop=(kc == KC - 1))
                if has_bv:
                    nc.vector.tensor_add(out=vt[:, b, :], in0=p[:], in1=bv_sb[:])
                else:
                    nc.vector.tensor_copy(out=vt[:, b, :], in_=p[:])

            # attention: scores [t, s] per (h, b), exp, mask, row-normalize
            ee = ap_.tile([T, H * NB, T], BF16, tag="ee")
            for h in range(H):
                p = ps.tile([T, NB, T], F32, tag="pa", bufs=4)
                for b in range(NB):
                    nc.tensor.matmul(p[:, b, :], qt[:, h, b * T : (b + 1) * T],
                                     kt[:, h, b * T : (b + 1) * T],
                                     start=True, stop=True)
                nc.scalar.activation(ee[:, h * NB : (h + 1) * NB, :], p[:],
                                     AF.Exp, scale=SCALE)
            nc.vector.tensor_mul(
                out=ee[:], in0=ee[:],
                in1=mask_sb[:].unsqueeze(1).to_broadcast([T, H * NB, T]))
            dsum = st.tile([T, H * NB], F32, tag="dsum")
            nc.vector.tensor_reduce(dsum[:], ee[:], axis=mybir.AxisListType.X,
                                    op=OP.add)
            rr = st.tile([T, H * NB], F32, tag="rr")
            nc.vector.reciprocal(rr[:], dsum[:])
            nc.vector.tensor_mul(
                out=ee[:], in0=ee[:],
                in1=rr[:].unsqueeze(2).to_broadcast([T, H * NB, T]))

            # transpose probs on PE, then attn @ V -> OT (T-layout, f32r)
            pt = ee  # probs are overwritten in place by their transpose
            for h in range(H):
                p = ps.tile([T, NB, T], BF16, tag="pa", bufs=4)
                for b in range(NB):
                    nc.tensor.transpose(p[:, b, :], ee[:, h * NB + b, :],
                                        ident_sb[:T, :T])
                nc.vector.tensor_copy(out=pt[:, h * NB : (h + 1) * NB, :], in_=p[:])
            ot = ap_.tile([128, H, TOK], F32R, tag="ot")
            for h in range(H):
                p = ps.tile([128, NB, T], F32, tag="pa", bufs=4)
                for b in range(NB):
                    nc.tensor.matmul(p[:, b, :], vt[:, b, h * 128 : (h + 1) * 128],
                                     pt[:, h * NB + b, :], start=True, stop=True)
                nc.vector.tensor_copy(out=ot[:, h, :], in_=p[:])
            return ot

        def stage_b(blk, x_sb, ot):
            """proj + residual, LN2, MLP, store."""
            row0 = blk * TOK
            x2 = ap_.tile([128, TCH, C], F32, tag="x2")
            sums2 = st.tile([128, TCH], F32, tag="bsums")
            sumsq2 = st.tile([128, TCH], F32, tag="bsumsq")
            for mc in range(TCH):
                p = ps.tile([128, C], F32, tag="pb", bufs=4)
                for kc in range(H):
                    nc.tensor.matmul(p[:], ot[:, kc, mc * 128 : (mc + 1) * 128],
                                     wp_sb[:, kc, :], start=(kc == 0),
                                     stop=(kc == H - 1))
                if has_bp:
                    nc.vector.tensor_add(out=p[:], in0=p[:], in1=bp_sb[:])
                # x2 = sa + x, with the LN2 row-sum accumulated for free
                nc.vector.scalar_tensor_tensor(
                    x2[:, mc, :], p[:], 1.0, x_sb[:, mc, :], OP.mult, OP.add,
                    accum_out=sums2[:, mc : mc + 1])
                scr2 = st.tile([128, C], BF16, tag="scr2", bufs=2)
                nc.vector.scalar_tensor_tensor(
                    scr2[:], x2[:, mc, :], 1.0, x2[:, mc, :], OP.mult, OP.mult,
                    accum_out=sumsq2[:, mc : mc + 1])

            # MLP
            xn2 = ln_stats_apply(x2, "b", sums2, sumsq2)
            xn2T = transpose_xn(xn2, "b", "pb")
            ht = hp.tile([128, FM, TOK], F32R, tag="ht")
            for fm in range(FM):
                p = ps.tile([128, TOK], F32, tag="pb", bufs=4)
                for kc in range(KC):
                    nc.tensor.matmul(p[:], w1_sb[:, kc, fm * 128 : (fm + 1) * 128],
                                     xn2T[:, kc, :], start=(kc == 0),
                                     stop=(kc == KC - 1))
                nc.scalar.activation(ht[:, fm, :], p[:], AF.Gelu,
                                     bias=b1_sb[:, fm : fm + 1])
            xo = ap_.tile([128, TCH, C], F32, tag="xo")
            for mc in range(TCH):
                p = ps.tile([128, C], F32, tag="pb", bufs=4)
                for fk in range(FM):
                    nc.tensor.matmul(p[:], ht[:, fk, mc * 128 : (mc + 1) * 128],
                                     w2_sb[:, fk, :], start=(fk == 0),
                                     stop=(fk == FM - 1))
                if has_b2:
                    nc.vector.tensor_add(out=p[:], in0=p[:], in1=b2_sb[:])
                nc.vector.tensor_add(out=xo[:, mc, :], in0=p[:],
                                     in1=x2[:, mc, :])
            nc.sync.dma_start(
                y_d[row0 : row0 + TOK, :].rearrange("(ch p) c -> p ch c", p=128),
                xo[:])

        # Software-pipelined emission (A1 two blocks ahead, A2 one ahead):
        # each engine's FIFO stream then has the next blocks' independent
        # front-half work queued before the current block's back half, so
        # nothing stalls behind the proj/LN2 dependency chains.
        xs, xnTs, ots = {}, {}, {}
        xs[0], xnTs[0] = stage_a1(0)
        if nblk > 1:
            xs[1], xnTs[1] = stage_a1(1)
        ots[0] = stage_a2(0, xnTs.pop(0))
        for blk in range(1, nblk):
            if blk + 1 < nblk:
                xs[blk + 1], xnTs[blk + 1] = stage_a1(blk + 1)
            ots[blk] = stage_a2(blk, xnTs.pop(blk))
            stage_b(blk - 1, xs.pop(blk - 1), ots.pop(blk - 1))
        stage_b(nblk - 1, xs.pop(nblk - 1), ots.pop(nblk - 1))

    nc.compile()
    return nc


def fold(inputs):
    """Host-side exact folding of LN affines and biases into weights.

    Returns dict of staged arrays for the device program + bias flags.
    """
    f32 = np.float32
    g1 = inputs["g1"].astype(f32)
    be1 = inputs["be1"].astype(f32)
    g2 = inputs["g2"].astype(f32)
    be2 = inputs["be2"].astype(f32)

    def headcat(w):  # [H, C, D] -> [C, H*D]
        return np.concatenate([w[h] for h in range(H)], axis=1)

    wq = headcat(np.asarray(inputs["wq"], f32))
    wk = headcat(np.asarray(inputs["wk"], f32))
    wv = headcat(np.asarray(inputs["wv"], f32))
    wp_ = np.asarray(inputs["w_proj"], f32)
    w1 = np.asarray(inputs["w1"], f32)
    w2 = np.asarray(inputs["w2"], f32)

    wq_f = g1[:, None] * wq
    wk_f = g1[:, None] * wk
    wv_f = g1[:, None] * wv
    bq = be1 @ wq
    bk = be1 @ wk
    bv = be1 @ wv
    bp = np.asarray(inputs["b_proj"], f32)
    w1_f = g2[:, None] * w1
    b1 = np.asarray(inputs["b1"], f32) + be2 @ w1
    b2 = np.asarray(inputs["b2"], f32)

    mask = np.tril(np.ones((T, T), np.float32)).astype(ml_dtypes.bfloat16)
    ident = np.eye(128, dtype=ml_dtypes.bfloat16)

    staged = {
        "wq": wq_f.astype(ml_dtypes.bfloat16),
        "wk": wk_f.astype(ml_dtypes.bfloat16),
        "wv": wv_f.astype(ml_dtypes.bfloat16),
        "wp": wp_.astype(f32),
        "w1": w1_f.astype(ml_dtypes.bfloat16),
        "w2": w2.astype(f32),
        "b1": b1,
        "mask": mask,
        "ident": ident,
    }
    flags = {
        "has_bq": bool(np.any(bq)),
        "has_bk": bool(np.any(bk)),
        "has_bv": bool(np.any(bv)),
        "has_bp": bool(np.any(bp)),
        "has_b2": bool(np.any(b2)),
    }
    if flags["has_bq"]:
        staged["bq"] = bq
    if flags["has_bk"]:
        staged["bk"] = bk
    if flags["has_bv"]:
        staged["bv_b"] = np.broadcast_to(bv, (T, C)).copy()
    if flags["has_bp"]:
        staged["bp_b"] = np.broadcast_to(bp, (128, C)).copy()
    if flags["has_b2"]:
        staged["b2_b"] = np.broadcast_to(b2, (128, C)).copy()
    return staged, flags


_CACHE = {}


def kernel(**inputs):
    staged, flags = fold(inputs)
    key = tuple(sorted(flags.items()))
    if key not in _CACHE:
        _CACHE[key] = build(**flags)
    nc = _CACHE[key]

    x = np.asarray(inputs["x"], np.float32).reshape(B, T * C)
    in_maps = []
    for c in range(NCORES):
        m = dict(staged)
        m["x"] = x[c * SEQ_PER_CORE : (c + 1) * SEQ_PER_CORE].reshape(S, C)
        in_maps.append(m)

    res = bass_utils.run_bass_kernel_spmd(nc, in_maps, core_ids=list(range(NCORES)))
    out = np.concatenate([r["y"] for r in res.results], axis=0)
    return out.reshape(B, T, C).astype(np.float32)


# revision 21
# speedup vs baseline: 1.7840x; 1.0128x over previous
"""Fused transformer-block kernel for TRN2, 8-way data parallel over batch.

Layout strategy per core (128 sequences of 96 tokens = 12288 tokens):
  - Residual stream kept in N-layout [token_part, feature_free]; LayerNorm
    stats are free-dim reductions.
  - LN outputs written as bf16 and transposed to feature-major T-layout
    [feature_part, token_free] via DMA-xbar transposes; these feed the QKV
    and MLP1 matmuls (bf16).
  - Attention computed per (seq, head) with T=96 <= 128: scores in [t, s]
    layout (softmax over free dim), exp without max-subtraction (scores are
    bounded for this problem scale), 0/1 causal mask multiply, probs
    transposed on the PE, then attn@V gives head outputs directly in
    T-layout.
  - proj and MLP2 run in float32r (full PE speed at N=512, ~1e-4 rel err).
  - gamma/beta of both LNs and all biases are folded into the weight
    matrices / bias vectors on the host (exact algebra, see fold()).
"""

import sys

sys.path.insert(0, "/opt/trn_rl_repo")

from contextlib import ExitStack

import ml_dtypes
import numpy as np

import concourse.bass as bass  # noqa: F401  (registers AP types)
import concourse.tile as tile
from concourse import bacc, bass_utils, mybir

# Cache walrus-compiled NEFFs on disk keyed by BIR hash: re-running an
# unchanged program skips the multi-minute backend compile.
try:
    import hashlib
    import os as _os
    import shutil as _shutil

    import concourse.bass2jax as _b2j

    _orig_cbk = _b2j.compile_bir_kernel

    def _cached_cbk(bir_json, tmpdir, neff_name="file.neff"):
        try:
            raw = bir_json if isinstance(bir_json, bytes) else bir_json.encode()
            h = hashlib.sha256(raw).hexdigest()[:24]
            cdir = "/tmp/neff_cache"
            _os.makedirs(cdir, exist_ok=True)
            cpath = _os.path.join(cdir, h + ".neff")
            if _os.path.exists(cpath):
                return cpath
        except Exception:
            return _orig_cbk(bir_json, tmpdir, neff_name)
        p = _orig_cbk(bir_json, tmpdir, neff_name)
        try:
            _shutil.copy(p, cpath)
        except Exception:
            pass
        return p

    if _orig_cbk.__name__ != "_cached_cbk":
        _b2j.compile_bir_kernel = _cached_cbk
except Exception:
    pass

B, T, C = 1024, 96, 512
H, D = 4, 128
F = 4 * C
EPS = 1e-5
SCALE = D**-0.5

NCORES = 8
SEQ_PER_CORE = B // NCORES  # 128
S = SEQ_PER_CORE * T  # 12288 tokens per core
NB = 4  # sequences per block
TOK = NB * T  # 384 tokens per block
NBLK = SEQ_PER_CORE // NB  # 32 blocks
TCH = TOK // 128  # 3 token chunks per block
KC = C // 128  # 4 feature chunks of C
FM = F // 128  # 16 feature chunks of F

F32 = mybir.dt.float32
F32R = mybir.dt.float32r
BF16 = mybir.dt.bfloat16
AF = mybir.ActivationFunctionType
OP = mybir.AluOpType


def build(nblk=NBLK, has_bq=False, has_bk=False, has_bv=False, has_bp=False,
          has_b2=False):
    nc = bacc.Bacc("TRN2", target_bir_lowering=False, debug=False)

    def din(name, shape, dt):
        return nc.dram_tensor(name, shape, dt, kind="ExternalInput").ap()

    x_d = din("x", [S, C], F32)
    wq_d = din("wq", [C, C], BF16)
    wk_d = din("wk", [C, C], BF16)
    wv_d = din("wv", [C, C], BF16)
    wp_d = din("wp", [C, C], F32R)
    w1_d = din("w1", [C, F], BF16)
    w2_d = din("w2", [F, C], F32R)
    b1_d = din("b1", [F], F32)
    mask_d = din("mask", [T, T], BF16)
    ident_d = din("ident", [128, 128], BF16)
    bq_d = din("bq", [C], F32) if has_bq else None
    bk_d = din("bk", [C], F32) if has_bk else None
    bv_d = din("bv_b", [T, C], F32) if has_bv else None
    bp_d = din("bp_b", [128, C], F32) if has_bp else None
    b2_d = din("b2_b", [128, C], F32) if has_b2 else None
    y_d = nc.dram_tensor("y", [S, C], F32, kind="ExternalOutput").ap()

    with tile.TileContext(nc) as tc, ExitStack() as ctx:
        wp = ctx.enter_context(tc.tile_pool(name="wpool", bufs=1))
        ap_ = ctx.enter_context(tc.tile_pool(name="act", bufs=2))
        st = ctx.enter_context(tc.tile_pool(name="stat", bufs=3))
        hp = ctx.enter_context(tc.tile_pool(name="ht", bufs=1))
        ps = ctx.enter_context(tc.tile_pool(name="psum", bufs=1, space="PSUM"))

        # ---- resident weights ----
        def wload(name, d_ap, kchunks, fdim, dt):
            t = wp.tile([128, kchunks, fdim], dt, tag=name)
            nc.sync.dma_start(t[:], d_ap.rearrange("(kc p) f -> p kc f", p=128))
            return t

        wq_sb = wload("wq", wq_d, KC, C, BF16)
        wk_sb = wload("wk", wk_d, KC, C, BF16)
        wv_sb = wload("wv", wv_d, KC, C, BF16)
        wp_sb = wload("wp", wp_d, KC, C, F32R)
        w1_sb = wload("w1", w1_d, KC, F, BF16)
        w2_sb = wload("w2", w2_d, FM, C, F32R)

        b1_sb = wp.tile([128, FM], F32, tag="b1")
        nc.sync.dma_start(b1_sb[:], b1_d.rearrange("(fm p) -> p fm", p=128))
        mask_sb = wp.tile([T, T], BF16, tag="mask")
        nc.sync.dma_start(mask_sb[:], mask_d)
        ident_sb = wp.tile([128, 128], BF16, tag="ident")
        nc.sync.dma_start(ident_sb[:], ident_d)
        eps_sb = wp.tile([128, 1], F32, tag="eps")
        nc.vector.memset(eps_sb[:], EPS)
        if has_bq:
            bq_sb = wp.tile([128, H], F32, tag="bq")
            nc.sync.dma_start(bq_sb[:], bq_d.rearrange("(h d) -> d h", d=128))
        if has_bk:
            bk_sb = wp.tile([128, H], F32, tag="bk")
            nc.sync.dma_start(bk_sb[:], bk_d.rearrange("(h d) -> d h", d=128))
        if has_bv:
            bv_sb = wp.tile([T, C], F32, tag="bv")
            nc.sync.dma_start(bv_sb[:], bv_d)
        if has_bp:
            bp_sb = wp.tile([128, C], F32, tag="bp")
            nc.sync.dma_start(bp_sb[:], bp_d)
        if has_b2:
            b2_sb = wp.tile([128, C], F32, tag="b2")
            nc.sync.dma_start(b2_sb[:], b2_d)

        # ---- per-block helpers ----
        def ln_stats_apply(src, pref, sums, sumsq):
            """Finish LN given per-chunk sums/sumsq [128, TCH]; apply on ACT."""
            mu = st.tile([128, TCH], F32, tag=pref + "mu")
            nc.vector.tensor_scalar_mul(mu[:], sums[:], 1.0 / C)
            msq = st.tile([128, TCH], F32, tag=pref + "msq")
            nc.vector.tensor_mul(out=msq[:], in0=mu[:], in1=mu[:])
            var = st.tile([128, TCH], F32, tag=pref + "var")
            nc.vector.scalar_tensor_tensor(var[:], sumsq[:], 1.0 / C, msq[:],
                                           OP.mult, OP.subtract)
            std = st.tile([128, TCH], F32, tag=pref + "std")
            nc.scalar.activation(std[:], var[:], AF.Sqrt, bias=eps_sb[:, 0:1])
            rstd = st.tile([128, TCH], F32, tag=pref + "rstd")
            nc.vector.reciprocal(rstd[:], std[:])
            nmr = st.tile([128, TCH], F32, tag=pref + "nmr")
            nc.vector.scalar_tensor_tensor(nmr[:], mu[:], -1.0, rstd[:],
                                           OP.mult, OP.mult)
            xn = ap_.tile([128, TCH, C], BF16, tag=pref + "xn")
            for i in range(TCH):
                nc.scalar.activation(xn[:, i, :], src[:, i, :], AF.Identity,
                                     scale=rstd[:, i : i + 1],
                                     bias=nmr[:, i : i + 1])
            return xn

        def layer_norm(src, pref):
            """src: [128, TCH, C] f32 -> xn bf16 [128, TCH, C]."""
            sums = st.tile([128, TCH], F32, tag=pref + "sums")
            sumsq = st.tile([128, TCH], F32, tag=pref + "sumsq")
            nc.vector.tensor_reduce(sums[:], src[:], axis=mybir.AxisListType.X,
                                    op=OP.add)
            for i in range(TCH):
                scr = st.tile([128, C], BF16, tag="scr", bufs=2)
                nc.vector.scalar_tensor_tensor(
                    scr[:], src[:, i, :], 1.0, src[:, i, :], OP.mult, OP.mult,
                    accum_out=sumsq[:, i : i + 1])
            return ln_stats_apply(src, pref, sums, sumsq)

        def transpose_xn(xn, pref, ptag):
            """Transpose LN output to T-layout on the PE (bf16)."""
            xnT = ap_.tile([128, KC, TOK], BF16, tag=pref + "xnT")
            for kc in range(KC):
                p = ps.tile([128, TCH, 128], BF16, tag=ptag, bufs=4, name="txp")
                for mc in range(TCH):
                    nc.tensor.transpose(p[:, mc, :],
                                        xn[:, mc, kc * 128 : (kc + 1) * 128],
                                        ident_sb[:])
                if kc % 2 == 0:
                    nc.scalar.activation(xnT[:, kc, :], p[:], AF.Identity)
                else:
                    nc.vector.tensor_copy(out=xnT[:, kc, :], in_=p[:])
            return xnT

        # ---- block stages ----
        def stage_a1(blk):
            """x load, LN1, transpose -> (x_sb, xnT)."""
            row0 = blk * TOK
            x_sb = ap_.tile([128, TCH, C], F32, tag="x", bufs=3)
            nc.sync.dma_start(
                x_sb[:],
                x_d[row0 : row0 + TOK, :].rearrange("(ch p) c -> p ch c", p=128))
            xn = layer_norm(x_sb, "a")
            xnT = transpose_xn(xn, "a", "pa")
            return x_sb, xnT

        def stage_a2(blk, xnT):
            """QKV + attention -> ot."""
            # QKV projections (bf16)
            qt = ap_.tile([128, H, TOK], BF16, tag="qt")
            kt = ap_.tile([128, H, TOK], BF16, tag="kt")
            for dst, w_sb, bias_sb in ((qt, wq_sb, bq_sb if has_bq else None),
                                       (kt, wk_sb, bk_sb if has_bk else None)):
                for h in range(H):
                    p = ps.tile([128, TOK], F32, tag="pa", bufs=4)
                    for kc in range(KC):
                        nc.tensor.matmul(p[:], w_sb[:, kc, h * 128 : (h + 1) * 128],
                                         xnT[:, kc, :], start=(kc == 0),
                                         stop=(kc == KC - 1))
                    if bias_sb is not None:
                        nc.scalar.activation(dst[:, h, :], p[:], AF.Identity,
                                             bias=bias_sb[:, h : h + 1])
                    else:
                        nc.vector.tensor_copy(out=dst[:, h, :], in_=p[:])
            vt = ap_.tile([T, NB, C], BF16, tag="vt")
            for b in range(NB):
                p = ps.tile([T, C], F32, tag="pa", bufs=4)
                for kc in range(KC):
                    nc.tensor.matmul(p[:], xnT[:, kc, b * T : (b + 1) * T],
                                     wv_sb[:, kc, :], start=(kc == 0),
                                     stop=(kc == KC - 1))
                if has_bv:
                    nc.vector.tensor_add(out=vt[:, b, :], in0=p[:], in1=bv_sb[:])
                else:
                    nc.vector.tensor_copy(out=vt[:, b, :], in_=p[:])

            # attention: scores [t, s] per (h, b), exp, mask, row-normalize
            ee = ap_.tile([T, H * NB, T], BF16, tag="ee")
            for h in range(H):
                p = ps.tile([T, NB, T], F32, tag="pa", bufs=4)
                for b in range(NB):
                    nc.tensor.matmul(p[:, b, :], qt[:, h, b * T : (b + 1) * T],
                                     kt[:, h, b * T : (b + 1) * T],
                                     start=True, stop=True)
                nc.scalar.activation(ee[:, h * NB : (h + 1) * NB, :], p[:],
                                     AF.Exp, scale=SCALE)
            nc.vector.tensor_mul(
                out=ee[:], in0=ee[:],
                in1=mask_sb[:].unsqueeze(1).to_broadcast([T, H * NB, T]))
            dsum = st.tile([T, H * NB], F32, tag="dsum")
            nc.vector.tensor_reduce(dsum[:], ee[:], axis=mybir.AxisListType.X,
                                    op=OP.add)
            rr = st.tile([T, H * NB], F32, tag="rr")
            nc.vector.reciprocal(rr[:], dsum[:])
            nc.vector.tensor_mul(
                out=ee[:], in0=ee[:],
                in1=rr[:].unsqueeze(2).to_broadcast([T, H * NB, T]))
            return vt, ee

        def stage_a2b(blk, vt, ee):
            """probs transpose + attn @ V -> ot (T-layout, f32r)."""
            pt = ee  # probs are overwritten in place by their transpose
            for h in range(H):
                p = ps.tile([T, NB, T], BF16, tag="pa", bufs=4)
                for b in range(NB):
                    nc.tensor.transpose(p[:, b, :], ee[:, h * NB + b, :],
                                        ident_sb[:T, :T])
                nc.vector.tensor_copy(out=pt[:, h * NB : (h + 1) * NB, :], in_=p[:])
            ot = ap_.tile([128, H, TOK], F32R, tag="ot")
            for h in range(H):
                p = ps.tile([128, NB, T], F32, tag="pa", bufs=4)
                for b in range(NB):
                    nc.tensor.matmul(p[:, b, :], vt[:, b, h * 128 : (h + 1) * 128],
                                     pt[:, h * NB + b, :], start=True, stop=True)
                nc.vector.tensor_copy(out=ot[:, h, :], in_=p[:])
            return ot

        def stage_b(blk, x_sb, ot):
            """proj + residual, LN2, MLP, store."""
            row0 = blk * TOK
            x2 = ap_.tile([128, TCH, C], F32, tag="x2")
            sums2 = st.tile([128, TCH], F32, tag="bsums")
            sumsq2 = st.tile([128, TCH], F32, tag="bsumsq")
            for mc in range(TCH):
                p = ps.tile([128, C], F32, tag="pb", bufs=4)
                for kc in range(H):
                    nc.tensor.matmul(p[:], ot[:, kc, mc * 128 : (mc + 1) * 128],
                                     wp_sb[:, kc, :], start=(kc == 0),
                                     stop=(kc == H - 1))
                if has_bp:
                    nc.vector.tensor_add(out=p[:], in0=p[:], in1=bp_sb[:])
                # x2 = sa + x, with the LN2 row-sum accumulated for free
                nc.vector.scalar_tensor_tensor(
                    x2[:, mc, :], p[:], 1.0, x_sb[:, mc, :], OP.mult, OP.add,
                    accum_out=sums2[:, mc : mc + 1])
                scr2 = st.tile([128, C], BF16, tag="scr2", bufs=2)
                nc.vector.scalar_tensor_tensor(
                    scr2[:], x2[:, mc, :], 1.0, x2[:, mc, :], OP.mult, OP.mult,
                    accum_out=sumsq2[:, mc : mc + 1])

            # MLP
            xn2 = ln_stats_apply(x2, "b", sums2, sumsq2)
            xn2T = transpose_xn(xn2, "b", "pb")
            ht = hp.tile([128, FM, TOK], F32R, tag="ht")
            for fm in range(FM):
                p = ps.tile([128, TOK], F32, tag="pb", bufs=4)
                for kc in range(KC):
                    nc.tensor.matmul(p[:], w1_sb[:, kc, fm * 128 : (fm + 1) * 128],
                                     xn2T[:, kc, :], start=(kc == 0),
                                     stop=(kc == KC - 1))
                nc.scalar.activation(ht[:, fm, :], p[:], AF.Gelu,
                                     bias=b1_sb[:, fm : fm + 1])
            xo = ap_.tile([128, TCH, C], F32, tag="xo")
            for mc in range(TCH):
                p = ps.tile([128, C], F32, tag="pb", bufs=4)
                for fk in range(FM):
                    nc.tensor.matmul(p[:], ht[:, fk, mc * 128 : (mc + 1) * 128],
                                     w2_sb[:, fk, :], start=(fk == 0),
                                     stop=(fk == FM - 1))
                if has_b2:
                    nc.vector.tensor_add(out=p[:], in0=p[:], in1=b2_sb[:])
                nc.vector.tensor_add(out=xo[:, mc, :], in0=p[:],
                                     in1=x2[:, mc, :])
            nc.sync.dma_start(
                y_d[row0 : row0 + TOK, :].rearrange("(ch p) c -> p ch c", p=128),
                xo[:])

        # Software-pipelined emission. Per-engine FIFO order interleaves the
        # stages so the next blocks' independent work is queued ahead of the
        # current block's dependency stalls: A1 runs two blocks ahead, the
        # attention front (QKV+scores+softmax issue) one block ahead, and the
        # attention tail (probs transpose + av) is emitted after the previous
        # block's MLP so the softmax latency hides behind it.
        xs, xnTs, sm, ots = {}, {}, {}, {}
        xs[0], xnTs[0] = stage_a1(0)
        if nblk > 1:
            xs[1], xnTs[1] = stage_a1(1)
        sm[0] = stage_a2(0, xnTs.pop(0))
        ots[0] = stage_a2b(0, *sm.pop(0))
        for blk in range(1, nblk):
            if blk + 1 < nblk:
                xs[blk + 1], xnTs[blk + 1] = stage_a1(blk + 1)
            sm[blk] = stage_a2(blk, xnTs.pop(blk))
            stage_b(blk - 1, xs.pop(blk - 1), ots.pop(blk - 1))
            ots[blk] = stage_a2b(blk, *sm.pop(blk))
        stage_b(nblk - 1, xs.pop(nblk - 1), ots.pop(nblk - 1))

    nc.compile()
    return nc


def fold(inputs):
    """Host-side exact folding of LN affines and biases into weights.

    Returns dict of staged arrays for the device program + bias flags.
    """
    f32 = np.float32
    g1 = inputs["g1"].astype(f32)
    be1 = inputs["be1"].astype(f32)
    g2 = inputs["g2"].astype(f32)
    be2 = inputs["be2"].astype(f32)

    def headcat(w):  # [H, C, D] -> [C, H*D]
        return np.concatenate([w[h] for h in range(H)], axis=1)

    wq = headcat(np.asarray(inputs["wq"], f32))
    wk = headcat(np.asarray(inputs["wk"], f32))
    wv = headcat(np.asarray(inputs["wv"], f32))
    wp_ = np.asarray(inputs["w_proj"], f32)
    w1 = np.asarray(inputs["w1"], f32)
    w2 = np.asarray(inputs["w2"], f32)

    wq_f = g1[:, None] * wq
    wk_f = g1[:, None] * wk
    wv_f = g1[:, None] * wv
    bq = be1 @ wq
    bk = be1 @ wk
    bv = be1 @ wv
    bp = np.asarray(inputs["b_proj"], f32)
    w1_f = g2[:, None] * w1
    b1 = np.asarray(inputs["b1"], f32) + be2 @ w1
    b2 = np.asarray(inputs["b2"], f32)

    mask = np.tril(np.ones((T, T), np.float32)).astype(ml_dtypes.bfloat16)
    ident = np.eye(128, dtype=ml_dtypes.bfloat16)

    staged = {
        "wq": wq_f.astype(ml_dtypes.bfloat16),
        "wk": wk_f.astype(ml_dtypes.bfloat16),
        "wv": wv_f.astype(ml_dtypes.bfloat16),
        "wp": wp_.astype(f32),
        "w1": w1_f.astype(ml_dtypes.bfloat16),
        "w2": w2.astype(f32),
        "b1": b1,
        "mask": mask,
        "ident": ident,
    }
    flags = {
        "has_bq": bool(np.any(bq)),
        "has_bk": bool(np.any(bk)),
        "has_bv": bool(np.any(bv)),
        "has_bp": bool(np.any(bp)),
        "has_b2": bool(np.any(b2)),
    }
    if flags["has_bq"]:
        staged["bq"] = bq
    if flags["has_bk"]:
        staged["bk"] = bk
    if flags["has_bv"]:
        staged["bv_b"] = np.broadcast_to(bv, (T, C)).copy()
    if flags["has_bp"]:
        staged["bp_b"] = np.broadcast_to(bp, (128, C)).copy()
    if flags["has_b2"]:
        staged["b2_b"] = np.broadcast_to(b2, (128, C)).copy()
    return staged, flags


_CACHE = {}


def kernel(**inputs):
    staged, flags = fold(inputs)
    key = tuple(sorted(flags.items()))
    if key not in _CACHE:
        _CACHE[key] = build(**flags)
    nc = _CACHE[key]

    x = np.asarray(inputs["x"], np.float32).reshape(B, T * C)
    in_maps = []
    for c in range(NCORES):
        m = dict(staged)
        m["x"] = x[c * SEQ_PER_CORE : (c + 1) * SEQ_PER_CORE].reshape(S, C)
        in_maps.append(m)

    res = bass_utils.run_bass_kernel_spmd(nc, in_maps, core_ids=list(range(NCORES)))
    out = np.concatenate([r["y"] for r in res.results], axis=0)
    return out.reshape(B, T, C).astype(np.float32)


# revision 23
# speedup vs baseline: 1.7922x; 1.0046x over previous
"""Fused transformer-block kernel for TRN2, 8-way data parallel over batch.

Layout strategy per core (128 sequences of 96 tokens = 12288 tokens):
  - Residual stream kept in N-layout [token_part, feature_free]; LayerNorm
    stats are free-dim reductions.
  - LN outputs written as bf16 and transposed to feature-major T-layout
    [feature_part, token_free] via DMA-xbar transposes; these feed the QKV
    and MLP1 matmuls (bf16).
  - Attention computed per (seq, head) with T=96 <= 128: scores in [t, s]
    layout (softmax over free dim), exp without max-subtraction (scores are
    bounded for this problem scale), 0/1 causal mask multiply, probs
    transposed on the PE, then attn@V gives head outputs directly in
    T-layout.
  - proj and MLP2 run in float32r (full PE speed at N=512, ~1e-4 rel err).
  - gamma/beta of both LNs and all biases are folded into the weight
    matrices / bias vectors on the host (exact algebra, see fold()).
"""

import sys

sys.path.insert(0, "/opt/trn_rl_repo")

from contextlib import ExitStack

import ml_dtypes
import numpy as np

import concourse.bass as bass  # noqa: F401  (registers AP types)
import concourse.tile as tile
from concourse import bacc, bass_utils, mybir

# Cache walrus-compiled NEFFs on disk keyed by BIR hash: re-running an
# unchanged program skips the multi-minute backend compile.
try:
    import hashlib
    import os as _os
    import shutil as _shutil

    import concourse.bass2jax as _b2j

    _orig_cbk = _b2j.compile_bir_kernel

    def _cached_cbk(bir_json, tmpdir, neff_name="file.neff"):
        try:
            raw = bir_json if isinstance(bir_json, bytes) else bir_json.encode()
            h = hashlib.sha256(raw).hexdigest()[:24]
            cdir = "/tmp/neff_cache"
            _os.makedirs(cdir, exist_ok=True)
            cpath = _os.path.join(cdir, h + ".neff")
            if _os.path.exists(cpath):
                return cpath
        except Exception:
            return _orig_cbk(bir_json, tmpdir, neff_name)
        p = _orig_cbk(bir_json, tmpdir, neff_name)
        try:
            _shutil.copy(p, cpath)
        except Exception:
            pass
        return p

    if _orig_cbk.__name__ != "_cached_cbk":
        _b2j.compile_bir_kernel = _cached_cbk
except Exception:
    pass

B, T, C = 1024, 96, 512
H, D = 4, 128
F = 4 * C
EPS = 1e-5
SCALE = D**-0.5

NCORES = 8
SEQ_PER_CORE = B // NCORES  # 128
S = SEQ_PER_CORE * T  # 12288 tokens per core
NB = 4  # sequences per block
TOK = NB * T  # 384 tokens per block
NBLK = SEQ_PER_CORE // NB  # 32 blocks
TCH = TOK // 128  # 3 token chunks per block
KC = C // 128  # 4 feature chunks of C
FM = F // 128  # 16 feature chunks of F

F32 = mybir.dt.float32
F32R = mybir.dt.float32r
BF16 = mybir.dt.bfloat16
AF = mybir.ActivationFunctionType
OP = mybir.AluOpType


def build(nblk=NBLK, has_bq=False, has_bk=False, has_bv=False, has_bp=False,
          has_b2=False):
    nc = bacc.Bacc("TRN2", target_bir_lowering=False, debug=False)

    def din(name, shape, dt):
        return nc.dram_tensor(name, shape, dt, kind="ExternalInput").ap()

    x_d = din("x", [S, C], F32)
    wq_d = din("wq", [C, C], BF16)
    wk_d = din("wk", [C, C], BF16)
    wv_d = din("wv", [C, C], BF16)
    wp_d = din("wp", [C, C], F32R)
    w1_d = din("w1", [C, F], BF16)
    w2_d = din("w2", [F, C], F32R)
    b1_d = din("b1", [F], F32)
    mask_d = din("mask", [T, T], BF16)
    ident_d = din("ident", [128, 128], BF16)
    bq_d = din("bq", [C], F32) if has_bq else None
    bk_d = din("bk", [C], F32) if has_bk else None
    bv_d = din("bv_b", [T, C], F32) if has_bv else None
    bp_d = din("bp_b", [128, C], F32) if has_bp else None
    b2_d = din("b2_b", [128, C], F32) if has_b2 else None
    y_d = nc.dram_tensor("y", [S, C], F32, kind="ExternalOutput").ap()

    with tile.TileContext(nc) as tc, ExitStack() as ctx:
        wp = ctx.enter_context(tc.tile_pool(name="wpool", bufs=1))
        ap_ = ctx.enter_context(tc.tile_pool(name="act", bufs=2))
        st = ctx.enter_context(tc.tile_pool(name="stat", bufs=3))
        hp = ctx.enter_context(tc.tile_pool(name="ht", bufs=1))
        ps = ctx.enter_context(tc.tile_pool(name="psum", bufs=1, space="PSUM"))

        # ---- resident weights ----
        def wload(name, d_ap, kchunks, fdim, dt):
            t = wp.tile([128, kchunks, fdim], dt, tag=name)
            nc.sync.dma_start(t[:], d_ap.rearrange("(kc p) f -> p kc f", p=128))
            return t

        wq_sb = wload("wq", wq_d, KC, C, BF16)
        wk_sb = wload("wk", wk_d, KC, C, BF16)
        wv_sb = wload("wv", wv_d, KC, C, BF16)
        wp_sb = wload("wp", wp_d, KC, C, F32R)
        w1_sb = wload("w1", w1_d, KC, F, BF16)
        w2_sb = wload("w2", w2_d, FM, C, F32R)

        b1_sb = wp.tile([128, FM], F32, tag="b1")
        nc.sync.dma_start(b1_sb[:], b1_d.rearrange("(fm p) -> p fm", p=128))
        mask_sb = wp.tile([T, T], BF16, tag="mask")
        nc.sync.dma_start(mask_sb[:], mask_d)
        ident_sb = wp.tile([128, 128], BF16, tag="ident")
        nc.sync.dma_start(ident_sb[:], ident_d)
        eps_sb = wp.tile([128, 1], F32, tag="eps")
        nc.vector.memset(eps_sb[:], EPS)
        if has_bq:
            bq_sb = wp.tile([128, H], F32, tag="bq")
            nc.sync.dma_start(bq_sb[:], bq_d.rearrange("(h d) -> d h", d=128))
        if has_bk:
            bk_sb = wp.tile([128, H], F32, tag="bk")
            nc.sync.dma_start(bk_sb[:], bk_d.rearrange("(h d) -> d h", d=128))
        if has_bv:
            bv_sb = wp.tile([T, C], F32, tag="bv")
            nc.sync.dma_start(bv_sb[:], bv_d)
        if has_bp:
            bp_sb = wp.tile([128, C], F32, tag="bp")
            nc.sync.dma_start(bp_sb[:], bp_d)
        if has_b2:
            b2_sb = wp.tile([128, C], F32, tag="b2")
            nc.sync.dma_start(b2_sb[:], b2_d)

        # ---- per-block helpers ----
        def ln_stats_apply(src, pref, sums, sumsq):
            """Finish LN given per-chunk sums/sumsq [128, TCH]; apply on ACT."""
            mu = st.tile([128, TCH], F32, tag=pref + "mu")
            nc.vector.tensor_scalar_mul(mu[:], sums[:], 1.0 / C)
            msq = st.tile([128, TCH], F32, tag=pref + "msq")
            nc.vector.tensor_mul(out=msq[:], in0=mu[:], in1=mu[:])
            var = st.tile([128, TCH], F32, tag=pref + "var")
            nc.vector.scalar_tensor_tensor(var[:], sumsq[:], 1.0 / C, msq[:],
                                           OP.mult, OP.subtract)
            std = st.tile([128, TCH], F32, tag=pref + "std")
            nc.scalar.activation(std[:], var[:], AF.Sqrt, bias=eps_sb[:, 0:1])
            rstd = st.tile([128, TCH], F32, tag=pref + "rstd")
            nc.vector.reciprocal(rstd[:], std[:])
            nmr = st.tile([128, TCH], F32, tag=pref + "nmr")
            nc.vector.scalar_tensor_tensor(nmr[:], mu[:], -1.0, rstd[:],
                                           OP.mult, OP.mult)
            xn = ap_.tile([128, TCH, C], BF16, tag=pref + "xn")
            for i in range(TCH):
                nc.scalar.activation(xn[:, i, :], src[:, i, :], AF.Identity,
                                     scale=rstd[:, i : i + 1],
                                     bias=nmr[:, i : i + 1])
            return xn

        def layer_norm(src, pref):
            """src: [128, TCH, C] f32 -> xn bf16 [128, TCH, C]."""
            sums = st.tile([128, TCH], F32, tag=pref + "sums")
            sumsq = st.tile([128, TCH], F32, tag=pref + "sumsq")
            nc.vector.tensor_reduce(sums[:], src[:], axis=mybir.AxisListType.X,
                                    op=OP.add)
            for i in range(TCH):
                scr = st.tile([128, C], BF16, tag="scr", bufs=2)
                nc.vector.scalar_tensor_tensor(
                    scr[:], src[:, i, :], 1.0, src[:, i, :], OP.mult, OP.mult,
                    accum_out=sumsq[:, i : i + 1])
            return ln_stats_apply(src, pref, sums, sumsq)

        def transpose_xn(xn, pref, ptag):
            """Transpose LN output to T-layout on the PE (bf16)."""
            xnT = ap_.tile([128, KC, TOK], BF16, tag=pref + "xnT")
            for kc in range(KC):
                p = ps.tile([128, TCH, 128], BF16, tag=ptag, bufs=4, name="txp")
                for mc in range(TCH):
                    nc.tensor.transpose(p[:, mc, :],
                                        xn[:, mc, kc * 128 : (kc + 1) * 128],
                                        ident_sb[:])
                if kc % 2 == 0:
                    nc.scalar.activation(xnT[:, kc, :], p[:], AF.Identity)
                else:
                    nc.vector.tensor_copy(out=xnT[:, kc, :], in_=p[:])
            return xnT

        # ---- block stages ----
        def stage_a1(blk):
            """x load, LN1, transpose -> (x_sb, xnT)."""
            row0 = blk * TOK
            x_sb = ap_.tile([128, TCH, C], F32, tag="x", bufs=3)
            nc.sync.dma_start(
                x_sb[:],
                x_d[row0 : row0 + TOK, :].rearrange("(ch p) c -> p ch c", p=128))
            xn = layer_norm(x_sb, "a")
            xnT = transpose_xn(xn, "a", "pa")
            return x_sb, xnT

        def stage_a2(blk, xnT):
            """QKV + attention -> ot."""
            # QKV projections (bf16)
            qt = ap_.tile([128, H, TOK], BF16, tag="qt")
            kt = ap_.tile([128, H, TOK], BF16, tag="kt")
            for dst, w_sb, bias_sb in ((qt, wq_sb, bq_sb if has_bq else None),
                                       (kt, wk_sb, bk_sb if has_bk else None)):
                for h in range(H):
                    p = ps.tile([128, TOK], F32, tag="pa", bufs=4)
                    for kc in range(KC):
                        nc.tensor.matmul(p[:], w_sb[:, kc, h * 128 : (h + 1) * 128],
                                         xnT[:, kc, :], start=(kc == 0),
                                         stop=(kc == KC - 1))
                    if bias_sb is not None:
                        nc.scalar.activation(dst[:, h, :], p[:], AF.Identity,
                                             bias=bias_sb[:, h : h + 1])
                    else:
                        nc.vector.tensor_copy(out=dst[:, h, :], in_=p[:])
            vt = ap_.tile([T, NB, C], BF16, tag="vt")
            for b in range(NB):
                p = ps.tile([T, C], F32, tag="pa", bufs=4)
                for kc in range(KC):
                    nc.tensor.matmul(p[:], xnT[:, kc, b * T : (b + 1) * T],
                                     wv_sb[:, kc, :], start=(kc == 0),
                                     stop=(kc == KC - 1))
                if has_bv:
                    nc.vector.tensor_add(out=vt[:, b, :], in0=p[:], in1=bv_sb[:])
                else:
                    nc.scalar.activation(vt[:, b, :], p[:], AF.Identity)

            # attention: scores [t, s] per (h, b), exp, mask, row-normalize
            ee = ap_.tile([T, H * NB, T], BF16, tag="ee")
            for h in range(H):
                p = ps.tile([T, NB, T], F32, tag="pa", bufs=4)
                for b in range(NB):
                    nc.tensor.matmul(p[:, b, :], qt[:, h, b * T : (b + 1) * T],
                                     kt[:, h, b * T : (b + 1) * T],
                                     start=True, stop=True)
                nc.scalar.activation(ee[:, h * NB : (h + 1) * NB, :], p[:],
                                     AF.Exp, scale=SCALE)
            nc.gpsimd.tensor_mul(
                out=ee[:], in0=ee[:],
                in1=mask_sb[:].unsqueeze(1).to_broadcast([T, H * NB, T]))
            dsum = st.tile([T, H * NB], F32, tag="dsum")
            nc.vector.tensor_reduce(dsum[:], ee[:], axis=mybir.AxisListType.X,
                                    op=OP.add)
            rr = st.tile([T, H * NB], F32, tag="rr")
            nc.vector.reciprocal(rr[:], dsum[:])
            nc.gpsimd.tensor_mul(
                out=ee[:], in0=ee[:],
                in1=rr[:].unsqueeze(2).to_broadcast([T, H * NB, T]))
            return vt, ee

        def stage_a2b(blk, vt, ee):
            """probs transpose + attn @ V -> ot (T-layout, f32r)."""
            pt = ee  # probs are overwritten in place by their transpose
            for h in range(H):
                p = ps.tile([T, NB, T], BF16, tag="pa", bufs=4)
                for b in range(NB):
                    nc.tensor.transpose(p[:, b, :], ee[:, h * NB + b, :],
                                        ident_sb[:T, :T])
                nc.vector.tensor_copy(out=pt[:, h * NB : (h + 1) * NB, :], in_=p[:])
            ot = ap_.tile([128, H, TOK], F32R, tag="ot")
            for h in range(H):
                p = ps.tile([128, NB, T], F32, tag="pa", bufs=4)
                for b in range(NB):
                    nc.tensor.matmul(p[:, b, :], vt[:, b, h * 128 : (h + 1) * 128],
                                     pt[:, h * NB + b, :], start=True, stop=True)
                nc.vector.tensor_copy(out=ot[:, h, :], in_=p[:])
            return ot

        def stage_b(blk, x_sb, ot):
            """proj + residual, LN2, MLP, store."""
            row0 = blk * TOK
            x2 = ap_.tile([128, TCH, C], F32, tag="x2")
            sums2 = st.tile([128, TCH], F32, tag="bsums")
            sumsq2 = st.tile([128, TCH], F32, tag="bsumsq")
            for mc in range(TCH):
                p = ps.tile([128, C], F32, tag="pb", bufs=4)
                for kc in range(H):
                    nc.tensor.matmul(p[:], ot[:, kc, mc * 128 : (mc + 1) * 128],
                                     wp_sb[:, kc, :], start=(kc == 0),
                                     stop=(kc == H - 1))
                if has_bp:
                    nc.vector.tensor_add(out=p[:], in0=p[:], in1=bp_sb[:])
                # x2 = sa + x, with the LN2 row-sum accumulated for free
                nc.vector.scalar_tensor_tensor(
                    x2[:, mc, :], p[:], 1.0, x_sb[:, mc, :], OP.mult, OP.add,
                    accum_out=sums2[:, mc : mc + 1])
                scr2 = st.tile([128, C], BF16, tag="scr2", bufs=2)
                nc.vector.scalar_tensor_tensor(
                    scr2[:], x2[:, mc, :], 1.0, x2[:, mc, :], OP.mult, OP.mult,
                    accum_out=sumsq2[:, mc : mc + 1])

            # MLP
            xn2 = ln_stats_apply(x2, "b", sums2, sumsq2)
            xn2T = transpose_xn(xn2, "b", "pb")
            ht = hp.tile([128, FM, TOK], F32R, tag="ht")
            for fm in range(FM):
                p = ps.tile([128, TOK], F32, tag="pb", bufs=4)
                for kc in range(KC):
                    nc.tensor.matmul(p[:], w1_sb[:, kc, fm * 128 : (fm + 1) * 128],
                                     xn2T[:, kc, :], start=(kc == 0),
                                     stop=(kc == KC - 1))
                nc.scalar.activation(ht[:, fm, :], p[:], AF.Gelu,
                                     bias=b1_sb[:, fm : fm + 1])
            xo = ap_.tile([128, TCH, C], F32, tag="xo")
            for mc in range(TCH):
                p = ps.tile([128, C], F32, tag="pb", bufs=4)
                for fk in range(FM):
                    nc.tensor.matmul(p[:], ht[:, fk, mc * 128 : (mc + 1) * 128],
                                     w2_sb[:, fk, :], start=(fk == 0),
                                     stop=(fk == FM - 1))
                if has_b2:
                    nc.vector.tensor_add(out=p[:], in0=p[:], in1=b2_sb[:])
                nc.vector.tensor_add(out=xo[:, mc, :], in0=p[:],
                                     in1=x2[:, mc, :])
            nc.sync.dma_start(
                y_d[row0 : row0 + TOK, :].rearrange("(ch p) c -> p ch c", p=128),
                xo[:])

        # Software-pipelined emission. Per-engine FIFO order interleaves the
        # stages so the next blocks' independent work is queued ahead of the
        # current block's dependency stalls: A1 runs two blocks ahead, the
        # attention front (QKV+scores+softmax issue) one block ahead, and the
        # attention tail (probs transpose + av) is emitted after the previous
        # block's MLP so the softmax latency hides behind it.
        xs, xnTs, sm, ots = {}, {}, {}, {}
        xs[0], xnTs[0] = stage_a1(0)
        if nblk > 1:
            xs[1], xnTs[1] = stage_a1(1)
        sm[0] = stage_a2(0, xnTs.pop(0))
        ots[0] = stage_a2b(0, *sm.pop(0))
        for blk in range(1, nblk):
            if blk + 1 < nblk:
                xs[blk + 1], xnTs[blk + 1] = stage_a1(blk + 1)
            sm[blk] = stage_a2(blk, xnTs.pop(blk))
            stage_b(blk - 1, xs.pop(blk - 1), ots.pop(blk - 1))
            ots[blk] = stage_a2b(blk, *sm.pop(blk))
        stage_b(nblk - 1, xs.pop(nblk - 1), ots.pop(nblk - 1))

    nc.compile()
    return nc


def fold(inputs):
    """Host-side exact folding of LN affines and biases into weights.

    Returns dict of staged arrays for the device program + bias flags.
    """
    f32 = np.float32
    g1 = inputs["g1"].astype(f32)
    be1 = inputs["be1"].astype(f32)
    g2 = inputs["g2"].astype(f32)
    be2 = inputs["be2"].astype(f32)

    def headcat(w):  # [H, C, D] -> [C, H*D]
        return np.concatenate([w[h] for h in range(H)], axis=1)

    wq = headcat(np.asarray(inputs["wq"], f32))
    wk = headcat(np.asarray(inputs["wk"], f32))
    wv = headcat(np.asarray(inputs["wv"], f32))
    wp_ = np.asarray(inputs["w_proj"], f32)
    w1 = np.asarray(inputs["w1"], f32)
    w2 = np.asarray(inputs["w2"], f32)

    wq_f = g1[:, None] * wq
    wk_f = g1[:, None] * wk
    wv_f = g1[:, None] * wv
    bq = be1 @ wq
    bk = be1 @ wk
    bv = be1 @ wv
    bp = np.asarray(inputs["b_proj"], f32)
    w1_f = g2[:, None] * w1
    b1 = np.asarray(inputs["b1"], f32) + be2 @ w1
    b2 = np.asarray(inputs["b2"], f32)

    mask = np.tril(np.ones((T, T), np.float32)).astype(ml_dtypes.bfloat16)
    ident = np.eye(128, dtype=ml_dtypes.bfloat16)

    staged = {
        "wq": wq_f.astype(ml_dtypes.bfloat16),
        "wk": wk_f.astype(ml_dtypes.bfloat16),
        "wv": wv_f.astype(ml_dtypes.bfloat16),
        "wp": wp_.astype(f32),
        "w1": w1_f.astype(ml_dtypes.bfloat16),
        "w2": w2.astype(f32),
        "b1": b1,
        "mask": mask,
        "ident": ident,
    }
    flags = {
        "has_bq": bool(np.any(bq)),
        "has_bk": bool(np.any(bk)),
        "has_bv": bool(np.any(bv)),
        "has_bp": bool(np.any(bp)),
        "has_b2": bool(np.any(b2)),
    }
    if flags["has_bq"]:
        staged["bq"] = bq
    if flags["has_bk"]:
        staged["bk"] = bk
    if flags["has_bv"]:
        staged["bv_b"] = np.broadcast_to(bv, (T, C)).copy()
    if flags["has_bp"]:
        staged["bp_b"] = np.broadcast_to(bp, (128, C)).copy()
    if flags["has_b2"]:
        staged["b2_b"] = np.broadcast_to(b2, (128, C)).copy()
    return staged, flags


_CACHE = {}


def kernel(**inputs):
    staged, flags = fold(inputs)
    key = tuple(sorted(flags.items()))
    if key not in _CACHE:
        _CACHE[key] = build(**flags)
    nc = _CACHE[key]

    x = np.asarray(inputs["x"], np.float32).reshape(B, T * C)
    in_maps = []
    for c in range(NCORES):
        m = dict(staged)
        m["x"] = x[c * SEQ_PER_CORE : (c + 1) * SEQ_PER_CORE].reshape(S, C)
        in_maps.append(m)

    res = bass_utils.run_bass_kernel_spmd(nc, in_maps, core_ids=list(range(NCORES)))
    out = np.concatenate([r["y"] for r in res.results], axis=0)
    return out.reshape(B, T, C).astype(np.float32)


# revision 24
# speedup vs baseline: 1.8650x; 1.0406x over previous
"""Fused transformer-block kernel for TRN2, 8-way data parallel over batch.

Layout strategy per core (128 sequences of 96 tokens = 12288 tokens):
  - Residual stream kept in N-layout [token_part, feature_free]; LayerNorm
    stats are free-dim reductions.
  - LN outputs written as bf16 and transposed to feature-major T-layout
    [feature_part, token_free] via DMA-xbar transposes; these feed the QKV
    and MLP1 matmuls (bf16).
  - Attention computed per (seq, head) with T=96 <= 128: scores in [t, s]
    layout (softmax over free dim), exp without max-subtraction (scores are
    bounded for this problem scale), 0/1 causal mask multiply, probs
    transposed on the PE, then attn@V gives head outputs directly in
    T-layout.
  - proj and MLP2 run in float32r (full PE speed at N=512, ~1e-4 rel err).
  - gamma/beta of both LNs and all biases are folded into the weight
    matrices / bias vectors on the host (exact algebra, see fold()).
"""

import sys

sys.path.insert(0, "/opt/trn_rl_repo")

from contextlib import ExitStack

import ml_dtypes
import numpy as np

import concourse.bass as bass  # noqa: F401  (registers AP types)
import concourse.tile as tile
from concourse import bacc, bass_utils, mybir

# Cache walrus-compiled NEFFs on disk keyed by BIR hash: re-running an
# unchanged program skips the multi-minute backend compile.
try:
    import hashlib
    import os as _os
    import shutil as _shutil

    import concourse.bass2jax as _b2j

    _orig_cbk = _b2j.compile_bir_kernel

    def _cached_cbk(bir_json, tmpdir, neff_name="file.neff"):
        try:
            raw = bir_json if isinstance(bir_json, bytes) else bir_json.encode()
            h = hashlib.sha256(raw).hexdigest()[:24]
            cdir = "/tmp/neff_cache"
            _os.makedirs(cdir, exist_ok=True)
            cpath = _os.path.join(cdir, h + ".neff")
            if _os.path.exists(cpath):
                return cpath
        except Exception:
            return _orig_cbk(bir_json, tmpdir, neff_name)
        p = _orig_cbk(bir_json, tmpdir, neff_name)
        try:
            _shutil.copy(p, cpath)
        except Exception:
            pass
        return p

    if _orig_cbk.__name__ != "_cached_cbk":
        _b2j.compile_bir_kernel = _cached_cbk
except Exception:
    pass

B, T, C = 1024, 96, 512
H, D = 4, 128
F = 4 * C
EPS = 1e-5
SCALE = D**-0.5

NCORES = 8
SEQ_PER_CORE = B // NCORES  # 128
S = SEQ_PER_CORE * T  # 12288 tokens per core
NB = 4  # sequences per block
TOK = NB * T  # 384 tokens per block
NBLK = SEQ_PER_CORE // NB  # 32 blocks
TCH = TOK // 128  # 3 token chunks per block
KC = C // 128  # 4 feature chunks of C
FM = F // 128  # 16 feature chunks of F

F32 = mybir.dt.float32
F32R = mybir.dt.float32r
BF16 = mybir.dt.bfloat16
AF = mybir.ActivationFunctionType
OP = mybir.AluOpType


def build(nblk=NBLK, has_bq=False, has_bk=False, has_bv=False, has_bp=False,
          has_b2=False):
    nc = bacc.Bacc("TRN2", target_bir_lowering=False, debug=False)

    def din(name, shape, dt):
        return nc.dram_tensor(name, shape, dt, kind="ExternalInput").ap()

    x_d = din("x", [S, C], F32)
    wq_d = din("wq", [C, C], BF16)
    wk_d = din("wk", [C, C], BF16)
    wv_d = din("wv", [C, C], BF16)
    wp_d = din("wp", [C, C], F32R)
    w1_d = din("w1", [C, F], BF16)
    w2_d = din("w2", [F, C], F32R)
    b1_d = din("b1", [F], F32)
    mask_d = din("mask", [T, T], BF16)
    ident_d = din("ident", [128, 128], BF16)
    bq_d = din("bq", [C], F32) if has_bq else None
    bk_d = din("bk", [C], F32) if has_bk else None
    bv_d = din("bv_b", [T, C], F32) if has_bv else None
    bp_d = din("bp_b", [128, C], F32) if has_bp else None
    b2_d = din("b2_b", [128, C], F32) if has_b2 else None
    y_d = nc.dram_tensor("y", [S, C], F32, kind="ExternalOutput").ap()

    with tile.TileContext(nc) as tc, ExitStack() as ctx:
        wp = ctx.enter_context(tc.tile_pool(name="wpool", bufs=1))
        ap_ = ctx.enter_context(tc.tile_pool(name="act", bufs=2))
        st = ctx.enter_context(tc.tile_pool(name="stat", bufs=3))
        hp = ctx.enter_context(tc.tile_pool(name="ht", bufs=1))
        ps = ctx.enter_context(tc.tile_pool(name="psum", bufs=1, space="PSUM"))

        # ---- resident weights ----
        def wload(name, d_ap, kchunks, fdim, dt):
            t = wp.tile([128, kchunks, fdim], dt, tag=name)
            nc.sync.dma_start(t[:], d_ap.rearrange("(kc p) f -> p kc f", p=128))
            return t

        wq_sb = wload("wq", wq_d, KC, C, BF16)
        wk_sb = wload("wk", wk_d, KC, C, BF16)
        wv_sb = wload("wv", wv_d, KC, C, BF16)
        wp_sb = wload("wp", wp_d, KC, C, F32R)
        w1_sb = wload("w1", w1_d, KC, F, BF16)
        w2_sb = wload("w2", w2_d, FM, C, F32R)

        b1_sb = wp.tile([128, FM], F32, tag="b1")
        nc.sync.dma_start(b1_sb[:], b1_d.rearrange("(fm p) -> p fm", p=128))
        mask_sb = wp.tile([T, T], BF16, tag="mask")
        nc.sync.dma_start(mask_sb[:], mask_d)
        ident_sb = wp.tile([128, 128], BF16, tag="ident")
        nc.sync.dma_start(ident_sb[:], ident_d)
        eps_sb = wp.tile([128, 1], F32, tag="eps")
        nc.vector.memset(eps_sb[:], EPS)
        if has_bq:
            bq_sb = wp.tile([128, H], F32, tag="bq")
            nc.sync.dma_start(bq_sb[:], bq_d.rearrange("(h d) -> d h", d=128))
        if has_bk:
            bk_sb = wp.tile([128, H], F32, tag="bk")
            nc.sync.dma_start(bk_sb[:], bk_d.rearrange("(h d) -> d h", d=128))
        if has_bv:
            bv_sb = wp.tile([T, C], F32, tag="bv")
            nc.sync.dma_start(bv_sb[:], bv_d)
        if has_bp:
            bp_sb = wp.tile([128, C], F32, tag="bp")
            nc.sync.dma_start(bp_sb[:], bp_d)
        if has_b2:
            b2_sb = wp.tile([128, C], F32, tag="b2")
            nc.sync.dma_start(b2_sb[:], b2_d)

        # ---- per-block helpers ----
        def ln_stats_apply(src, pref, sums, sumsq):
            """Finish LN given per-chunk sums/sumsq [128, TCH]; apply on ACT."""
            mu = st.tile([128, TCH], F32, tag=pref + "mu")
            nc.vector.tensor_scalar_mul(mu[:], sums[:], 1.0 / C)
            msq = st.tile([128, TCH], F32, tag=pref + "msq")
            nc.vector.tensor_mul(out=msq[:], in0=mu[:], in1=mu[:])
            var = st.tile([128, TCH], F32, tag=pref + "var")
            nc.vector.scalar_tensor_tensor(var[:], sumsq[:], 1.0 / C, msq[:],
                                           OP.mult, OP.subtract)
            std = st.tile([128, TCH], F32, tag=pref + "std")
            nc.scalar.activation(std[:], var[:], AF.Sqrt, bias=eps_sb[:, 0:1])
            rstd = st.tile([128, TCH], F32, tag=pref + "rstd")
            nc.vector.reciprocal(rstd[:], std[:])
            nmr = st.tile([128, TCH], F32, tag=pref + "nmr")
            nc.vector.scalar_tensor_tensor(nmr[:], mu[:], -1.0, rstd[:],
                                           OP.mult, OP.mult)
            xn = ap_.tile([128, TCH, C], BF16, tag=pref + "xn")
            for i in range(TCH):
                nc.scalar.activation(xn[:, i, :], src[:, i, :], AF.Identity,
                                     scale=rstd[:, i : i + 1],
                                     bias=nmr[:, i : i + 1])
            return xn

        def layer_norm(src, pref):
            """src: [128, TCH, C] f32 -> xn bf16 [128, TCH, C]."""
            sums = st.tile([128, TCH], F32, tag=pref + "sums")
            sumsq = st.tile([128, TCH], F32, tag=pref + "sumsq")
            nc.vector.tensor_reduce(sums[:], src[:], axis=mybir.AxisListType.X,
                                    op=OP.add)
            for i in range(TCH):
                scr = st.tile([128, C], BF16, tag="scr", bufs=2)
                nc.vector.scalar_tensor_tensor(
                    scr[:], src[:, i, :], 1.0, src[:, i, :], OP.mult, OP.mult,
                    accum_out=sumsq[:, i : i + 1])
            return ln_stats_apply(src, pref, sums, sumsq)

        def transpose_xn(xn, pref, ptag, dma=False):
            """Transpose LN output to T-layout (PE bf16, or DMA xbar)."""
            xnT = ap_.tile([128, KC, TOK], BF16, tag=pref + "xnT")
            if dma:
                for kc in range(KC):
                    for mc in range(TCH):
                        nc.sync.dma_start_transpose(
                            out=xnT[:, kc, mc * 128 : (mc + 1) * 128],
                            in_=xn[:, mc, kc * 128 : (kc + 1) * 128])
                return xnT
            for kc in range(KC):
                p = ps.tile([128, TCH, 128], BF16, tag=ptag, bufs=4, name="txp")
                for mc in range(TCH):
                    nc.tensor.transpose(p[:, mc, :],
                                        xn[:, mc, kc * 128 : (kc + 1) * 128],
                                        ident_sb[:])
                if kc % 2 == 0:
                    nc.scalar.activation(xnT[:, kc, :], p[:], AF.Identity)
                else:
                    nc.vector.tensor_copy(out=xnT[:, kc, :], in_=p[:])
            return xnT

        # ---- block stages ----
        def stage_a1(blk):
            """x load, LN1, transpose -> (x_sb, xnT)."""
            row0 = blk * TOK
            x_sb = ap_.tile([128, TCH, C], F32, tag="x", bufs=3)
            nc.sync.dma_start(
                x_sb[:],
                x_d[row0 : row0 + TOK, :].rearrange("(ch p) c -> p ch c", p=128))
            xn = layer_norm(x_sb, "a")
            xnT = transpose_xn(xn, "a", "pa", dma=True)
            return x_sb, xnT

        def stage_a2(blk, xnT):
            """QKV + attention -> ot."""
            # QKV projections (bf16)
            qt = ap_.tile([128, H, TOK], BF16, tag="qt")
            kt = ap_.tile([128, H, TOK], BF16, tag="kt")
            for dst, w_sb, bias_sb in ((qt, wq_sb, bq_sb if has_bq else None),
                                       (kt, wk_sb, bk_sb if has_bk else None)):
                for h in range(H):
                    p = ps.tile([128, TOK], F32, tag="pa", bufs=4)
                    for kc in range(KC):
                        nc.tensor.matmul(p[:], w_sb[:, kc, h * 128 : (h + 1) * 128],
                                         xnT[:, kc, :], start=(kc == 0),
                                         stop=(kc == KC - 1))
                    if bias_sb is not None:
                        nc.scalar.activation(dst[:, h, :], p[:], AF.Identity,
                                             bias=bias_sb[:, h : h + 1])
                    else:
                        nc.vector.tensor_copy(out=dst[:, h, :], in_=p[:])
            vt = ap_.tile([T, NB, C], BF16, tag="vt")
            for b in range(NB):
                p = ps.tile([T, C], F32, tag="pa", bufs=4)
                for kc in range(KC):
                    nc.tensor.matmul(p[:], xnT[:, kc, b * T : (b + 1) * T],
                                     wv_sb[:, kc, :], start=(kc == 0),
                                     stop=(kc == KC - 1))
                if has_bv:
                    nc.vector.tensor_add(out=vt[:, b, :], in0=p[:], in1=bv_sb[:])
                else:
                    nc.scalar.activation(vt[:, b, :], p[:], AF.Identity)

            # attention: scores [t, s] per (h, b), exp, mask, row-normalize
            ee = ap_.tile([T, H * NB, T], BF16, tag="ee")
            for h in range(H):
                p = ps.tile([T, NB, T], F32, tag="pa", bufs=4)
                for b in range(NB):
                    nc.tensor.matmul(p[:, b, :], qt[:, h, b * T : (b + 1) * T],
                                     kt[:, h, b * T : (b + 1) * T],
                                     start=True, stop=True)
                nc.scalar.activation(ee[:, h * NB : (h + 1) * NB, :], p[:],
                                     AF.Exp, scale=SCALE)
            nc.gpsimd.tensor_mul(
                out=ee[:], in0=ee[:],
                in1=mask_sb[:].unsqueeze(1).to_broadcast([T, H * NB, T]))
            dsum = st.tile([T, H * NB], F32, tag="dsum")
            nc.vector.tensor_reduce(dsum[:], ee[:], axis=mybir.AxisListType.X,
                                    op=OP.add)
            rr = st.tile([T, H * NB], F32, tag="rr")
            nc.vector.reciprocal(rr[:], dsum[:])
            nc.gpsimd.tensor_mul(
                out=ee[:], in0=ee[:],
                in1=rr[:].unsqueeze(2).to_broadcast([T, H * NB, T]))
            return vt, ee

        def stage_a2b(blk, vt, ee):
            """probs transpose + attn @ V -> ot (T-layout, f32r)."""
            pt = ee  # probs are overwritten in place by their transpose
            for h in range(H):
                p = ps.tile([T, NB, T], BF16, tag="pa", bufs=4)
                for b in range(NB):
                    nc.tensor.transpose(p[:, b, :], ee[:, h * NB + b, :],
                                        ident_sb[:T, :T])
                nc.vector.tensor_copy(out=pt[:, h * NB : (h + 1) * NB, :], in_=p[:])
            ot = ap_.tile([128, H, TOK], F32R, tag="ot")
            for h in range(H):
                p = ps.tile([128, NB, T], F32, tag="pa", bufs=4)
                for b in range(NB):
                    nc.tensor.matmul(p[:, b, :], vt[:, b, h * 128 : (h + 1) * 128],
                                     pt[:, h * NB + b, :], start=True, stop=True)
                nc.vector.tensor_copy(out=ot[:, h, :], in_=p[:])
            return ot

        def stage_b(blk, x_sb, ot):
            """proj + residual, LN2, MLP, store."""
            row0 = blk * TOK
            x2 = ap_.tile([128, TCH, C], F32, tag="x2")
            sums2 = st.tile([128, TCH], F32, tag="bsums")
            sumsq2 = st.tile([128, TCH], F32, tag="bsumsq")
            for mc in range(TCH):
                p = ps.tile([128, C], F32, tag="pb", bufs=4)
                for kc in range(H):
                    nc.tensor.matmul(p[:], ot[:, kc, mc * 128 : (mc + 1) * 128],
                                     wp_sb[:, kc, :], start=(kc == 0),
                                     stop=(kc == H - 1))
                if has_bp:
                    nc.vector.tensor_add(out=p[:], in0=p[:], in1=bp_sb[:])
                # x2 = sa + x, with the LN2 row-sum accumulated for free
                nc.vector.scalar_tensor_tensor(
                    x2[:, mc, :], p[:], 1.0, x_sb[:, mc, :], OP.mult, OP.add,
                    accum_out=sums2[:, mc : mc + 1])
                scr2 = st.tile([128, C], BF16, tag="scr2", bufs=2)
                nc.vector.scalar_tensor_tensor(
                    scr2[:], x2[:, mc, :], 1.0, x2[:, mc, :], OP.mult, OP.mult,
                    accum_out=sumsq2[:, mc : mc + 1])

            # MLP
            xn2 = ln_stats_apply(x2, "b", sums2, sumsq2)
            xn2T = transpose_xn(xn2, "b", "pb")
            ht = hp.tile([128, FM, TOK], F32R, tag="ht")
            for fm in range(FM):
                p = ps.tile([128, TOK], F32, tag="pb", bufs=4)
                for kc in range(KC):
                    nc.tensor.matmul(p[:], w1_sb[:, kc, fm * 128 : (fm + 1) * 128],
                                     xn2T[:, kc, :], start=(kc == 0),
                                     stop=(kc == KC - 1))
                nc.scalar.activation(ht[:, fm, :], p[:], AF.Gelu,
                                     bias=b1_sb[:, fm : fm + 1])
            xo = ap_.tile([128, TCH, C], F32, tag="xo")
            for mc in range(TCH):
                p = ps.tile([128, C], F32, tag="pb", bufs=4)
                for fk in range(FM):
                    nc.tensor.matmul(p[:], ht[:, fk, mc * 128 : (mc + 1) * 128],
                                     w2_sb[:, fk, :], start=(fk == 0),
                                     stop=(fk == FM - 1))
                if has_b2:
                    nc.vector.tensor_add(out=p[:], in0=p[:], in1=b2_sb[:])
                nc.vector.tensor_add(out=xo[:, mc, :], in0=p[:],
                                     in1=x2[:, mc, :])
            nc.sync.dma_start(
                y_d[row0 : row0 + TOK, :].rearrange("(ch p) c -> p ch c", p=128),
                xo[:])

        # Software-pipelined emission. Per-engine FIFO order interleaves the
        # stages so the next blocks' independent work is queued ahead of the
        # current block's dependency stalls: A1 runs two blocks ahead, the
        # attention front (QKV+scores+softmax issue) one block ahead, and the
        # attention tail (probs transpose + av) is emitted after the previous
        # block's MLP so the softmax latency hides behind it.
        xs, xnTs, sm, ots = {}, {}, {}, {}
        xs[0], xnTs[0] = stage_a1(0)
        if nblk > 1:
            xs[1], xnTs[1] = stage_a1(1)
        sm[0] = stage_a2(0, xnTs.pop(0))
        ots[0] = stage_a2b(0, *sm.pop(0))
        for blk in range(1, nblk):
            if blk + 1 < nblk:
                xs[blk + 1], xnTs[blk + 1] = stage_a1(blk + 1)
            sm[blk] = stage_a2(blk, xnTs.pop(blk))
            stage_b(blk - 1, xs.pop(blk - 1), ots.pop(blk - 1))
            ots[blk] = stage_a2b(blk, *sm.pop(blk))
        stage_b(nblk - 1, xs.pop(nblk - 1), ots.pop(nblk - 1))

    nc.compile()
    return nc


def fold(inputs):
    """Host-side exact folding of LN affines and biases into weights.

    Returns dict of staged arrays for the device program + bias flags.
    """
    f32 = np.float32
    g1 = inputs["g1"].astype(f32)
    be1 = inputs["be1"].astype(f32)
    g2 = inputs["g2"].astype(f32)
    be2 = inputs["be2"].astype(f32)

    def headcat(w):  # [H, C, D] -> [C, H*D]
        return np.concatenate([w[h] for h in range(H)], axis=1)

    wq = headcat(np.asarray(inputs["wq"], f32))
    wk = headcat(np.asarray(inputs["wk"], f32))
    wv = headcat(np.asarray(inputs["wv"], f32))
    wp_ = np.asarray(inputs["w_proj"], f32)
    w1 = np.asarray(inputs["w1"], f32)
    w2 = np.asarray(inputs["w2"], f32)

    wq_f = g1[:, None] * wq
    wk_f = g1[:, None] * wk
    wv_f = g1[:, None] * wv
    bq = be1 @ wq
    bk = be1 @ wk
    bv = be1 @ wv
    bp = np.asarray(inputs["b_proj"], f32)
    w1_f = g2[:, None] * w1
    b1 = np.asarray(inputs["b1"], f32) + be2 @ w1
    b2 = np.asarray(inputs["b2"], f32)

    mask = np.tril(np.ones((T, T), np.float32)).astype(ml_dtypes.bfloat16)
    ident = np.eye(128, dtype=ml_dtypes.bfloat16)

    staged = {
        "wq": wq_f.astype(ml_dtypes.bfloat16),
        "wk": wk_f.astype(ml_dtypes.bfloat16),
        "wv": wv_f.astype(ml_dtypes.bfloat16),
        "wp": wp_.astype(f32),
        "w1": w1_f.astype(ml_dtypes.bfloat16),
        "w2": w2.astype(f32),
        "b1": b1,
        "mask": mask,
        "ident": ident,
    }
    flags = {
        "has_bq": bool(np.any(bq)),
        "has_bk": bool(np.any(bk)),
        "has_bv": bool(np.any(bv)),
        "has_bp": bool(np.any(bp)),
        "has_b2": bool(np.any(b2)),
    }
    if flags["has_bq"]:
        staged["bq"] = bq
    if flags["has_bk"]:
        staged["bk"] = bk
    if flags["has_bv"]:
        staged["bv_b"] = np.broadcast_to(bv, (T, C)).copy()
    if flags["has_bp"]:
        staged["bp_b"] = np.broadcast_to(bp, (128, C)).copy()
    if flags["has_b2"]:
        staged["b2_b"] = np.broadcast_to(b2, (128, C)).copy()
    return staged, flags


_CACHE = {}


def kernel(**inputs):
    staged, flags = fold(inputs)
    key = tuple(sorted(flags.items()))
    if key not in _CACHE:
        _CACHE[key] = build(**flags)
    nc = _CACHE[key]

    x = np.asarray(inputs["x"], np.float32).reshape(B, T * C)
    in_maps = []
    for c in range(NCORES):
        m = dict(staged)
        m["x"] = x[c * SEQ_PER_CORE : (c + 1) * SEQ_PER_CORE].reshape(S, C)
        in_maps.append(m)

    res = bass_utils.run_bass_kernel_spmd(nc, in_maps, core_ids=list(range(NCORES)))
    out = np.concatenate([r["y"] for r in res.results], axis=0)
    return out.reshape(B, T, C).astype(np.float32)
